# revision 1
# baseline (speedup 1.0000x reference)
"""Trainium2 Bass kernel for nn_CNN2LWithRPE (transformer layer + CNN head).

Sharding: data-parallel over batch across 8 NeuronCores (2 batch rows each).
All parameters replicated. The only cross-core communication is two tiny
AllReduces for the training-mode BatchNorm statistics.

Per-core layout (B_loc batches, T = B_loc*L tokens):
  - activations transposed in SBUF: xT/qT/x1T... are [D=128 part, T free]
  - attention as scores^T tiles [keys=128 part, queries=512 free]:
    QK^T row-packed 4x on PE (K=32) from a shuffled kTp layout + a 4x
    replicated qrep tile; exp on ACT as wide [128,2048] ops with the
    clipped-RPE bias folded into a per-group bias constant (c_lo/c_hi)
    plus narrow host-precomputed banded correction tiles added on DVE;
    PV uses v in natural layout with an appended ones-column so the
    softmax denominator falls out of the same matmul.
  - layernorm in transposed layout: partition stats via ones-matmul,
    rstd = Exp(-0.5*Ln(var+eps)), rank-1 K=1 matmul broadcast back.
  - conv1d as K accumulating shifted matmuls; BN apply fused into
    ACT Relu(scale*x+shift) with per-channel scale/shift APs.
"""

import numpy as np

B, L = 16, 2048
NCAT, ED = 25, 120
D, H, HD = 128, 4, 32
FF = 256
MD = 32
C1, C2, K = 128, 256, 5
NC = 2
EPS = 1e-5
NCORES = 8
BLOC = B // NCORES
ISQ = float(1.0 / np.sqrt(HD))

QT = 512
KTILE = 128

BAND_DELTAS = [-128, 0, 128, 256, 384, 512]
BAND_W = [32, 160, 288, 416, 512, 32]
BAND_C0 = [0, 0, 0, 0, 0, 480]
BAND_OFF = [0, 32, 192, 480, 896, 1408]
BAND_TOT = 1440


def _build(n_cores, bloc, lp, dbg=False):
    import contextlib
    import concourse.bass as bass
    import concourse.tile as tile
    from concourse import bacc, mybir

    f32 = mybir.dt.float32
    bf16 = mybir.dt.bfloat16
    AF = mybir.ActivationFunctionType
    OP = mybir.AluOpType
    AX = mybir.AxisListType

    T = bloc * lp
    NET = T // 512
    NQT = lp // QT
    NKT = lp // KTILE
    NG = NKT // 4
    NLT = lp // 512
    NDEN = bloc * NQT * H
    LP2 = lp // 2
    L2 = LP2 - (K - 1)
    LT2 = L2 // 2
    n1 = float(n_cores * bloc * lp)
    n2 = float(n_cores * bloc * L2)

    nc = bacc.Bacc("TRN2", target_bir_lowering=False, debug=False,
                   num_devices=n_cores)

    def din(name, shape):
        return nc.dram_tensor(name, list(shape), f32, kind="ExternalInput")

    Xf = din("Xf", [T])
    saf = din("saf", [T])
    ptmf = din("ptmf", [T])
    emb_d = din("embp", [NCAT, D])
    pemb_d = din("pembp", [10, D])
    iota25_d = din("iota25", [NCAT, 1])
    iota10_d = din("iota10", [10, 1])
    inwT_d = din("inwT", [D, 3 * D])
    qb_d = din("qb", [D, 1])
    kb_d = din("kb", [D, 1])
    woT_d = din("woT", [D, D])
    ob_d = din("ob_eff", [D, 1])
    l1wT_d = din("l1wT", [D, FF])
    l1b_d = din("l1b", [D, 2])
    l2cat_d = din("l2cat", [D, 2 * D])
    l2b_d = din("l2b", [D, 1])
    ln1g_d = din("ln1g", [D, 1])
    ln1b_d = din("ln1b", [D, 1])
    ln2g_d = din("ln2g", [D, 1])
    ln2b_d = din("ln2b", [D, 1])
    band_d = din("bandcat", [D, H * BAND_TOT])
    cexp_d = din("cexp", [D, 2 * H])
    c1wT_d = din("c1wT", [D, K * C1])
    c2wT_d = din("c2wT", [C1, K * C2])
    bn1g_d = din("bn1g", [C1, 1])
    bn1b_d = din("bn1b", [C1, 1])
    bn2g_d = din("bn2g", [C1, 2])
    bn2b_d = din("bn2b", [C1, 2])
    fcwT_d = din("fcwT", [C1, 2 * NC])
    fcb_d = din("fcb", [NC, 1])

    out_d = nc.dram_tensor("out", [bloc, NC], f32, kind="ExternalOutput")
    dbg_outs = {}
    if dbg:
        for nm, shp in [("dbg_xT", [D, T]), ("dbg_qT", [D, T]),
                        ("dbg_kT", [D, T]), ("dbg_attnT", [D, T]),
                        ("dbg_x1T", [D, T]), ("dbg_x2", [D, T]),
                        ("dbg_c1", [C1, bloc * lp]),
                        ("dbg_p1", [C1, bloc * LP2]),
                        ("dbg_feat", [C1, 2 * bloc])]:
            dbg_outs[nm] = nc.dram_tensor(nm, shp, f32, kind="ExternalOutput")

    def bc(ap1d, parts):
        return bass.AP(tensor=ap1d.tensor, offset=ap1d.offset,
                       ap=[[0, parts]] + [list(p) for p in ap1d.ap])

    with tile.TileContext(nc) as tc:
        ctx = contextlib.ExitStack()
        with ctx:
            pp = ctx.enter_context(tc.tile_pool(name="params", bufs=1))
            big = ctx.enter_context(tc.tile_pool(name="big", bufs=1))
            wk = ctx.enter_context(tc.tile_pool(name="wk", bufs=12))
            wkb = ctx.enter_context(tc.tile_pool(name="wkb", bufs=2))
            longs = ctx.enter_context(tc.tile_pool(name="longs", bufs=1))
            mid = ctx.enter_context(tc.tile_pool(name="mid", bufs=6))
            rowp = ctx.enter_context(tc.tile_pool(name="rows", bufs=4))
            ptp = ctx.enter_context(tc.tile_pool(name="pt", bufs=2))
            qrp = ctx.enter_context(tc.tile_pool(name="qrep", bufs=1))
            ps_sc = ctx.enter_context(tc.tile_pool(name="ps_sc", bufs=1, space="PSUM"))
            ps_sm = ctx.enter_context(tc.tile_pool(name="ps_sm", bufs=4, space="PSUM"))
            dram = ctx.enter_context(tc.tile_pool(name="dram", bufs=1, space="DRAM"))

            def wkt(p=D, f=512):
                return wk.tile([p, f], f32, tag="wk", name="wkt")

            def midt(p, f):
                return mid.tile([p, f], f32, tag="mid", name="midt")

            # ---- params ----
            def ld(dten, shape):
                t = pp.tile(list(shape), f32, tag=dten.name, name=dten.name + "_s")
                nc.sync.dma_start(out=t, in_=dten[tuple(slice(0, s) for s in shape)])
                return t

            emb_s = ld(emb_d, [NCAT, D])
            pemb_s = ld(pemb_d, [10, D])
            io25 = ld(iota25_d, [NCAT, 1])
            io10 = ld(iota10_d, [10, 1])
            inwT = ld(inwT_d, [D, 3 * D])
            qb_s = ld(qb_d, [D, 1])
            kb_s = ld(kb_d, [D, 1])
            woT = ld(woT_d, [D, D])
            ob_s = ld(ob_d, [D, 1])
            l1wT = ld(l1wT_d, [D, FF])
            l1b_s = ld(l1b_d, [D, 2])
            l2cat = ld(l2cat_d, [D, 2 * D])
            l2b_s = ld(l2b_d, [D, 1])
            ln1g = ld(ln1g_d, [D, 1]); ln1b = ld(ln1b_d, [D, 1])
            ln2g = ld(ln2g_d, [D, 1]); ln2b = ld(ln2b_d, [D, 1])
            cexp = ld(cexp_d, [D, 2 * H])
            c1wT = ld(c1wT_d, [D, K * C1])
            c2wT = ld(c2wT_d, [C1, K * C2])
            bn1g = ld(bn1g_d, [C1, 1]); bn1b = ld(bn1b_d, [C1, 1])
            bn2g = ld(bn2g_d, [C1, 2]); bn2b = ld(bn2b_d, [C1, 2])
            fcwT = ld(fcwT_d, [C1, 2 * NC])
            fcb_s = ld(fcb_d, [NC, 1])

            band = big.tile([D, H * BAND_TOT], f32, tag="chF")
            nc.sync.dma_start(out=band, in_=band_d[:, :])

            ones128 = pp.tile([D, 1], f32, tag="ones128")
            nc.vector.memset(ones128, 1.0)
            ones_r = pp.tile([1, D], f32, tag="ones_r")
            nc.vector.memset(ones_r, 1.0)
            ones32 = pp.tile([1, HD], f32, tag="ones32")
            nc.vector.memset(ones32, 1.0)
            epsNLT = pp.tile([NLT, 1], f32, tag="epsNLT")
            nc.vector.memset(epsNLT, EPS)
            eps128 = pp.tile([D, 1], f32, tag="eps128")
            nc.vector.memset(eps128, EPS)

            # ---- persistent activations ----
            xT = big.tile([D, T], f32, tag="chA")
            qT = big.tile([D, T], f32, tag="chE")
            kTp = big.tile([D, bloc, H, NG, KTILE], f32, tag="chB")
            v_sb = big.tile([D, T // KTILE, H, HD + 1], bf16, tag="chC")
            attnT = big.tile([D, T], f32, tag="chD")
            x1T = big.tile([D, T], f32, tag="chG")
            nc.vector.memset(v_sb[:, :, :, HD:HD + 1], 1.0)

            # ================= embedding =================
            for e in range(NET):
                sl = slice(e * 512, (e + 1) * 512)
                xb = wkt(NCAT)
                nc.sync.dma_start(out=xb, in_=bc(Xf[sl], NCAT))
                sb_ = wkt(NCAT)
                nc.sync.dma_start(out=sb_, in_=bc(saf[sl], NCAT))
                oh = wkt(NCAT)
                nc.vector.tensor_scalar(out=oh, in0=xb, scalar1=io25,
                                        scalar2=None, op0=OP.is_equal)
                nc.vector.tensor_mul(oh, oh, sb_)
                pb = wkt(10)
                nc.sync.dma_start(out=pb, in_=bc(ptmf[sl], 10))
                ohp = wkt(10)
                nc.vector.tensor_scalar(out=ohp, in0=pb, scalar1=io10,
                                        scalar2=None, op0=OP.is_equal)
                pe = ps_sm.tile([D, 512], f32, tag="sm")
                nc.tensor.matmul(pe, pemb_s, ohp, start=True, stop=False)
                nc.tensor.matmul(pe, emb_s, oh, start=False, stop=True)
                nc.vector.tensor_copy(xT[:, sl], pe)

            if dbg:
                nc.sync.dma_start(out=dbg_outs["dbg_xT"][:, :], in_=xT)

            # ================= qkv =================
            for e in range(NET):
                sl = slice(e * 512, (e + 1) * 512)
                pq = ps_sm.tile([D, 512], f32, tag="sm")
                nc.tensor.matmul(pq, inwT[:, 0:D], xT[:, sl], start=True, stop=True)
                nc.vector.tensor_scalar(out=qT[:, sl], in0=pq, scalar1=ISQ,
                                        scalar2=qb_s, op0=OP.mult, op1=OP.add)
                pk = ps_sm.tile([D, 512], f32, tag="sm")
                nc.tensor.matmul(pk, inwT[:, D:2 * D], xT[:, sl], start=True, stop=True)
                ktmp = midt(D, 512)
                nc.vector.tensor_scalar(out=ktmp, in0=pk, scalar1=kb_s,
                                        scalar2=None, op0=OP.add)
                b_ = (e * 512) // lp
                for h in range(H):
                    for sub in range(4):
                        ktb = ((e * 512) % lp) // KTILE + sub
                        nc.sync.dma_start(
                            out=kTp[32 * (ktb % 4):32 * (ktb % 4) + 32,
                                    b_, h, ktb // 4, :],
                            in_=ktmp[32 * h:32 * h + 32,
                                     sub * KTILE:(sub + 1) * KTILE])
                for sub in range(4):
                    tt = (e * 512) // KTILE + sub
                    pv = ps_sm.tile([KTILE, D], f32, tag="sm")
                    nc.tensor.matmul(pv, xT[:, e * 512 + sub * KTILE:
                                            e * 512 + (sub + 1) * KTILE],
                                     inwT[:, 2 * D:3 * D], start=True, stop=True)
                    nc.vector.tensor_copy(
                        v_sb[:, tt, :, 0:HD],
                        pv.rearrange("p (h d) -> p h d", h=H))

            if dbg:
                nc.sync.dma_start(out=dbg_outs["dbg_qT"][:, :], in_=qT)
                kT_dbg = big.tile([D, T], f32, tag="chDBG")
                for b_ in range(bloc):
                    for h in range(H):
                        for g in range(NG):
                            for r in range(4):
                                kt = 4 * g + r
                                nc.sync.dma_start(
                                    out=kT_dbg[32 * h:32 * h + 32,
                                               b_ * lp + kt * KTILE:
                                               b_ * lp + (kt + 1) * KTILE],
                                    in_=kTp[32 * r:32 * r + 32, b_, h, g, :])
                nc.sync.dma_start(out=dbg_outs["dbg_kT"][:, :], in_=kT_dbg)

            # ================= attention =================
            den32 = longs.tile([NDEN, 512], f32, tag="den32")
            dmap = {(-1, 3): 0, (0, 0): 1, (0, 1): 2, (0, 2): 3, (0, 3): 4, (1, 0): 5}
            for b_ in range(bloc):
                for h in range(H):
                    qrep = qrp.tile([D, lp], f32, tag="qr")
                    for r in range(4):
                        nc.sync.dma_start(
                            out=qrep[32 * r:32 * r + 32, :],
                            in_=qT[32 * h:32 * h + 32, b_ * lp:(b_ + 1) * lp])
                    for qt in range(NQT):
                        ppv = ps_sm.tile([HD + 1, 512], f32, tag="sm")
                        for g in range(NG):
                            sc = ps_sc.tile([D, 4 * 512], f32, tag="sc")
                            for q in range(4):
                                nc.tensor.matmul(
                                    sc[:, q * 512:(q + 1) * 512],
                                    kTp[32 * q:32 * q + 32, b_, h, g, :],
                                    qrep[32 * q:32 * q + 32, qt * QT:(qt + 1) * QT],
                                    start=True, stop=True,
                                    tile_position=(32 * q, 0))
                            for q in range(4):
                                di = dmap.get((g - qt, q))
                                if di is not None:
                                    c0, w = BAND_C0[di], BAND_W[di]
                                    nc.vector.tensor_tensor(
                                        out=sc[:, q * 512 + c0:q * 512 + c0 + w],
                                        in0=sc[:, q * 512 + c0:q * 512 + c0 + w],
                                        in1=band[:, h * BAND_TOT + BAND_OFF[di]:
                                                 h * BAND_TOT + BAND_OFF[di] + w],
                                        op=OP.add)
                            side = 0 if g <= qt else 1
                            pt = ptp.tile([D, 4 * 512], bf16, tag="pt")
                            nc.scalar.activation(pt, sc, AF.Exp,
                                                 bias=cexp[:, 2 * h + side:
                                                           2 * h + side + 1],
                                                 scale=1.0)
                            for q in range(4):
                                kt = 4 * g + q
                                nc.tensor.matmul(
                                    ppv, v_sb[:, b_ * NKT + kt, h, :],
                                    pt[:, q * 512:(q + 1) * 512],
                                    start=(kt == 0), stop=(kt == NKT - 1))
                        pv_sb = wk.tile([HD + 1, 512], f32, tag="wk", name="pv_sb")
                        nc.vector.tensor_copy(pv_sb, ppv)
                        nc.sync.dma_start(
                            out=attnT[32 * h:32 * h + 32,
                                      b_ * lp + qt * QT:b_ * lp + (qt + 1) * QT],
                            in_=pv_sb[0:HD, :])
                        nc.sync.dma_start(
                            out=den32[(b_ * NQT + qt) * H + h:
                                      (b_ * NQT + qt) * H + h + 1, :],
                            in_=pv_sb[HD:HD + 1, :])

            lnden = midt(NDEN, 512)
            nc.scalar.activation(lnden, den32, AF.Ln, bias=0.0, scale=1.0)
            recip = longs.tile([NDEN, 512], f32, tag="recip")
            nc.scalar.activation(recip, lnden, AF.Exp, bias=0.0, scale=-1.0)

            for b_ in range(bloc):
                for qt in range(NQT):
                    bcp = ps_sm.tile([D, 512], f32, tag="sm")
                    for h in range(H):
                        rr = rowp.tile([1, 512], f32, tag="row")
                        nc.sync.dma_start(
                            out=rr, in_=recip[(b_ * NQT + qt) * H + h:
                                              (b_ * NQT + qt) * H + h + 1, :])
                        nc.tensor.matmul(bcp[32 * h:32 * h + 32, :], ones32, rr,
                                         start=True, stop=True,
                                         tile_position=(0, 32 * h))
                    sl = slice(b_ * lp + qt * QT, b_ * lp + (qt + 1) * QT)
                    nc.vector.tensor_mul(attnT[:, sl], attnT[:, sl], bcp)

            if dbg:
                nc.sync.dma_start(out=dbg_outs["dbg_attnT"][:, :], in_=attnT)

            # ======== layernorm helper (transposed layout) ========
            def layernorm_T(src, dst_fn, g_s, b_s, b_base):
                s1c = midt(NLT, 512)
                s2c = midt(NLT, 512)
                for t_ in range(NLT):
                    sl = slice(b_base * lp + t_ * 512, b_base * lp + (t_ + 1) * 512)
                    sq = wkt()
                    nc.vector.tensor_mul(sq, src[:, sl], src[:, sl])
                    p1_ = ps_sm.tile([1, 512], f32, tag="sm")
                    nc.tensor.matmul(p1_, ones128, src[:, sl], start=True, stop=True)
                    p2_ = ps_sm.tile([1, 512], f32, tag="sm")
                    nc.tensor.matmul(p2_, ones128, sq, start=True, stop=True)
                    s1t = rowp.tile([1, 512], f32, tag="row", name="s1t")
                    nc.vector.tensor_copy(s1t, p1_)
                    s2t = rowp.tile([1, 512], f32, tag="row", name="s2t")
                    nc.vector.tensor_copy(s2t, p2_)
                    nc.sync.dma_start(out=s1c[t_:t_ + 1, :], in_=s1t)
                    nc.sync.dma_start(out=s2c[t_:t_ + 1, :], in_=s2t)
                m_ = wkt(NLT)
                nc.vector.tensor_scalar(out=m_, in0=s1c, scalar1=1.0 / D,
                                        scalar2=None, op0=OP.mult)
                var = wkt(NLT)
                nc.vector.tensor_scalar(out=var, in0=s2c, scalar1=1.0 / D,
                                        scalar2=None, op0=OP.mult)
                msq = wkt(NLT)
                nc.vector.tensor_mul(msq, m_, m_)
                nc.vector.tensor_tensor(out=var, in0=var, in1=msq, op=OP.subtract)
                lnv = wkt(NLT)
                nc.scalar.activation(lnv, var, AF.Ln, bias=epsNLT[0:NLT, :],
                                     scale=1.0)
                rstd = wkt(NLT)
                nc.scalar.activation(rstd, lnv, AF.Exp, bias=0.0, scale=-0.5)
                mr = wkt(NLT)
                nc.vector.tensor_mul(mr, m_, rstd)
                for t_ in range(NLT):
                    sl = slice(b_base * lp + t_ * 512, b_base * lp + (t_ + 1) * 512)
                    rr = rowp.tile([1, 512], f32, tag="row")
                    nc.sync.dma_start(out=rr, in_=rstd[t_:t_ + 1, :])
                    rm = rowp.tile([1, 512], f32, tag="row")
                    nc.sync.dma_start(out=rm, in_=mr[t_:t_ + 1, :])
                    br = ps_sm.tile([D, 512], f32, tag="sm")
                    nc.tensor.matmul(br, ones_r, rr, start=True, stop=True)
                    bm = ps_sm.tile([D, 512], f32, tag="sm")
                    nc.tensor.matmul(bm, ones_r, rm, start=True, stop=True)
                    tmp = wkt()
                    nc.vector.tensor_mul(tmp, src[:, sl], br)
                    nc.vector.tensor_tensor(out=tmp, in0=tmp, in1=bm,
                                            op=OP.subtract)
                    nc.vector.tensor_scalar(out=dst_fn(t_), in0=tmp, scalar1=g_s,
                                            scalar2=b_s, op0=OP.mult, op1=OP.add)

            # ================= out-proj + residual + LN1 =================
            r1T = big.tile([D, T], f32, tag="chB")   # after kTp's last read
            for b_ in range(bloc):
                for qt in range(NQT):
                    sl = slice(b_ * lp + qt * QT, b_ * lp + (qt + 1) * QT)
                    po = ps_sm.tile([D, 512], f32, tag="sm")
                    nc.tensor.matmul(po, woT, attnT[:, sl], start=True, stop=True)
                    nc.vector.tensor_scalar(out=r1T[:, sl], in0=po, scalar1=ob_s,
                                            scalar2=None, op0=OP.add)
                    nc.vector.tensor_tensor(out=r1T[:, sl], in0=r1T[:, sl],
                                            in1=xT[:, sl], op=OP.add)
            for b_ in range(bloc):
                layernorm_T(
                    r1T,
                    lambda t_, b0=b_: x1T[:, b0 * lp + t_ * 512:
                                          b0 * lp + (t_ + 1) * 512],
                    ln1g, ln1b, b_)

            if dbg:
                nc.sync.dma_start(out=dbg_outs["dbg_x1T"][:, :], in_=x1T)

            # ================= FFN + residual + LN2 =================
            x2pad = big.tile([D, bloc * (lp + 4)], f32, tag="chA")  # after xT
            nc.vector.memset(x2pad[:, :], 0.0)
            r2T = big.tile([D, T], f32, tag="chC")                  # after v_sb
            for b_ in range(bloc):
                for qt in range(NQT):
                    sl = slice(b_ * lp + qt * QT, b_ * lp + (qt + 1) * QT)
                    h1a = wkt()
                    h1b = wkt()
                    for half, dest in ((0, h1a), (1, h1b)):
                        ph = ps_sm.tile([D, 512], f32, tag="sm")
                        nc.tensor.matmul(ph, l1wT[:, half * D:(half + 1) * D],
                                         x1T[:, sl], start=True, stop=True)
                        nc.scalar.activation(dest, ph, AF.Relu,
                                             bias=l1b_s[:, half:half + 1],
                                             scale=1.0)
                    py = ps_sm.tile([D, 512], f32, tag="sm")
                    nc.tensor.matmul(py, l2cat[:, 0:D], h1a, start=True, stop=False)
                    nc.tensor.matmul(py, l2cat[:, D:2 * D], h1b,
                                     start=False, stop=True)
                    nc.vector.tensor_scalar(out=r2T[:, sl], in0=py, scalar1=l2b_s,
                                            scalar2=None, op0=OP.add)
                    nc.vector.tensor_tensor(out=r2T[:, sl], in0=r2T[:, sl],
                                            in1=x1T[:, sl], op=OP.add)
            for b_ in range(bloc):
                layernorm_T(
                    r2T,
                    lambda t_, b0=b_: x2pad[:, b0 * (lp + 4) + 2 + t_ * 512:
                                            b0 * (lp + 4) + 2 + (t_ + 1) * 512],
                    ln2g, ln2b, b_)

            if dbg:
                for b_ in range(bloc):
                    nc.sync.dma_start(
                        out=dbg_outs["dbg_x2"][:, b_ * lp:(b_ + 1) * lp],
                        in_=x2pad[:, b_ * (lp + 4) + 2:b_ * (lp + 4) + 2 + lp])

            # ================= conv1 + bn1 =================
            c1_sb = big.tile([C1, bloc * lp], f32, tag="chD")   # after attnT
            bnst1 = longs.tile([C1, bloc * NLT, 6], f32, tag="bnst1")
            for b_ in range(bloc):
                for t_ in range(NLT):
                    pc = ps_sm.tile([C1, 512], f32, tag="sm")
                    for k_ in range(K):
                        nc.tensor.matmul(
                            pc, c1wT[:, k_ * C1:(k_ + 1) * C1],
                            x2pad[:, b_ * (lp + 4) + t_ * 512 + k_:
                                  b_ * (lp + 4) + t_ * 512 + k_ + 512],
                            start=(k_ == 0), stop=(k_ == K - 1))
                    nc.vector.bn_stats(out=bnst1[:, b_ * NLT + t_, :], in_=pc)
                    nc.vector.tensor_copy(
                        c1_sb[:, b_ * lp + t_ * 512:b_ * lp + (t_ + 1) * 512], pc)
            mv1 = wk.tile([C1, 2], f32, tag="wk")
            nc.vector.bn_aggr(out=mv1, in_=bnst1)
            part1 = wk.tile([C1, 2], f32, tag="wk")
            sqm = wk.tile([C1, 1], f32, tag="wk")
            nc.vector.tensor_mul(sqm, mv1[:, 0:1], mv1[:, 0:1])
            nc.vector.tensor_tensor(out=sqm, in0=sqm, in1=mv1[:, 1:2], op=OP.add)
            nl_ = float(bloc * lp)
            nc.vector.tensor_scalar(out=part1[:, 0:1], in0=mv1[:, 0:1],
                                    scalar1=nl_, scalar2=None, op0=OP.mult)
            nc.vector.tensor_scalar(out=part1[:, 1:2], in0=sqm,
                                    scalar1=nl_, scalar2=None, op0=OP.mult)
            bn1_in = dram.tile([C1, 2], f32, tag="bn1i")
            bn1_out = dram.tile([C1, 2], f32, tag="bn1o")
            nc.sync.dma_start(out=bn1_in, in_=part1)
            nc.gpsimd.collective_compute(
                "AllReduce", OP.add, replica_groups=[list(range(n_cores))],
                ins=[bn1_in[:, :].opt()], outs=[bn1_out[:, :].opt()])
            glob1 = wk.tile([C1, 2], f32, tag="wk")
            nc.sync.dma_start(out=glob1, in_=bn1_out)

            def bn_scale_shift(globc, n_, g_ap, b_ap):
                mean = wk.tile([C1, 1], f32, tag="wk")
                nc.vector.tensor_scalar(out=mean, in0=globc[:, 0:1],
                                        scalar1=1.0 / n_, scalar2=None, op0=OP.mult)
                ex2 = wk.tile([C1, 1], f32, tag="wk")
                nc.vector.tensor_scalar(out=ex2, in0=globc[:, 1:2],
                                        scalar1=1.0 / n_, scalar2=None, op0=OP.mult)
                msq_ = wk.tile([C1, 1], f32, tag="wk")
                nc.vector.tensor_mul(msq_, mean, mean)
                nc.vector.tensor_tensor(out=ex2, in0=ex2, in1=msq_, op=OP.subtract)
                lnv_ = wk.tile([C1, 1], f32, tag="wk")
                nc.scalar.activation(lnv_, ex2, AF.Ln, bias=eps128, scale=1.0)
                rstd_ = wk.tile([C1, 1], f32, tag="wk")
                nc.scalar.activation(rstd_, lnv_, AF.Exp, bias=0.0, scale=-0.5)
                scale = longs.tile([C1, 1], f32, tag="bnsc")
                nc.vector.tensor_mul(scale, rstd_, g_ap)
                shift = longs.tile([C1, 1], f32, tag="bnsh")
                nc.vector.tensor_mul(shift, mean, scale)
                nc.vector.tensor_tensor(out=shift, in0=b_ap, in1=shift,
                                        op=OP.subtract)
                return scale, shift

            sc1, sh1 = bn_scale_shift(glob1, n1, bn1g, bn1b)
            p1_sb = big.tile([C1, bloc * LP2], f32, tag="chF")   # after band
            if dbg:
                nc.sync.dma_start(out=dbg_outs["dbg_c1"][:, :], in_=c1_sb)
            for b_ in range(bloc):
                for t_ in range(NLT):
                    rel = wkt()
                    nc.scalar.activation(
                        rel, c1_sb[:, b_ * lp + t_ * 512:b_ * lp + (t_ + 1) * 512],
                        AF.Relu, bias=sh1, scale=sc1)
                    rel2 = rel.rearrange("p (l two) -> p l two", two=2)
                    nc.vector.tensor_tensor(
                        out=p1_sb[:, b_ * LP2 + t_ * 256:b_ * LP2 + (t_ + 1) * 256],
                        in0=rel2[:, :, 0], in1=rel2[:, :, 1], op=OP.max)
            if dbg:
                nc.sync.dma_start(out=dbg_outs["dbg_p1"][:, :], in_=p1_sb)

            # ================= conv2 + bn2 =================
            c2_sb = big.tile([C1, bloc * 2 * L2], f32, tag="chE")  # after qT
            bnst2 = longs.tile([C1, 2, bloc * 2, 6], f32, tag="bnst2")
            for b_ in range(bloc):
                for half in range(2):
                    for t_ in range(2):
                        pc = ps_sm.tile([C1, LT2], f32, tag="sm")
                        for k_ in range(K):
                            nc.tensor.matmul(
                                pc, c2wT[:, k_ * C2 + half * C1:
                                         k_ * C2 + (half + 1) * C1],
                                p1_sb[:, b_ * LP2 + t_ * LT2 + k_:
                                      b_ * LP2 + t_ * LT2 + k_ + LT2],
                                start=(k_ == 0), stop=(k_ == K - 1))
                        nc.vector.bn_stats(out=bnst2[:, half, b_ * 2 + t_, :],
                                           in_=pc)
                        nc.vector.tensor_copy(
                            c2_sb[:, (b_ * 2 + half) * L2 + t_ * LT2:
                                  (b_ * 2 + half) * L2 + (t_ + 1) * LT2], pc)
            part2 = longs.tile([C1, 4], f32, tag="part2")
            for half in range(2):
                mv2 = wk.tile([C1, 2], f32, tag="wk")
                nc.vector.bn_aggr(out=mv2, in_=bnst2[:, half, :, :])
                sqm2 = wk.tile([C1, 1], f32, tag="wk")
                nc.vector.tensor_mul(sqm2, mv2[:, 0:1], mv2[:, 0:1])
                nc.vector.tensor_tensor(out=sqm2, in0=sqm2, in1=mv2[:, 1:2],
                                        op=OP.add)
                nl2 = float(bloc * L2)
                nc.vector.tensor_scalar(out=part2[:, 2 * half:2 * half + 1],
                                        in0=mv2[:, 0:1], scalar1=nl2,
                                        scalar2=None, op0=OP.mult)
                nc.vector.tensor_scalar(out=part2[:, 2 * half + 1:2 * half + 2],
                                        in0=sqm2, scalar1=nl2,
                                        scalar2=None, op0=OP.mult)
            bn2_in = dram.tile([C1, 4], f32, tag="bn2i")
            bn2_out = dram.tile([C1, 4], f32, tag="bn2o")
            nc.sync.dma_start(out=bn2_in, in_=part2)
            nc.gpsimd.collective_compute(
                "AllReduce", OP.add, replica_groups=[list(range(n_cores))],
                ins=[bn2_in[:, :].opt()], outs=[bn2_out[:, :].opt()])
            glob2 = longs.tile([C1, 4], f32, tag="glob2")
            nc.sync.dma_start(out=glob2, in_=bn2_out)

            feat = longs.tile([C1, 2 * bloc], f32, tag="feat")
            for half in range(2):
                sc2, sh2 = bn_scale_shift(glob2[:, 2 * half:2 * half + 2], n2,
                                          bn2g[:, half:half + 1],
                                          bn2b[:, half:half + 1])
                for b_ in range(bloc):
                    rel = wkb.tile([C1, L2], f32, tag="wkb")
                    nc.scalar.activation(
                        rel, c2_sb[:, (b_ * 2 + half) * L2:
                                   (b_ * 2 + half + 1) * L2],
                        AF.Relu, bias=sh2, scale=sc2)
                    nc.vector.reduce_max(
                        out=feat[:, (b_ * 2 + half):(b_ * 2 + half) + 1],
                        in_=rel, axis=AX.X)
            if dbg:
                nc.sync.dma_start(out=dbg_outs["dbg_feat"][:, :], in_=feat)

            # ================= fc =================
            for b_ in range(bloc):
                pf = ps_sm.tile([NC, 1], f32, tag="sm")
                for half in range(2):
                    nc.tensor.matmul(pf, fcwT[:, half * NC:(half + 1) * NC],
                                     feat[:, b_ * 2 + half:b_ * 2 + half + 1],
                                     start=(half == 0), stop=(half == 1))
                ob2 = wk.tile([NC, 1], f32, tag="wk")
                nc.vector.tensor_scalar(out=ob2, in0=pf, scalar1=fcb_s,
                                        scalar2=None, op0=OP.add)
                nc.sync.dma_start(out=out_d[b_, :], in_=ob2[:, 0])

    nc.compile()
    return nc


def _host_inputs(inputs, n_cores, bloc, lp):
    X = np.asarray(inputs["X"]).astype(np.float32)[:, :lp]
    sa = np.asarray(inputs["surface_availability"], dtype=np.float32)[:, :lp]
    ptm = np.asarray(inputs["ptm"]).astype(np.float32)[:, :lp]
    emb = np.asarray(inputs["emb"], dtype=np.float32)
    pemb = np.asarray(inputs["ptm_emb"], dtype=np.float32)
    rpe = np.asarray(inputs["rpe"], dtype=np.float32)
    inw = np.asarray(inputs["in_proj_w"], dtype=np.float32)
    inb = np.asarray(inputs["in_proj_b"], dtype=np.float32)
    wo = np.asarray(inputs["out_proj_w"], dtype=np.float32)
    bo = np.asarray(inputs["out_proj_b"], dtype=np.float32)
    w1 = np.asarray(inputs["lin1_w"], dtype=np.float32)
    b1 = np.asarray(inputs["lin1_b"], dtype=np.float32)
    w2 = np.asarray(inputs["lin2_w"], dtype=np.float32)
    b2 = np.asarray(inputs["lin2_b"], dtype=np.float32)
    c1w = np.asarray(inputs["conv1_w"], dtype=np.float32)
    c2w = np.asarray(inputs["conv2_w"], dtype=np.float32)
    fcw = np.asarray(inputs["fc_w"], dtype=np.float32)

    pembp = np.zeros((10, D), np.float32)
    pembp[:, ED:] = pemb
    embp = np.zeros((NCAT, D), np.float32)
    embp[:, :ED] = emb

    clo, chi = rpe[0], rpe[2 * MD]
    bandcat = np.zeros((D, H * BAND_TOT), np.float32)
    jj = np.arange(128)[:, None]
    for h in range(H):
        for di, dl in enumerate(BAND_DELTAS):
            w = BAND_W[di]
            ii = np.arange(BAND_C0[di], BAND_C0[di] + w)[None, :]
            e = dl + jj - ii
            val = rpe[np.clip(e, -MD, MD) + MD, h]
            beta = chi[h] if di == 5 else clo[h]
            bandcat[:, h * BAND_TOT + BAND_OFF[di]:
                    h * BAND_TOT + BAND_OFF[di] + w] = val - beta
    cexp = np.zeros((D, 2 * H), np.float32)
    for h in range(H):
        cexp[:, 2 * h] = clo[h]
        cexp[:, 2 * h + 1] = chi[h]

    ob_eff = bo + wo @ inb[2 * D:3 * D]
    l2t = w2.T   # [FF, D]
    shared = {
        "embp": embp, "pembp": pembp,
        "iota25": np.arange(NCAT, dtype=np.float32)[:, None],
        "iota10": np.arange(10, dtype=np.float32)[:, None],
        "inwT": np.ascontiguousarray(inw.T),
        "qb": (inb[0:D] * ISQ)[:, None],
        "kb": inb[D:2 * D][:, None],
        "woT": np.ascontiguousarray(wo.T),
        "ob_eff": ob_eff[:, None].astype(np.float32),
        "l1wT": np.ascontiguousarray(w1.T),
        "l1b": b1.reshape(2, D).T.copy(),
        "l2cat": np.concatenate([l2t[0:D], l2t[D:2 * D]], axis=1).copy(),
        "l2b": b2[:, None],
        "ln1g": np.asarray(inputs["ln1_g"], np.float32)[:, None],
        "ln1b": np.asarray(inputs["ln1_b"], np.float32)[:, None],
        "ln2g": np.asarray(inputs["ln2_g"], np.float32)[:, None],
        "ln2b": np.asarray(inputs["ln2_b"], np.float32)[:, None],
        "bandcat": bandcat, "cexp": cexp,
        "c1wT": np.ascontiguousarray(c1w.transpose(1, 2, 0).reshape(D, K * C1)),
        "c2wT": np.ascontiguousarray(c2w.transpose(1, 2, 0).reshape(C1, K * C2)),
        "bn1g": np.asarray(inputs["bn1_g"], np.float32)[:, None],
        "bn1b": np.asarray(inputs["bn1_b"], np.float32)[:, None],
        "bn2g": np.asarray(inputs["bn2_g"], np.float32).reshape(2, C1).T.copy(),
        "bn2b": np.asarray(inputs["bn2_b"], np.float32).reshape(2, C1).T.copy(),
        "fcwT": np.ascontiguousarray(
            fcw.T.reshape(2, C1, NC).transpose(1, 0, 2).reshape(C1, 2 * NC)),
        "fcb": np.asarray(inputs["fc_b"], np.float32)[:, None],
    }
    in_maps = []
    for c in range(n_cores):
        rows = slice(c * bloc, (c + 1) * bloc)
        m = dict(shared)
        m["Xf"] = np.ascontiguousarray(X[rows].reshape(-1))
        m["saf"] = np.ascontiguousarray(sa[rows].reshape(-1))
        m["ptmf"] = np.ascontiguousarray(ptm[rows].reshape(-1))
        in_maps.append(m)
    return in_maps


_NC_CACHE = {}


def _get_nc(n_cores, bloc, lp, dbg=False):
    key = (n_cores, bloc, lp, dbg)
    if key not in _NC_CACHE:
        _NC_CACHE[key] = _build(n_cores, bloc, lp, dbg=dbg)
    return _NC_CACHE[key]


def kernel(**inputs):
    from concourse.bass_utils import run_bass_kernel_spmd
    nc = _get_nc(NCORES, BLOC, L)
    in_maps = _host_inputs(inputs, NCORES, BLOC, L)
    res = run_bass_kernel_spmd(nc, in_maps, list(range(NCORES)))
    out = np.concatenate([res.results[i]["out"] for i in range(NCORES)], axis=0)
    return out.astype(np.float32)



# revision 14
# speedup vs baseline: 1.1316x; 1.1316x over previous
"""Trainium2 Bass kernel for nn_CNN2LWithRPE (transformer layer + CNN head).

Sharding: data-parallel over batch across 8 NeuronCores (2 batch rows each).
All parameters replicated. The only cross-core communication is two tiny
AllReduces for the training-mode BatchNorm statistics.

Per-core layout (B_loc batches, T = B_loc*L tokens):
  - activations transposed in SBUF as bf16: xT/qT/x1T... are [D=128, T]
    (bf16 operands run the PE at 1 cycle/row vs 4 for fp32; PSUM
    accumulation stays fp32)
  - attention as scores^T tiles [keys=128 part, queries=512 free]:
    QK^T row-packed 4x on PE (K=32) from a shuffled kTp layout + a 4x
    replicated qrep tile; exp on ACT as wide [128,2048] ops with the
    clipped-RPE bias folded into a per-group bias constant (c_lo/c_hi)
    plus narrow host-precomputed banded correction tiles added on DVE;
    PV uses v in natural layout with an appended ones-column so the
    softmax denominator falls out of the same matmul.
  - layernorm in transposed layout: partition stats via ones-matmul,
    rstd = Exp(-0.5*Ln(var+eps)), rank-1 K=1 matmul broadcast back.
  - conv1d as K accumulating shifted matmuls; BN apply fused into
    ACT Relu(scale*x+shift) with per-channel scale/shift APs.
"""

import numpy as np

B, L = 16, 2048
NCAT, ED = 25, 120
D, H, HD = 128, 4, 32
FF = 256
MD = 32
C1, C2, K = 128, 256, 5
NC = 2
EPS = 1e-5
NCORES = 8
BLOC = B // NCORES
ISQ = float(1.0 / np.sqrt(HD))

QT = 512
KTILE = 128

BAND_DELTAS = [-128, 0, 128, 256, 384, 512]
BAND_W = [32, 160, 288, 416, 512, 32]
BAND_C0 = [0, 0, 0, 0, 0, 480]
BAND_OFF = [0, 32, 192, 480, 896, 1408]
BAND_TOT = 1440


def _build(n_cores, bloc, lp, dbg=False):
    import contextlib
    import concourse.bass as bass
    import concourse.tile as tile
    from concourse import bacc, mybir

    f32 = mybir.dt.float32
    bf16 = mybir.dt.bfloat16
    AF = mybir.ActivationFunctionType
    OP = mybir.AluOpType
    AX = mybir.AxisListType

    T = bloc * lp
    NET = T // 512
    NQT = lp // QT
    NKT = lp // KTILE
    NG = NKT // 4
    NLT = lp // 512
    NDEN = bloc * NQT * H
    LP2 = lp // 2
    L2 = LP2 - (K - 1)
    LT2 = L2 // 2
    n1 = float(n_cores * bloc * lp)
    n2 = float(n_cores * bloc * L2)

    nc = bacc.Bacc("TRN2", target_bir_lowering=False, debug=False,
                   num_devices=n_cores)

    def din(name, shape):
        return nc.dram_tensor(name, list(shape), f32, kind="ExternalInput")

    Xf = din("Xf", [T])
    saf = din("saf", [T])
    ptmf = din("ptmf", [T])
    emb_d = din("embp", [NCAT, D])
    pemb_d = din("pembp", [10, D])
    iota25_d = din("iota25", [NCAT, 1])
    iota10_d = din("iota10", [10, 1])
    inwT_d = din("inwT", [D, 3 * D])
    qb_d = din("qb", [D, 1])
    kb_d = din("kb", [D, 1])
    woT_d = din("woT", [D, D])
    ob_d = din("ob_eff", [D, 1])
    l1wT_d = din("l1wT", [D, FF])
    l1b_d = din("l1b", [D, 2])
    l2cat_d = din("l2cat", [D, 2 * D])
    l2b_d = din("l2b", [D, 1])
    ln1g_d = din("ln1g", [D, 1])
    ln1b_d = din("ln1b", [D, 1])
    ln2g_d = din("ln2g", [D, 1])
    ln2b_d = din("ln2b", [D, 1])
    band_d = din("bandcat", [D, H * BAND_TOT])
    cexp_d = din("cexp", [D, 2 * H])
    c1wT_d = din("c1wT", [D, K * C1])
    c2wT_d = din("c2wT", [C1, K * C2])
    bn1g_d = din("bn1g", [C1, 1])
    bn1b_d = din("bn1b", [C1, 1])
    bn2g_d = din("bn2g", [C1, 2])
    bn2b_d = din("bn2b", [C1, 2])
    fcwT_d = din("fcwT", [C1, 2 * NC])
    fcb_d = din("fcb", [NC, 1])

    out_d = nc.dram_tensor("out", [bloc, NC], f32, kind="ExternalOutput")
    dbg_outs = {}
    if dbg:
        for nm, shp, dt_ in [("dbg_xT", [D, T], bf16),
                             ("dbg_qT", [D, T], bf16),
                             ("dbg_kT", [D, T], bf16),
                             ("dbg_attnT", [D, T], bf16),
                             ("dbg_x1T", [D, T], bf16),
                             ("dbg_x2", [D, T], bf16),
                             ("dbg_c1", [C1, bloc * lp], bf16),
                             ("dbg_p1", [C1, bloc * LP2], bf16),
                             ("dbg_feat", [C1, 2 * bloc], f32)]:
            dbg_outs[nm] = nc.dram_tensor(nm, shp, dt_, kind="ExternalOutput")

    def bc(ap1d, parts):
        return bass.AP(tensor=ap1d.tensor, offset=ap1d.offset,
                       ap=[[0, parts]] + [list(p) for p in ap1d.ap])

    with tile.TileContext(nc) as tc:
        ctx = contextlib.ExitStack()
        with ctx:
            pp = ctx.enter_context(tc.tile_pool(name="params", bufs=1))
            big = ctx.enter_context(tc.tile_pool(name="big", bufs=1))
            wk = ctx.enter_context(tc.tile_pool(name="wk", bufs=12))
            wkb = ctx.enter_context(tc.tile_pool(name="wkb", bufs=2))
            longs = ctx.enter_context(tc.tile_pool(name="longs", bufs=1))
            mid = ctx.enter_context(tc.tile_pool(name="mid", bufs=6))
            rowp = ctx.enter_context(tc.tile_pool(name="rows", bufs=4))
            ptp = ctx.enter_context(tc.tile_pool(name="pt", bufs=2))
            qrp = ctx.enter_context(tc.tile_pool(name="qrep", bufs=1))
            ps_sc = ctx.enter_context(tc.tile_pool(name="ps_sc", bufs=1, space="PSUM"))
            ps_sm = ctx.enter_context(tc.tile_pool(name="ps_sm", bufs=4, space="PSUM"))
            dram = ctx.enter_context(tc.tile_pool(name="dram", bufs=1, space="DRAM"))

            def wkt(p=D, f=512):
                return wk.tile([p, f], f32, tag="wk", name="wkt")

            def midt(p, f):
                return mid.tile([p, f], f32, tag="mid", name="midt")

            # ---- params ----
            def ld(dten, shape):
                t = pp.tile(list(shape), f32, tag=dten.name, name=dten.name + "_s")
                nc.sync.dma_start(out=t, in_=dten[tuple(slice(0, s) for s in shape)])
                return t

            def ldb(dten, shape):
                # load f32 param, convert to a bf16 copy for PE consumption
                t = ld(dten, shape)
                tb = pp.tile(list(shape), bf16, tag=dten.name + "_b",
                             name=dten.name + "_b")
                nc.vector.tensor_copy(tb, t)
                return tb

            emb_b = ldb(emb_d, [NCAT, D])
            pemb_b = ldb(pemb_d, [10, D])
            io25 = ld(iota25_d, [NCAT, 1])
            io10 = ld(iota10_d, [10, 1])
            inwT_b = ldb(inwT_d, [D, 3 * D])
            qb_s = ld(qb_d, [D, 1])
            kb_s = ld(kb_d, [D, 1])
            woT_b = ldb(woT_d, [D, D])
            ob_s = ld(ob_d, [D, 1])
            l1wT_b = ldb(l1wT_d, [D, FF])
            l1b_s = ld(l1b_d, [D, 2])
            l2cat_b = ldb(l2cat_d, [D, 2 * D])
            l2b_s = ld(l2b_d, [D, 1])
            ln1g = ld(ln1g_d, [D, 1]); ln1b = ld(ln1b_d, [D, 1])
            ln2g = ld(ln2g_d, [D, 1]); ln2b = ld(ln2b_d, [D, 1])
            cexp = ld(cexp_d, [D, 2 * H])
            c1wT_b = ldb(c1wT_d, [D, K * C1])
            c2wT_b = ldb(c2wT_d, [C1, K * C2])
            bn1g = ld(bn1g_d, [C1, 1]); bn1b = ld(bn1b_d, [C1, 1])
            bn2g = ld(bn2g_d, [C1, 2]); bn2b = ld(bn2b_d, [C1, 2])
            fcwT = ld(fcwT_d, [C1, 2 * NC])
            fcb_s = ld(fcb_d, [NC, 1])

            band = big.tile([D, H * BAND_TOT], f32, tag="chF")
            nc.sync.dma_start(out=band, in_=band_d[:, :])

            ones128b = pp.tile([D, 1], bf16, tag="ones128b")
            nc.vector.memset(ones128b, 1.0)
            ones_rb = pp.tile([1, D], bf16, tag="ones_rb")
            nc.vector.memset(ones_rb, 1.0)
            ones32b = pp.tile([1, HD], bf16, tag="ones32b")
            nc.vector.memset(ones32b, 1.0)
            epsNLT = pp.tile([NLT, 1], f32, tag="epsNLT")
            nc.vector.memset(epsNLT, EPS)
            eps128 = pp.tile([D, 1], f32, tag="eps128")
            nc.vector.memset(eps128, EPS)

            # ---- persistent activations (bf16) ----
            xT = big.tile([D, T], bf16, tag="chA")
            qT = big.tile([D, T], bf16, tag="chE")
            kTp = big.tile([D, bloc, H, NG, KTILE], bf16, tag="chB")
            v_sb = big.tile([D, T // KTILE, H, HD + 1], bf16, tag="chC")
            attnT = big.tile([D, T], bf16, tag="chD")
            x1T = big.tile([D, T], bf16, tag="chG")
            nc.vector.memset(v_sb[:, :, :, HD:HD + 1], 1.0)

            # ================= embedding =================
            for e in range(NET):
                sl = slice(e * 512, (e + 1) * 512)
                xb = wkt(NCAT)
                nc.sync.dma_start(out=xb, in_=bc(Xf[sl], NCAT))
                sb_ = wkt(NCAT)
                nc.sync.dma_start(out=sb_, in_=bc(saf[sl], NCAT))
                oh = wk.tile([NCAT, 512], bf16, tag="wk", name="oh")
                nc.vector.tensor_scalar(out=oh, in0=xb, scalar1=io25,
                                        scalar2=None, op0=OP.is_equal)
                nc.vector.tensor_mul(oh, oh, sb_)
                pb = wkt(10)
                nc.sync.dma_start(out=pb, in_=bc(ptmf[sl], 10))
                ohp = wk.tile([10, 512], bf16, tag="wk", name="ohp")
                nc.vector.tensor_scalar(out=ohp, in0=pb, scalar1=io10,
                                        scalar2=None, op0=OP.is_equal)
                pe = ps_sm.tile([D, 512], f32, tag="sm")
                nc.tensor.matmul(pe, pemb_b, ohp, start=True, stop=False)
                nc.tensor.matmul(pe, emb_b, oh, start=False, stop=True)
                nc.vector.tensor_copy(xT[:, sl], pe)

            if dbg:
                nc.sync.dma_start(out=dbg_outs["dbg_xT"][:, :], in_=xT)

            # ================= qkv =================
            for e in range(NET):
                sl = slice(e * 512, (e + 1) * 512)
                pq = ps_sm.tile([D, 512], f32, tag="sm")
                nc.tensor.matmul(pq, inwT_b[:, 0:D], xT[:, sl],
                                 start=True, stop=True)
                nc.vector.tensor_scalar(out=qT[:, sl], in0=pq, scalar1=ISQ,
                                        scalar2=qb_s, op0=OP.mult, op1=OP.add)
                pk = ps_sm.tile([D, 512], f32, tag="sm")
                nc.tensor.matmul(pk, inwT_b[:, D:2 * D], xT[:, sl],
                                 start=True, stop=True)
                ktmp = mid.tile([D, 512], bf16, tag="mid", name="ktmp")
                nc.vector.tensor_scalar(out=ktmp, in0=pk, scalar1=kb_s,
                                        scalar2=None, op0=OP.add)
                b_ = (e * 512) // lp
                for h in range(H):
                    for sub in range(4):
                        ktb = ((e * 512) % lp) // KTILE + sub
                        nc.sync.dma_start(
                            out=kTp[32 * (ktb % 4):32 * (ktb % 4) + 32,
                                    b_, h, ktb // 4, :],
                            in_=ktmp[32 * h:32 * h + 32,
                                     sub * KTILE:(sub + 1) * KTILE])
                for sub in range(4):
                    tt = (e * 512) // KTILE + sub
                    pv = ps_sm.tile([KTILE, D], f32, tag="sm")
                    nc.tensor.matmul(pv, xT[:, e * 512 + sub * KTILE:
                                            e * 512 + (sub + 1) * KTILE],
                                     inwT_b[:, 2 * D:3 * D], start=True, stop=True)
                    nc.vector.tensor_copy(
                        v_sb[:, tt, :, 0:HD],
                        pv.rearrange("p (h d) -> p h d", h=H))

            if dbg:
                nc.sync.dma_start(out=dbg_outs["dbg_qT"][:, :], in_=qT)
                kT_dbg = big.tile([D, T], bf16, tag="chDBG")
                for b_ in range(bloc):
                    for h in range(H):
                        for g in range(NG):
                            for r in range(4):
                                kt = 4 * g + r
                                nc.sync.dma_start(
                                    out=kT_dbg[32 * h:32 * h + 32,
                                               b_ * lp + kt * KTILE:
                                               b_ * lp + (kt + 1) * KTILE],
                                    in_=kTp[32 * r:32 * r + 32, b_, h, g, :])
                nc.sync.dma_start(out=dbg_outs["dbg_kT"][:, :], in_=kT_dbg)

            # ================= attention =================
            den32 = longs.tile([NDEN, 512], bf16, tag="den32")
            dmap = {(-1, 3): 0, (0, 0): 1, (0, 1): 2, (0, 2): 3, (0, 3): 4, (1, 0): 5}
            for b_ in range(bloc):
                for h in range(H):
                    qrep = qrp.tile([D, lp], bf16, tag="qr")
                    for r in range(4):
                        nc.sync.dma_start(
                            out=qrep[32 * r:32 * r + 32, :],
                            in_=qT[32 * h:32 * h + 32, b_ * lp:(b_ + 1) * lp])
                    for qt in range(NQT):
                        ppv = ps_sm.tile([HD + 1, 512], f32, tag="sm")
                        for g in range(NG):
                            sc = ps_sc.tile([D, 4 * 512], f32, tag="sc")
                            for q in range(4):
                                nc.tensor.matmul(
                                    sc[:, q * 512:(q + 1) * 512],
                                    kTp[32 * q:32 * q + 32, b_, h, g, :],
                                    qrep[32 * q:32 * q + 32, qt * QT:(qt + 1) * QT],
                                    start=True, stop=True,
                                    tile_position=(32 * q, 0))
                            for q in range(4):
                                di = dmap.get((g - qt, q))
                                if di is not None:
                                    c0, w = BAND_C0[di], BAND_W[di]
                                    nc.vector.tensor_tensor(
                                        out=sc[:, q * 512 + c0:q * 512 + c0 + w],
                                        in0=sc[:, q * 512 + c0:q * 512 + c0 + w],
                                        in1=band[:, h * BAND_TOT + BAND_OFF[di]:
                                                 h * BAND_TOT + BAND_OFF[di] + w],
                                        op=OP.add)
                            side = 0 if g <= qt else 1
                            pt = ptp.tile([D, 4 * 512], bf16, tag="pt")
                            nc.scalar.activation(pt, sc, AF.Exp,
                                                 bias=cexp[:, 2 * h + side:
                                                           2 * h + side + 1],
                                                 scale=1.0)
                            for q in range(4):
                                kt = 4 * g + q
                                nc.tensor.matmul(
                                    ppv, v_sb[:, b_ * NKT + kt, h, :],
                                    pt[:, q * 512:(q + 1) * 512],
                                    start=(kt == 0), stop=(kt == NKT - 1))
                        pv_sb = wk.tile([HD + 1, 512], bf16, tag="wk", name="pv_sb")
                        nc.vector.tensor_copy(pv_sb, ppv)
                        nc.sync.dma_start(
                            out=attnT[32 * h:32 * h + 32,
                                      b_ * lp + qt * QT:b_ * lp + (qt + 1) * QT],
                            in_=pv_sb[0:HD, :])
                        nc.sync.dma_start(
                            out=den32[(b_ * NQT + qt) * H + h:
                                      (b_ * NQT + qt) * H + h + 1, :],
                            in_=pv_sb[HD:HD + 1, :])

            lnden = midt(NDEN, 512)
            nc.scalar.activation(lnden, den32, AF.Ln, bias=0.0, scale=1.0)
            recip = longs.tile([NDEN, 512], bf16, tag="recip")
            nc.scalar.activation(recip, lnden, AF.Exp, bias=0.0, scale=-1.0)

            for b_ in range(bloc):
                for qt in range(NQT):
                    bcp = ps_sm.tile([D, 512], f32, tag="sm")
                    for h in range(H):
                        rr = rowp.tile([1, 512], bf16, tag="row")
                        nc.sync.dma_start(
                            out=rr, in_=recip[(b_ * NQT + qt) * H + h:
                                              (b_ * NQT + qt) * H + h + 1, :])
                        nc.tensor.matmul(bcp[32 * h:32 * h + 32, :], ones32b, rr,
                                         start=True, stop=True,
                                         tile_position=(0, 32 * h))
                    sl = slice(b_ * lp + qt * QT, b_ * lp + (qt + 1) * QT)
                    nc.vector.tensor_mul(attnT[:, sl], attnT[:, sl], bcp)

            if dbg:
                nc.sync.dma_start(out=dbg_outs["dbg_attnT"][:, :], in_=attnT)

            # ======== layernorm helper (bf16 transposed layout) ========
            def layernorm_T(src, dst_fn, g_s, b_s, b_base):
                s1c = midt(NLT, 512)
                s2c = midt(NLT, 512)
                for t_ in range(NLT):
                    sl = slice(b_base * lp + t_ * 512, b_base * lp + (t_ + 1) * 512)
                    sq = wk.tile([D, 512], bf16, tag="wk", name="sq")
                    nc.vector.tensor_mul(sq, src[:, sl], src[:, sl])
                    p1_ = ps_sm.tile([1, 512], f32, tag="sm")
                    nc.tensor.matmul(p1_, ones128b, src[:, sl], start=True, stop=True)
                    p2_ = ps_sm.tile([1, 512], f32, tag="sm")
                    nc.tensor.matmul(p2_, ones128b, sq, start=True, stop=True)
                    s1t = rowp.tile([1, 512], f32, tag="row", name="s1t")
                    nc.vector.tensor_copy(s1t, p1_)
                    s2t = rowp.tile([1, 512], f32, tag="row", name="s2t")
                    nc.vector.tensor_copy(s2t, p2_)
                    nc.sync.dma_start(out=s1c[t_:t_ + 1, :], in_=s1t)
                    nc.sync.dma_start(out=s2c[t_:t_ + 1, :], in_=s2t)
                m_ = wkt(NLT)
                nc.vector.tensor_scalar(out=m_, in0=s1c, scalar1=1.0 / D,
                                        scalar2=None, op0=OP.mult)
                var = wkt(NLT)
                nc.vector.tensor_scalar(out=var, in0=s2c, scalar1=1.0 / D,
                                        scalar2=None, op0=OP.mult)
                msq = wkt(NLT)
                nc.vector.tensor_mul(msq, m_, m_)
                nc.vector.tensor_tensor(out=var, in0=var, in1=msq, op=OP.subtract)
                lnv = wkt(NLT)
                nc.scalar.activation(lnv, var, AF.Ln, bias=epsNLT[0:NLT, :],
                                     scale=1.0)
                rstd = wk.tile([NLT, 512], bf16, tag="wk", name="rstd")
                nc.scalar.activation(rstd, lnv, AF.Exp, bias=0.0, scale=-0.5)
                mr = wk.tile([NLT, 512], bf16, tag="wk", name="mr")
                nc.vector.tensor_mul(mr, m_, rstd)
                for t_ in range(NLT):
                    sl = slice(b_base * lp + t_ * 512, b_base * lp + (t_ + 1) * 512)
                    rr = rowp.tile([1, 512], bf16, tag="row")
                    nc.sync.dma_start(out=rr, in_=rstd[t_:t_ + 1, :])
                    rm = rowp.tile([1, 512], bf16, tag="row")
                    nc.sync.dma_start(out=rm, in_=mr[t_:t_ + 1, :])
                    br = ps_sm.tile([D, 512], f32, tag="sm")
                    nc.tensor.matmul(br, ones_rb, rr, start=True, stop=True)
                    bm = ps_sm.tile([D, 512], f32, tag="sm")
                    nc.tensor.matmul(bm, ones_rb, rm, start=True, stop=True)
                    tmp = wkt()
                    nc.vector.tensor_mul(tmp, src[:, sl], br)
                    nc.vector.tensor_tensor(out=tmp, in0=tmp, in1=bm,
                                            op=OP.subtract)
                    nc.vector.tensor_scalar(out=dst_fn(t_), in0=tmp, scalar1=g_s,
                                            scalar2=b_s, op0=OP.mult, op1=OP.add)

            # ================= out-proj + residual + LN1 =================
            r1T = big.tile([D, T], bf16, tag="chB")   # after kTp's last read
            for b_ in range(bloc):
                for qt in range(NQT):
                    sl = slice(b_ * lp + qt * QT, b_ * lp + (qt + 1) * QT)
                    po = ps_sm.tile([D, 512], f32, tag="sm")
                    nc.tensor.matmul(po, woT_b, attnT[:, sl], start=True, stop=True)
                    nc.vector.tensor_scalar(out=r1T[:, sl], in0=po, scalar1=ob_s,
                                            scalar2=None, op0=OP.add)
                    nc.vector.tensor_tensor(out=r1T[:, sl], in0=r1T[:, sl],
                                            in1=xT[:, sl], op=OP.add)
            for b_ in range(bloc):
                layernorm_T(
                    r1T,
                    lambda t_, b0=b_: x1T[:, b0 * lp + t_ * 512:
                                          b0 * lp + (t_ + 1) * 512],
                    ln1g, ln1b, b_)

            if dbg:
                nc.sync.dma_start(out=dbg_outs["dbg_x1T"][:, :], in_=x1T)

            # ================= FFN + residual + LN2 =================
            x2pad = big.tile([D, bloc * (lp + 4)], bf16, tag="chA")  # after xT
            nc.vector.memset(x2pad[:, :], 0.0)
            r2T = big.tile([D, T], bf16, tag="chC")                  # after v_sb
            for b_ in range(bloc):
                for qt in range(NQT):
                    sl = slice(b_ * lp + qt * QT, b_ * lp + (qt + 1) * QT)
                    h1a = wk.tile([D, 512], bf16, tag="wk", name="h1a")
                    h1b = wk.tile([D, 512], bf16, tag="wk", name="h1b")
                    for half, dest in ((0, h1a), (1, h1b)):
                        ph = ps_sm.tile([D, 512], f32, tag="sm")
                        nc.tensor.matmul(ph, l1wT_b[:, half * D:(half + 1) * D],
                                         x1T[:, sl], start=True, stop=True)
                        nc.scalar.activation(dest, ph, AF.Relu,
                                             bias=l1b_s[:, half:half + 1],
                                             scale=1.0)
                    py = ps_sm.tile([D, 512], f32, tag="sm")
                    nc.tensor.matmul(py, l2cat_b[:, 0:D], h1a, start=True, stop=False)
                    nc.tensor.matmul(py, l2cat_b[:, D:2 * D], h1b,
                                     start=False, stop=True)
                    nc.vector.tensor_scalar(out=r2T[:, sl], in0=py, scalar1=l2b_s,
                                            scalar2=None, op0=OP.add)
                    nc.vector.tensor_tensor(out=r2T[:, sl], in0=r2T[:, sl],
                                            in1=x1T[:, sl], op=OP.add)
            for b_ in range(bloc):
                layernorm_T(
                    r2T,
                    lambda t_, b0=b_: x2pad[:, b0 * (lp + 4) + 2 + t_ * 512:
                                            b0 * (lp + 4) + 2 + (t_ + 1) * 512],
                    ln2g, ln2b, b_)

            if dbg:
                for b_ in range(bloc):
                    nc.sync.dma_start(
                        out=dbg_outs["dbg_x2"][:, b_ * lp:(b_ + 1) * lp],
                        in_=x2pad[:, b_ * (lp + 4) + 2:b_ * (lp + 4) + 2 + lp])

            # ================= conv1 + bn1 =================
            c1_sb = big.tile([C1, bloc * lp], bf16, tag="chD")   # after attnT
            bnst1 = longs.tile([C1, bloc * NLT, 6], f32, tag="bnst1")
            for b_ in range(bloc):
                for t_ in range(NLT):
                    pc = ps_sm.tile([C1, 512], f32, tag="sm")
                    for k_ in range(K):
                        nc.tensor.matmul(
                            pc, c1wT_b[:, k_ * C1:(k_ + 1) * C1],
                            x2pad[:, b_ * (lp + 4) + t_ * 512 + k_:
                                  b_ * (lp + 4) + t_ * 512 + k_ + 512],
                            start=(k_ == 0), stop=(k_ == K - 1))
                    nc.vector.bn_stats(out=bnst1[:, b_ * NLT + t_, :], in_=pc)
                    nc.vector.tensor_copy(
                        c1_sb[:, b_ * lp + t_ * 512:b_ * lp + (t_ + 1) * 512], pc)
            mv1 = wk.tile([C1, 2], f32, tag="wk")
            nc.vector.bn_aggr(out=mv1, in_=bnst1)
            part1 = wk.tile([C1, 2], f32, tag="wk")
            sqm = wk.tile([C1, 1], f32, tag="wk")
            nc.vector.tensor_mul(sqm, mv1[:, 0:1], mv1[:, 0:1])
            nc.vector.tensor_tensor(out=sqm, in0=sqm, in1=mv1[:, 1:2], op=OP.add)
            nl_ = float(bloc * lp)
            nc.vector.tensor_scalar(out=part1[:, 0:1], in0=mv1[:, 0:1],
                                    scalar1=nl_, scalar2=None, op0=OP.mult)
            nc.vector.tensor_scalar(out=part1[:, 1:2], in0=sqm,
                                    scalar1=nl_, scalar2=None, op0=OP.mult)
            bn1_in = dram.tile([C1, 2], f32, tag="bn1i")
            bn1_out = dram.tile([C1, 2], f32, tag="bn1o")
            nc.sync.dma_start(out=bn1_in, in_=part1)
            nc.gpsimd.collective_compute(
                "AllReduce", OP.add, replica_groups=[list(range(n_cores))],
                ins=[bn1_in[:, :].opt()], outs=[bn1_out[:, :].opt()])
            glob1 = wk.tile([C1, 2], f32, tag="wk")
            nc.sync.dma_start(out=glob1, in_=bn1_out)

            def bn_scale_shift(globc, n_, g_ap, b_ap):
                mean = wk.tile([C1, 1], f32, tag="wk")
                nc.vector.tensor_scalar(out=mean, in0=globc[:, 0:1],
                                        scalar1=1.0 / n_, scalar2=None, op0=OP.mult)
                ex2 = wk.tile([C1, 1], f32, tag="wk")
                nc.vector.tensor_scalar(out=ex2, in0=globc[:, 1:2],
                                        scalar1=1.0 / n_, scalar2=None, op0=OP.mult)
                msq_ = wk.tile([C1, 1], f32, tag="wk")
                nc.vector.tensor_mul(msq_, mean, mean)
                nc.vector.tensor_tensor(out=ex2, in0=ex2, in1=msq_, op=OP.subtract)
                lnv_ = wk.tile([C1, 1], f32, tag="wk")
                nc.scalar.activation(lnv_, ex2, AF.Ln, bias=eps128, scale=1.0)
                rstd_ = wk.tile([C1, 1], f32, tag="wk")
                nc.scalar.activation(rstd_, lnv_, AF.Exp, bias=0.0, scale=-0.5)
                scale = longs.tile([C1, 1], f32, tag="bnsc")
                nc.vector.tensor_mul(scale, rstd_, g_ap)
                shift = longs.tile([C1, 1], f32, tag="bnsh")
                nc.vector.tensor_mul(shift, mean, scale)
                nc.vector.tensor_tensor(out=shift, in0=b_ap, in1=shift,
                                        op=OP.subtract)
                return scale, shift

            sc1, sh1 = bn_scale_shift(glob1, n1, bn1g, bn1b)
            p1_sb = big.tile([C1, bloc * LP2], bf16, tag="chF")   # after band
            if dbg:
                nc.sync.dma_start(out=dbg_outs["dbg_c1"][:, :], in_=c1_sb)
            for b_ in range(bloc):
                for t_ in range(NLT):
                    rel = wk.tile([D, 512], bf16, tag="wk", name="rel")
                    nc.scalar.activation(
                        rel, c1_sb[:, b_ * lp + t_ * 512:b_ * lp + (t_ + 1) * 512],
                        AF.Relu, bias=sh1, scale=sc1)
                    rel2 = rel.rearrange("p (l two) -> p l two", two=2)
                    nc.vector.tensor_tensor(
                        out=p1_sb[:, b_ * LP2 + t_ * 256:b_ * LP2 + (t_ + 1) * 256],
                        in0=rel2[:, :, 0], in1=rel2[:, :, 1], op=OP.max)
            if dbg:
                nc.sync.dma_start(out=dbg_outs["dbg_p1"][:, :], in_=p1_sb)

            # ================= conv2 + bn2 =================
            c2_sb = big.tile([C1, bloc * 2 * L2], bf16, tag="chE")  # after qT
            bnst2 = longs.tile([C1, 2, bloc * 2, 6], f32, tag="bnst2")
            for b_ in range(bloc):
                for half in range(2):
                    for t_ in range(2):
                        pc = ps_sm.tile([C1, LT2], f32, tag="sm")
                        for k_ in range(K):
                            nc.tensor.matmul(
                                pc, c2wT_b[:, k_ * C2 + half * C1:
                                           k_ * C2 + (half + 1) * C1],
                                p1_sb[:, b_ * LP2 + t_ * LT2 + k_:
                                      b_ * LP2 + t_ * LT2 + k_ + LT2],
                                start=(k_ == 0), stop=(k_ == K - 1))
                        nc.vector.bn_stats(out=bnst2[:, half, b_ * 2 + t_, :],
                                           in_=pc)
                        nc.vector.tensor_copy(
                            c2_sb[:, (b_ * 2 + half) * L2 + t_ * LT2:
                                  (b_ * 2 + half) * L2 + (t_ + 1) * LT2], pc)
            part2 = longs.tile([C1, 4], f32, tag="part2")
            for half in range(2):
                mv2 = wk.tile([C1, 2], f32, tag="wk")
                nc.vector.bn_aggr(out=mv2, in_=bnst2[:, half, :, :])
                sqm2 = wk.tile([C1, 1], f32, tag="wk")
                nc.vector.tensor_mul(sqm2, mv2[:, 0:1], mv2[:, 0:1])
                nc.vector.tensor_tensor(out=sqm2, in0=sqm2, in1=mv2[:, 1:2],
                                        op=OP.add)
                nl2 = float(bloc * L2)
                nc.vector.tensor_scalar(out=part2[:, 2 * half:2 * half + 1],
                                        in0=mv2[:, 0:1], scalar1=nl2,
                                        scalar2=None, op0=OP.mult)
                nc.vector.tensor_scalar(out=part2[:, 2 * half + 1:2 * half + 2],
                                        in0=sqm2, scalar1=nl2,
                                        scalar2=None, op0=OP.mult)
            bn2_in = dram.tile([C1, 4], f32, tag="bn2i")
            bn2_out = dram.tile([C1, 4], f32, tag="bn2o")
            nc.sync.dma_start(out=bn2_in, in_=part2)
            nc.gpsimd.collective_compute(
                "AllReduce", OP.add, replica_groups=[list(range(n_cores))],
                ins=[bn2_in[:, :].opt()], outs=[bn2_out[:, :].opt()])
            glob2 = longs.tile([C1, 4], f32, tag="glob2")
            nc.sync.dma_start(out=glob2, in_=bn2_out)

            feat = longs.tile([C1, 2 * bloc], f32, tag="feat")
            for half in range(2):
                sc2, sh2 = bn_scale_shift(glob2[:, 2 * half:2 * half + 2], n2,
                                          bn2g[:, half:half + 1],
                                          bn2b[:, half:half + 1])
                for b_ in range(bloc):
                    rel = wkb.tile([C1, L2], bf16, tag="wkb")
                    nc.scalar.activation(
                        rel, c2_sb[:, (b_ * 2 + half) * L2:
                                   (b_ * 2 + half + 1) * L2],
                        AF.Relu, bias=sh2, scale=sc2)
                    nc.vector.reduce_max(
                        out=feat[:, (b_ * 2 + half):(b_ * 2 + half) + 1],
                        in_=rel, axis=AX.X)
            if dbg:
                nc.sync.dma_start(out=dbg_outs["dbg_feat"][:, :], in_=feat)

            # ================= fc =================
            for b_ in range(bloc):
                pf = ps_sm.tile([NC, 1], f32, tag="sm")
                for half in range(2):
                    nc.tensor.matmul(pf, fcwT[:, half * NC:(half + 1) * NC],
                                     feat[:, b_ * 2 + half:b_ * 2 + half + 1],
                                     start=(half == 0), stop=(half == 1))
                ob2 = wk.tile([NC, 1], f32, tag="wk")
                nc.vector.tensor_scalar(out=ob2, in0=pf, scalar1=fcb_s,
                                        scalar2=None, op0=OP.add)
                nc.sync.dma_start(out=out_d[b_, :], in_=ob2[:, 0])

    nc.compile()
    return nc


def _host_inputs(inputs, n_cores, bloc, lp):
    X = np.asarray(inputs["X"]).astype(np.float32)[:, :lp]
    sa = np.asarray(inputs["surface_availability"], dtype=np.float32)[:, :lp]
    ptm = np.asarray(inputs["ptm"]).astype(np.float32)[:, :lp]
    emb = np.asarray(inputs["emb"], dtype=np.float32)
    pemb = np.asarray(inputs["ptm_emb"], dtype=np.float32)
    rpe = np.asarray(inputs["rpe"], dtype=np.float32)
    inw = np.asarray(inputs["in_proj_w"], dtype=np.float32)
    inb = np.asarray(inputs["in_proj_b"], dtype=np.float32)
    wo = np.asarray(inputs["out_proj_w"], dtype=np.float32)
    bo = np.asarray(inputs["out_proj_b"], dtype=np.float32)
    w1 = np.asarray(inputs["lin1_w"], dtype=np.float32)
    b1 = np.asarray(inputs["lin1_b"], dtype=np.float32)
    w2 = np.asarray(inputs["lin2_w"], dtype=np.float32)
    b2 = np.asarray(inputs["lin2_b"], dtype=np.float32)
    c1w = np.asarray(inputs["conv1_w"], dtype=np.float32)
    c2w = np.asarray(inputs["conv2_w"], dtype=np.float32)
    fcw = np.asarray(inputs["fc_w"], dtype=np.float32)

    pembp = np.zeros((10, D), np.float32)
    pembp[:, ED:] = pemb
    embp = np.zeros((NCAT, D), np.float32)
    embp[:, :ED] = emb

    clo, chi = rpe[0], rpe[2 * MD]
    bandcat = np.zeros((D, H * BAND_TOT), np.float32)
    jj = np.arange(128)[:, None]
    for h in range(H):
        for di, dl in enumerate(BAND_DELTAS):
            w = BAND_W[di]
            ii = np.arange(BAND_C0[di], BAND_C0[di] + w)[None, :]
            e = dl + jj - ii
            val = rpe[np.clip(e, -MD, MD) + MD, h]
            beta = chi[h] if di == 5 else clo[h]
            bandcat[:, h * BAND_TOT + BAND_OFF[di]:
                    h * BAND_TOT + BAND_OFF[di] + w] = val - beta
    cexp = np.zeros((D, 2 * H), np.float32)
    for h in range(H):
        cexp[:, 2 * h] = clo[h]
        cexp[:, 2 * h + 1] = chi[h]

    ob_eff = bo + wo @ inb[2 * D:3 * D]
    l2t = w2.T   # [FF, D]
    shared = {
        "embp": embp, "pembp": pembp,
        "iota25": np.arange(NCAT, dtype=np.float32)[:, None],
        "iota10": np.arange(10, dtype=np.float32)[:, None],
        "inwT": np.ascontiguousarray(inw.T),
        "qb": (inb[0:D] * ISQ)[:, None],
        "kb": inb[D:2 * D][:, None],
        "woT": np.ascontiguousarray(wo.T),
        "ob_eff": ob_eff[:, None].astype(np.float32),
        "l1wT": np.ascontiguousarray(w1.T),
        "l1b": b1.reshape(2, D).T.copy(),
        "l2cat": np.concatenate([l2t[0:D], l2t[D:2 * D]], axis=1).copy(),
        "l2b": b2[:, None],
        "ln1g": np.asarray(inputs["ln1_g"], np.float32)[:, None],
        "ln1b": np.asarray(inputs["ln1_b"], np.float32)[:, None],
        "ln2g": np.asarray(inputs["ln2_g"], np.float32)[:, None],
        "ln2b": np.asarray(inputs["ln2_b"], np.float32)[:, None],
        "bandcat": bandcat, "cexp": cexp,
        "c1wT": np.ascontiguousarray(c1w.transpose(1, 2, 0).reshape(D, K * C1)),
        "c2wT": np.ascontiguousarray(c2w.transpose(1, 2, 0).reshape(C1, K * C2)),
        "bn1g": np.asarray(inputs["bn1_g"], np.float32)[:, None],
        "bn1b": np.asarray(inputs["bn1_b"], np.float32)[:, None],
        "bn2g": np.asarray(inputs["bn2_g"], np.float32).reshape(2, C1).T.copy(),
        "bn2b": np.asarray(inputs["bn2_b"], np.float32).reshape(2, C1).T.copy(),
        "fcwT": np.ascontiguousarray(
            fcw.T.reshape(2, C1, NC).transpose(1, 0, 2).reshape(C1, 2 * NC)),
        "fcb": np.asarray(inputs["fc_b"], np.float32)[:, None],
    }
    in_maps = []
    for c in range(n_cores):
        rows = slice(c * bloc, (c + 1) * bloc)
        m = dict(shared)
        m["Xf"] = np.ascontiguousarray(X[rows].reshape(-1))
        m["saf"] = np.ascontiguousarray(sa[rows].reshape(-1))
        m["ptmf"] = np.ascontiguousarray(ptm[rows].reshape(-1))
        in_maps.append(m)
    return in_maps


_NC_CACHE = {}


def _get_nc(n_cores, bloc, lp, dbg=False):
    key = (n_cores, bloc, lp, dbg)
    if key not in _NC_CACHE:
        _NC_CACHE[key] = _build(n_cores, bloc, lp, dbg=dbg)
    return _NC_CACHE[key]


def kernel(**inputs):
    from concourse.bass_utils import run_bass_kernel_spmd
    nc = _get_nc(NCORES, BLOC, L)
    in_maps = _host_inputs(inputs, NCORES, BLOC, L)
    res = run_bass_kernel_spmd(nc, in_maps, list(range(NCORES)))
    out = np.concatenate([res.results[i]["out"] for i in range(NCORES)], axis=0)
    return out.astype(np.float32)


# revision 17
# speedup vs baseline: 1.6487x; 1.4570x over previous
"""Trainium2 Bass kernel for nn_CNN2LWithRPE (transformer layer + CNN head).

Sharding: data-parallel over batch across 8 NeuronCores (2 batch rows each).
All parameters replicated. The only cross-core communication is two tiny
AllReduces for the training-mode BatchNorm statistics.

Per-core layout (B_loc batches, T = B_loc*L tokens):
  - activations transposed in SBUF as bf16: xT/qT/x1T... are [D=128, T]
    (bf16 operands run the PE at 1 cycle/row vs 4 for fp32; PSUM
    accumulation stays fp32)
  - attention as scores^T tiles [keys=128 part, queries=512 free]:
    QK^T row-packed 4x on PE (K=32) from a shuffled kTp layout + a 4x
    replicated qrep tile; exp on ACT as wide [128,2048] ops with the
    clipped-RPE bias folded into a per-group bias constant (c_lo/c_hi)
    plus narrow host-precomputed banded correction tiles added on DVE;
    PV uses v in natural layout with an appended ones-column so the
    softmax denominator falls out of the same matmul.
  - layernorm in transposed layout: partition stats via ones-matmul,
    rstd = Exp(-0.5*Ln(var+eps)), rank-1 K=1 matmul broadcast back.
  - conv1d as K accumulating shifted matmuls; BN apply fused into
    ACT Relu(scale*x+shift) with per-channel scale/shift APs.
"""

import numpy as np

B, L = 16, 2048
NCAT, ED = 25, 120
D, H, HD = 128, 4, 32
FF = 256
MD = 32
C1, C2, K = 128, 256, 5
NC = 2
EPS = 1e-5
NCORES = 8
BLOC = B // NCORES
ISQ = float(1.0 / np.sqrt(HD))

QT = 512
KTILE = 128

BAND_DELTAS = [-128, 0, 128, 256, 384, 512]
BAND_W = [32, 160, 288, 416, 512, 32]
BAND_C0 = [0, 0, 0, 0, 0, 480]
BAND_OFF = [0, 32, 192, 480, 896, 1408]
BAND_TOT = 1440


def _build(n_cores, bloc, lp, dbg=False):
    import contextlib
    import concourse.bass as bass
    import concourse.tile as tile
    from concourse import bacc, mybir

    f32 = mybir.dt.float32
    bf16 = mybir.dt.bfloat16
    AF = mybir.ActivationFunctionType
    OP = mybir.AluOpType
    AX = mybir.AxisListType

    T = bloc * lp
    NET = T // 512
    NQT = lp // QT
    NKT = lp // KTILE
    NG = NKT // 4
    NLT = lp // 512
    NDEN = bloc * NQT * H
    LP2 = lp // 2
    L2 = LP2 - (K - 1)
    LT2 = L2 // 2
    n1 = float(n_cores * bloc * lp)
    n2 = float(n_cores * bloc * L2)

    nc = bacc.Bacc("TRN2", target_bir_lowering=False, debug=False,
                   num_devices=n_cores)

    def din(name, shape):
        return nc.dram_tensor(name, list(shape), f32, kind="ExternalInput")

    Xf = din("Xf", [T])
    saf = din("saf", [T])
    ptmf = din("ptmf", [T])
    emb_d = din("embp", [NCAT, D])
    pemb_d = din("pembp", [10, D])
    iota25_d = din("iota25", [NCAT, 1])
    iota10_d = din("iota10", [10, 1])
    inwT_d = din("inwT", [D, 3 * D])
    qb_d = din("qb", [D, 1])
    kb_d = din("kb", [D, 1])
    woT_d = din("woT", [D, D])
    ob_d = din("ob_eff", [D, 1])
    l1wT_d = din("l1wT", [D, FF])
    l1b_d = din("l1b", [D, 2])
    l2cat_d = din("l2cat", [D, 2 * D])
    l2b_d = din("l2b", [D, 1])
    ln1g_d = din("ln1g", [D, 1])
    ln1b_d = din("ln1b", [D, 1])
    ln2g_d = din("ln2g", [D, 1])
    ln2b_d = din("ln2b", [D, 1])
    band_d = din("bandcat", [D, H * BAND_TOT])
    cexp_d = din("cexp", [D, 2 * H])
    c1wT_d = din("c1wT", [D, K * C1])
    c2wT_d = din("c2wT", [C1, K * C2])
    bn1g_d = din("bn1g", [C1, 1])
    bn1b_d = din("bn1b", [C1, 1])
    bn2g_d = din("bn2g", [C1, 2])
    bn2b_d = din("bn2b", [C1, 2])
    fcwT_d = din("fcwT", [C1, 2 * NC])
    fcb_d = din("fcb", [NC, 1])

    out_d = nc.dram_tensor("out", [bloc, NC], f32, kind="ExternalOutput")
    dbg_outs = {}
    if dbg:
        for nm, shp, dt_ in [("dbg_xT", [D, T], bf16),
                             ("dbg_qT", [D, T], bf16),
                             ("dbg_kT", [D, T], bf16),
                             ("dbg_attnT", [D, T], bf16),
                             ("dbg_x1T", [D, T], bf16),
                             ("dbg_x2", [D, T], bf16),
                             ("dbg_c1", [C1, bloc * lp], bf16),
                             ("dbg_p1", [C1, bloc * LP2], bf16),
                             ("dbg_feat", [C1, 2 * bloc], f32)]:
            dbg_outs[nm] = nc.dram_tensor(nm, shp, dt_, kind="ExternalOutput")

    def bc(ap1d, parts):
        return bass.AP(tensor=ap1d.tensor, offset=ap1d.offset,
                       ap=[[0, parts]] + [list(p) for p in ap1d.ap])

    with tile.TileContext(nc) as tc:
        ctx = contextlib.ExitStack()
        with ctx:
            pp = ctx.enter_context(tc.tile_pool(name="params", bufs=1))
            big = ctx.enter_context(tc.tile_pool(name="big", bufs=1))
            wk = ctx.enter_context(tc.tile_pool(name="wk", bufs=12))
            wkb = ctx.enter_context(tc.tile_pool(name="wkb", bufs=2))
            longs = ctx.enter_context(tc.tile_pool(name="longs", bufs=1))
            mid = ctx.enter_context(tc.tile_pool(name="mid", bufs=6))
            rowp = ctx.enter_context(tc.tile_pool(name="rows", bufs=4))
            ptp = ctx.enter_context(tc.tile_pool(name="pt", bufs=2))
            qrp = ctx.enter_context(tc.tile_pool(name="qrep", bufs=1))
            ps_sc = ctx.enter_context(tc.tile_pool(name="ps_sc", bufs=2, space="PSUM"))
            ps_sm = ctx.enter_context(tc.tile_pool(name="ps_sm", bufs=4, space="PSUM"))
            dram = ctx.enter_context(tc.tile_pool(name="dram", bufs=1, space="DRAM"))

            def wkt(p=D, f=512):
                return wk.tile([p, f], f32, tag="wk", name="wkt")

            def midt(p, f):
                return mid.tile([p, f], f32, tag="mid", name="midt")

            # ---- params ----
            def ld(dten, shape):
                t = pp.tile(list(shape), f32, tag=dten.name, name=dten.name + "_s")
                nc.sync.dma_start(out=t, in_=dten[tuple(slice(0, s) for s in shape)])
                return t

            def ldb(dten, shape):
                # load f32 param, convert to a bf16 copy for PE consumption
                t = ld(dten, shape)
                tb = pp.tile(list(shape), bf16, tag=dten.name + "_b",
                             name=dten.name + "_b")
                nc.vector.tensor_copy(tb, t)
                return tb

            emb_b = ldb(emb_d, [NCAT, D])
            pemb_b = ldb(pemb_d, [10, D])
            io25 = ld(iota25_d, [NCAT, 1])
            io10 = ld(iota10_d, [10, 1])
            inwT_b = ldb(inwT_d, [D, 3 * D])
            qb_s = ld(qb_d, [D, 1])
            kb_s = ld(kb_d, [D, 1])
            woT_b = ldb(woT_d, [D, D])
            ob_s = ld(ob_d, [D, 1])
            l1wT_b = ldb(l1wT_d, [D, FF])
            l1b_s = ld(l1b_d, [D, 2])
            l2cat_b = ldb(l2cat_d, [D, 2 * D])
            l2b_s = ld(l2b_d, [D, 1])
            ln1g = ld(ln1g_d, [D, 1]); ln1b = ld(ln1b_d, [D, 1])
            ln2g = ld(ln2g_d, [D, 1]); ln2b = ld(ln2b_d, [D, 1])
            cexp = ld(cexp_d, [D, 2 * H])
            c1wT_b = ldb(c1wT_d, [D, K * C1])
            c2wT_b = ldb(c2wT_d, [C1, K * C2])
            bn1g = ld(bn1g_d, [C1, 1]); bn1b = ld(bn1b_d, [C1, 1])
            bn2g = ld(bn2g_d, [C1, 2]); bn2b = ld(bn2b_d, [C1, 2])
            fcwT = ld(fcwT_d, [C1, 2 * NC])
            fcb_s = ld(fcb_d, [NC, 1])

            band = big.tile([D, H * BAND_TOT], f32, tag="chF")
            nc.sync.dma_start(out=band, in_=band_d[:, :])

            ones128b = pp.tile([D, 1], bf16, tag="ones128b")
            nc.vector.memset(ones128b, 1.0)
            ones_rb = pp.tile([1, D], bf16, tag="ones_rb")
            nc.vector.memset(ones_rb, 1.0)
            ones32b = pp.tile([1, HD], bf16, tag="ones32b")
            nc.vector.memset(ones32b, 1.0)
            epsNLT = pp.tile([NLT, 1], f32, tag="epsNLT")
            nc.vector.memset(epsNLT, EPS)
            eps128 = pp.tile([D, 1], f32, tag="eps128")
            nc.vector.memset(eps128, EPS)

            # ---- persistent activations (bf16) ----
            xT = big.tile([D, T], bf16, tag="chA")
            qT = big.tile([D, T], bf16, tag="chE")
            kTp = big.tile([D, bloc, H, NG, KTILE], bf16, tag="chB")
            v_sb = big.tile([D, T // KTILE, H, HD + 1], bf16, tag="chC")
            attnT = big.tile([D, T], bf16, tag="chD")
            x1T = big.tile([D, T], bf16, tag="chG")
            nc.vector.memset(v_sb[:, :, :, HD:HD + 1], 1.0)

            # ================= embedding =================
            for e in range(NET):
                sl = slice(e * 512, (e + 1) * 512)
                xb = wkt(NCAT)
                nc.sync.dma_start(out=xb, in_=bc(Xf[sl], NCAT))
                sb_ = wkt(NCAT)
                nc.sync.dma_start(out=sb_, in_=bc(saf[sl], NCAT))
                oh = wk.tile([NCAT, 512], bf16, tag="wk", name="oh")
                nc.vector.tensor_scalar(out=oh, in0=xb, scalar1=io25,
                                        scalar2=None, op0=OP.is_equal)
                nc.vector.tensor_mul(oh, oh, sb_)
                pb = wkt(10)
                nc.sync.dma_start(out=pb, in_=bc(ptmf[sl], 10))
                ohp = wk.tile([10, 512], bf16, tag="wk", name="ohp")
                nc.vector.tensor_scalar(out=ohp, in0=pb, scalar1=io10,
                                        scalar2=None, op0=OP.is_equal)
                pe = ps_sm.tile([D, 512], f32, tag="sm")
                nc.tensor.matmul(pe, pemb_b, ohp, start=True, stop=False)
                nc.tensor.matmul(pe, emb_b, oh, start=False, stop=True)
                nc.vector.tensor_copy(xT[:, sl], pe)

            if dbg:
                nc.sync.dma_start(out=dbg_outs["dbg_xT"][:, :], in_=xT)

            # ================= qkv =================
            for e in range(NET):
                sl = slice(e * 512, (e + 1) * 512)
                pq = ps_sm.tile([D, 512], f32, tag="sm")
                nc.tensor.matmul(pq, inwT_b[:, 0:D], xT[:, sl],
                                 start=True, stop=True)
                nc.vector.tensor_scalar(out=qT[:, sl], in0=pq, scalar1=ISQ,
                                        scalar2=qb_s, op0=OP.mult, op1=OP.add)
                pk = ps_sm.tile([D, 512], f32, tag="sm")
                nc.tensor.matmul(pk, inwT_b[:, D:2 * D], xT[:, sl],
                                 start=True, stop=True)
                ktmp = mid.tile([D, 512], bf16, tag="mid", name="ktmp")
                nc.vector.tensor_scalar(out=ktmp, in0=pk, scalar1=kb_s,
                                        scalar2=None, op0=OP.add)
                b_ = (e * 512) // lp
                for h in range(H):
                    for sub in range(4):
                        ktb = ((e * 512) % lp) // KTILE + sub
                        nc.gpsimd.dma_start(
                            out=kTp[32 * (ktb % 4):32 * (ktb % 4) + 32,
                                    b_, h, ktb // 4, :],
                            in_=ktmp[32 * h:32 * h + 32,
                                     sub * KTILE:(sub + 1) * KTILE])
                for sub in range(4):
                    tt = (e * 512) // KTILE + sub
                    pv = ps_sm.tile([KTILE, D], f32, tag="sm")
                    nc.tensor.matmul(pv, xT[:, e * 512 + sub * KTILE:
                                            e * 512 + (sub + 1) * KTILE],
                                     inwT_b[:, 2 * D:3 * D], start=True, stop=True)
                    nc.vector.tensor_copy(
                        v_sb[:, tt, :, 0:HD],
                        pv.rearrange("p (h d) -> p h d", h=H))

            if dbg:
                nc.sync.dma_start(out=dbg_outs["dbg_qT"][:, :], in_=qT)
                kT_dbg = big.tile([D, T], bf16, tag="chDBG")
                for b_ in range(bloc):
                    for h in range(H):
                        for g in range(NG):
                            for r in range(4):
                                kt = 4 * g + r
                                nc.sync.dma_start(
                                    out=kT_dbg[32 * h:32 * h + 32,
                                               b_ * lp + kt * KTILE:
                                               b_ * lp + (kt + 1) * KTILE],
                                    in_=kTp[32 * r:32 * r + 32, b_, h, g, :])
                nc.sync.dma_start(out=dbg_outs["dbg_kT"][:, :], in_=kT_dbg)

            # ================= attention =================
            # 2 key-tiles per subgroup so sc fits 2 PSUM banks and can be
            # double-buffered: QK/band of subgroup p+1 overlap exp of p,
            # PV of p overlaps exp of p+1.
            den32 = longs.tile([NDEN, 512], bf16, tag="den32")
            for b_ in range(bloc):
                for h in range(H):
                    qrep = qrp.tile([D, lp], bf16, tag="qr")
                    for r in range(4):
                        nc.gpsimd.dma_start(
                            out=qrep[32 * r:32 * r + 32, :],
                            in_=qT[32 * h:32 * h + 32, b_ * lp:(b_ + 1) * lp])
                    for qt in range(NQT):
                        ppv = ps_sm.tile([HD + 1, 512], f32, tag="sm")
                        for p_ in range(2 * NG):
                            sc = ps_sc.tile([D, 2 * 512], f32, tag="sc")
                            for j in range(2):
                                kt = 2 * p_ + j
                                nc.tensor.matmul(
                                    sc[:, j * 512:(j + 1) * 512],
                                    kTp[32 * (kt % 4):32 * (kt % 4) + 32,
                                        b_, h, kt // 4, :],
                                    qrep[32 * (kt % 4):32 * (kt % 4) + 32,
                                         qt * QT:(qt + 1) * QT],
                                    start=True, stop=True,
                                    tile_position=(32 * (kt % 4), 0))
                            for j in range(2):
                                kt = 2 * p_ + j
                                dd = kt - 4 * qt
                                if -1 <= dd <= 4:
                                    di = dd + 1
                                    c0, w = BAND_C0[di], BAND_W[di]
                                    nc.vector.tensor_tensor(
                                        out=sc[:, j * 512 + c0:j * 512 + c0 + w],
                                        in0=sc[:, j * 512 + c0:j * 512 + c0 + w],
                                        in1=band[:, h * BAND_TOT + BAND_OFF[di]:
                                                 h * BAND_TOT + BAND_OFF[di] + w],
                                        op=OP.add)
                            side = 0 if (p_ >> 1) <= qt else 1
                            pt = ptp.tile([D, 2 * 512], bf16, tag="pt")
                            nc.scalar.activation(pt, sc, AF.Exp,
                                                 bias=cexp[:, 2 * h + side:
                                                           2 * h + side + 1],
                                                 scale=1.0)
                            for j in range(2):
                                kt = 2 * p_ + j
                                nc.tensor.matmul(
                                    ppv, v_sb[:, b_ * NKT + kt, h, :],
                                    pt[:, j * 512:(j + 1) * 512],
                                    start=(kt == 0), stop=(kt == NKT - 1))
                        pv_sb = wk.tile([HD + 1, 512], bf16, tag="wk", name="pv_sb")
                        nc.vector.tensor_copy(pv_sb, ppv)
                        nc.gpsimd.dma_start(
                            out=attnT[32 * h:32 * h + 32,
                                      b_ * lp + qt * QT:b_ * lp + (qt + 1) * QT],
                            in_=pv_sb[0:HD, :])
                        nc.gpsimd.dma_start(
                            out=den32[(b_ * NQT + qt) * H + h:
                                      (b_ * NQT + qt) * H + h + 1, :],
                            in_=pv_sb[HD:HD + 1, :])

            lnden = midt(NDEN, 512)
            nc.scalar.activation(lnden, den32, AF.Ln, bias=0.0, scale=1.0)
            recip = longs.tile([NDEN, 512], bf16, tag="recip")
            nc.scalar.activation(recip, lnden, AF.Exp, bias=0.0, scale=-1.0)

            for b_ in range(bloc):
                for qt in range(NQT):
                    bcp = ps_sm.tile([D, 512], f32, tag="sm")
                    for h in range(H):
                        rr = rowp.tile([1, 512], bf16, tag="row")
                        nc.gpsimd.dma_start(
                            out=rr, in_=recip[(b_ * NQT + qt) * H + h:
                                              (b_ * NQT + qt) * H + h + 1, :])
                        nc.tensor.matmul(bcp[32 * h:32 * h + 32, :], ones32b, rr,
                                         start=True, stop=True,
                                         tile_position=(0, 32 * h))
                    sl = slice(b_ * lp + qt * QT, b_ * lp + (qt + 1) * QT)
                    nc.vector.tensor_mul(attnT[:, sl], attnT[:, sl], bcp)

            if dbg:
                nc.sync.dma_start(out=dbg_outs["dbg_attnT"][:, :], in_=attnT)

            # ======== layernorm helper (bf16 transposed layout) ========
            def layernorm_T(src, dst_fn, g_s, b_s, b_base):
                s1c = midt(NLT, 512)
                s2c = midt(NLT, 512)
                for t_ in range(NLT):
                    sl = slice(b_base * lp + t_ * 512, b_base * lp + (t_ + 1) * 512)
                    sq = wk.tile([D, 512], bf16, tag="wk", name="sq")
                    nc.vector.tensor_mul(sq, src[:, sl], src[:, sl])
                    p1_ = ps_sm.tile([1, 512], f32, tag="sm")
                    nc.tensor.matmul(p1_, ones128b, src[:, sl], start=True, stop=True)
                    p2_ = ps_sm.tile([1, 512], f32, tag="sm")
                    nc.tensor.matmul(p2_, ones128b, sq, start=True, stop=True)
                    s1t = rowp.tile([1, 512], f32, tag="row", name="s1t")
                    nc.vector.tensor_copy(s1t, p1_)
                    s2t = rowp.tile([1, 512], f32, tag="row", name="s2t")
                    nc.vector.tensor_copy(s2t, p2_)
                    nc.gpsimd.dma_start(out=s1c[t_:t_ + 1, :], in_=s1t)
                    nc.gpsimd.dma_start(out=s2c[t_:t_ + 1, :], in_=s2t)
                m_ = wkt(NLT)
                nc.vector.tensor_scalar(out=m_, in0=s1c, scalar1=1.0 / D,
                                        scalar2=None, op0=OP.mult)
                var = wkt(NLT)
                nc.vector.tensor_scalar(out=var, in0=s2c, scalar1=1.0 / D,
                                        scalar2=None, op0=OP.mult)
                msq = wkt(NLT)
                nc.vector.tensor_mul(msq, m_, m_)
                nc.vector.tensor_tensor(out=var, in0=var, in1=msq, op=OP.subtract)
                lnv = wkt(NLT)
                nc.scalar.activation(lnv, var, AF.Ln, bias=epsNLT[0:NLT, :],
                                     scale=1.0)
                rstd = wk.tile([NLT, 512], bf16, tag="wk", name="rstd")
                nc.scalar.activation(rstd, lnv, AF.Exp, bias=0.0, scale=-0.5)
                mr = wk.tile([NLT, 512], bf16, tag="wk", name="mr")
                nc.vector.tensor_mul(mr, m_, rstd)
                for t_ in range(NLT):
                    sl = slice(b_base * lp + t_ * 512, b_base * lp + (t_ + 1) * 512)
                    rr = rowp.tile([1, 512], bf16, tag="row")
                    nc.gpsimd.dma_start(out=rr, in_=rstd[t_:t_ + 1, :])
                    rm = rowp.tile([1, 512], bf16, tag="row")
                    nc.gpsimd.dma_start(out=rm, in_=mr[t_:t_ + 1, :])
                    br = ps_sm.tile([D, 512], f32, tag="sm")
                    nc.tensor.matmul(br, ones_rb, rr, start=True, stop=True)
                    bm = ps_sm.tile([D, 512], f32, tag="sm")
                    nc.tensor.matmul(bm, ones_rb, rm, start=True, stop=True)
                    tmp = wkt()
                    nc.vector.tensor_mul(tmp, src[:, sl], br)
                    nc.vector.tensor_tensor(out=tmp, in0=tmp, in1=bm,
                                            op=OP.subtract)
                    nc.vector.tensor_scalar(out=dst_fn(t_), in0=tmp, scalar1=g_s,
                                            scalar2=b_s, op0=OP.mult, op1=OP.add)

            # ================= out-proj + residual + LN1 =================
            r1T = big.tile([D, T], bf16, tag="chB")   # after kTp's last read
            for b_ in range(bloc):
                for qt in range(NQT):
                    sl = slice(b_ * lp + qt * QT, b_ * lp + (qt + 1) * QT)
                    po = ps_sm.tile([D, 512], f32, tag="sm")
                    nc.tensor.matmul(po, woT_b, attnT[:, sl], start=True, stop=True)
                    nc.vector.tensor_scalar(out=r1T[:, sl], in0=po, scalar1=ob_s,
                                            scalar2=None, op0=OP.add)
                    nc.vector.tensor_tensor(out=r1T[:, sl], in0=r1T[:, sl],
                                            in1=xT[:, sl], op=OP.add)
            for b_ in range(bloc):
                layernorm_T(
                    r1T,
                    lambda t_, b0=b_: x1T[:, b0 * lp + t_ * 512:
                                          b0 * lp + (t_ + 1) * 512],
                    ln1g, ln1b, b_)

            if dbg:
                nc.sync.dma_start(out=dbg_outs["dbg_x1T"][:, :], in_=x1T)

            # ================= FFN + residual + LN2 =================
            x2pad = big.tile([D, bloc * (lp + 4)], bf16, tag="chA")  # after xT
            nc.vector.memset(x2pad[:, :], 0.0)
            r2T = big.tile([D, T], bf16, tag="chC")                  # after v_sb
            for b_ in range(bloc):
                for qt in range(NQT):
                    sl = slice(b_ * lp + qt * QT, b_ * lp + (qt + 1) * QT)
                    h1a = wk.tile([D, 512], bf16, tag="wk", name="h1a")
                    h1b = wk.tile([D, 512], bf16, tag="wk", name="h1b")
                    for half, dest in ((0, h1a), (1, h1b)):
                        ph = ps_sm.tile([D, 512], f32, tag="sm")
                        nc.tensor.matmul(ph, l1wT_b[:, half * D:(half + 1) * D],
                                         x1T[:, sl], start=True, stop=True)
                        nc.scalar.activation(dest, ph, AF.Relu,
                                             bias=l1b_s[:, half:half + 1],
                                             scale=1.0)
                    py = ps_sm.tile([D, 512], f32, tag="sm")
                    nc.tensor.matmul(py, l2cat_b[:, 0:D], h1a, start=True, stop=False)
                    nc.tensor.matmul(py, l2cat_b[:, D:2 * D], h1b,
                                     start=False, stop=True)
                    nc.vector.tensor_scalar(out=r2T[:, sl], in0=py, scalar1=l2b_s,
                                            scalar2=None, op0=OP.add)
                    nc.vector.tensor_tensor(out=r2T[:, sl], in0=r2T[:, sl],
                                            in1=x1T[:, sl], op=OP.add)
            for b_ in range(bloc):
                layernorm_T(
                    r2T,
                    lambda t_, b0=b_: x2pad[:, b0 * (lp + 4) + 2 + t_ * 512:
                                            b0 * (lp + 4) + 2 + (t_ + 1) * 512],
                    ln2g, ln2b, b_)

            if dbg:
                for b_ in range(bloc):
                    nc.sync.dma_start(
                        out=dbg_outs["dbg_x2"][:, b_ * lp:(b_ + 1) * lp],
                        in_=x2pad[:, b_ * (lp + 4) + 2:b_ * (lp + 4) + 2 + lp])

            # ================= conv1 + bn1 =================
            c1_sb = big.tile([C1, bloc * lp], bf16, tag="chD")   # after attnT
            bnst1 = longs.tile([C1, bloc * NLT, 6], f32, tag="bnst1")
            for b_ in range(bloc):
                for t_ in range(NLT):
                    pc = ps_sm.tile([C1, 512], f32, tag="sm")
                    for k_ in range(K):
                        nc.tensor.matmul(
                            pc, c1wT_b[:, k_ * C1:(k_ + 1) * C1],
                            x2pad[:, b_ * (lp + 4) + t_ * 512 + k_:
                                  b_ * (lp + 4) + t_ * 512 + k_ + 512],
                            start=(k_ == 0), stop=(k_ == K - 1))
                    nc.vector.bn_stats(out=bnst1[:, b_ * NLT + t_, :], in_=pc)
                    nc.vector.tensor_copy(
                        c1_sb[:, b_ * lp + t_ * 512:b_ * lp + (t_ + 1) * 512], pc)
            mv1 = wk.tile([C1, 2], f32, tag="wk")
            nc.vector.bn_aggr(out=mv1, in_=bnst1)
            part1 = wk.tile([C1, 2], f32, tag="wk")
            sqm = wk.tile([C1, 1], f32, tag="wk")
            nc.vector.tensor_mul(sqm, mv1[:, 0:1], mv1[:, 0:1])
            nc.vector.tensor_tensor(out=sqm, in0=sqm, in1=mv1[:, 1:2], op=OP.add)
            nl_ = float(bloc * lp)
            nc.vector.tensor_scalar(out=part1[:, 0:1], in0=mv1[:, 0:1],
                                    scalar1=nl_, scalar2=None, op0=OP.mult)
            nc.vector.tensor_scalar(out=part1[:, 1:2], in0=sqm,
                                    scalar1=nl_, scalar2=None, op0=OP.mult)
            bn1_in = dram.tile([C1, 2], f32, tag="bn1i")
            bn1_out = dram.tile([C1, 2], f32, tag="bn1o")
            nc.sync.dma_start(out=bn1_in, in_=part1)
            nc.gpsimd.collective_compute(
                "AllReduce", OP.add, replica_groups=[list(range(n_cores))],
                ins=[bn1_in[:, :].opt()], outs=[bn1_out[:, :].opt()])
            glob1 = wk.tile([C1, 2], f32, tag="wk")
            nc.sync.dma_start(out=glob1, in_=bn1_out)

            def bn_scale_shift(globc, n_, g_ap, b_ap):
                mean = wk.tile([C1, 1], f32, tag="wk")
                nc.vector.tensor_scalar(out=mean, in0=globc[:, 0:1],
                                        scalar1=1.0 / n_, scalar2=None, op0=OP.mult)
                ex2 = wk.tile([C1, 1], f32, tag="wk")
                nc.vector.tensor_scalar(out=ex2, in0=globc[:, 1:2],
                                        scalar1=1.0 / n_, scalar2=None, op0=OP.mult)
                msq_ = wk.tile([C1, 1], f32, tag="wk")
                nc.vector.tensor_mul(msq_, mean, mean)
                nc.vector.tensor_tensor(out=ex2, in0=ex2, in1=msq_, op=OP.subtract)
                lnv_ = wk.tile([C1, 1], f32, tag="wk")
                nc.scalar.activation(lnv_, ex2, AF.Ln, bias=eps128, scale=1.0)
                rstd_ = wk.tile([C1, 1], f32, tag="wk")
                nc.scalar.activation(rstd_, lnv_, AF.Exp, bias=0.0, scale=-0.5)
                scale = longs.tile([C1, 1], f32, tag="bnsc")
                nc.vector.tensor_mul(scale, rstd_, g_ap)
                shift = longs.tile([C1, 1], f32, tag="bnsh")
                nc.vector.tensor_mul(shift, mean, scale)
                nc.vector.tensor_tensor(out=shift, in0=b_ap, in1=shift,
                                        op=OP.subtract)
                return scale, shift

            sc1, sh1 = bn_scale_shift(glob1, n1, bn1g, bn1b)
            p1_sb = big.tile([C1, bloc * LP2], bf16, tag="chF")   # after band
            if dbg:
                nc.sync.dma_start(out=dbg_outs["dbg_c1"][:, :], in_=c1_sb)
            for b_ in range(bloc):
                for t_ in range(NLT):
                    rel = wk.tile([D, 512], bf16, tag="wk", name="rel")
                    nc.scalar.activation(
                        rel, c1_sb[:, b_ * lp + t_ * 512:b_ * lp + (t_ + 1) * 512],
                        AF.Relu, bias=sh1, scale=sc1)
                    rel2 = rel.rearrange("p (l two) -> p l two", two=2)
                    nc.vector.tensor_tensor(
                        out=p1_sb[:, b_ * LP2 + t_ * 256:b_ * LP2 + (t_ + 1) * 256],
                        in0=rel2[:, :, 0], in1=rel2[:, :, 1], op=OP.max)
            if dbg:
                nc.sync.dma_start(out=dbg_outs["dbg_p1"][:, :], in_=p1_sb)

            # ================= conv2 + bn2 =================
            c2_sb = big.tile([C1, bloc * 2 * L2], bf16, tag="chE")  # after qT
            bnst2 = longs.tile([C1, 2, bloc * 2, 6], f32, tag="bnst2")
            for b_ in range(bloc):
                for half in range(2):
                    for t_ in range(2):
                        pc = ps_sm.tile([C1, LT2], f32, tag="sm")
                        for k_ in range(K):
                            nc.tensor.matmul(
                                pc, c2wT_b[:, k_ * C2 + half * C1:
                                           k_ * C2 + (half + 1) * C1],
                                p1_sb[:, b_ * LP2 + t_ * LT2 + k_:
                                      b_ * LP2 + t_ * LT2 + k_ + LT2],
                                start=(k_ == 0), stop=(k_ == K - 1))
                        nc.vector.bn_stats(out=bnst2[:, half, b_ * 2 + t_, :],
                                           in_=pc)
                        nc.vector.tensor_copy(
                            c2_sb[:, (b_ * 2 + half) * L2 + t_ * LT2:
                                  (b_ * 2 + half) * L2 + (t_ + 1) * LT2], pc)
            part2 = longs.tile([C1, 4], f32, tag="part2")
            for half in range(2):
                mv2 = wk.tile([C1, 2], f32, tag="wk")
                nc.vector.bn_aggr(out=mv2, in_=bnst2[:, half, :, :])
                sqm2 = wk.tile([C1, 1], f32, tag="wk")
                nc.vector.tensor_mul(sqm2, mv2[:, 0:1], mv2[:, 0:1])
                nc.vector.tensor_tensor(out=sqm2, in0=sqm2, in1=mv2[:, 1:2],
                                        op=OP.add)
                nl2 = float(bloc * L2)
                nc.vector.tensor_scalar(out=part2[:, 2 * half:2 * half + 1],
                                        in0=mv2[:, 0:1], scalar1=nl2,
                                        scalar2=None, op0=OP.mult)
                nc.vector.tensor_scalar(out=part2[:, 2 * half + 1:2 * half + 2],
                                        in0=sqm2, scalar1=nl2,
                                        scalar2=None, op0=OP.mult)
            bn2_in = dram.tile([C1, 4], f32, tag="bn2i")
            bn2_out = dram.tile([C1, 4], f32, tag="bn2o")
            nc.sync.dma_start(out=bn2_in, in_=part2)
            nc.gpsimd.collective_compute(
                "AllReduce", OP.add, replica_groups=[list(range(n_cores))],
                ins=[bn2_in[:, :].opt()], outs=[bn2_out[:, :].opt()])
            glob2 = longs.tile([C1, 4], f32, tag="glob2")
            nc.sync.dma_start(out=glob2, in_=bn2_out)

            feat = longs.tile([C1, 2 * bloc], f32, tag="feat")
            for half in range(2):
                sc2, sh2 = bn_scale_shift(glob2[:, 2 * half:2 * half + 2], n2,
                                          bn2g[:, half:half + 1],
                                          bn2b[:, half:half + 1])
                for b_ in range(bloc):
                    rel = wkb.tile([C1, L2], bf16, tag="wkb")
                    nc.scalar.activation(
                        rel, c2_sb[:, (b_ * 2 + half) * L2:
                                   (b_ * 2 + half + 1) * L2],
                        AF.Relu, bias=sh2, scale=sc2)
                    nc.vector.reduce_max(
                        out=feat[:, (b_ * 2 + half):(b_ * 2 + half) + 1],
                        in_=rel, axis=AX.X)
            if dbg:
                nc.sync.dma_start(out=dbg_outs["dbg_feat"][:, :], in_=feat)

            # ================= fc =================
            for b_ in range(bloc):
                pf = ps_sm.tile([NC, 1], f32, tag="sm")
                for half in range(2):
                    nc.tensor.matmul(pf, fcwT[:, half * NC:(half + 1) * NC],
                                     feat[:, b_ * 2 + half:b_ * 2 + half + 1],
                                     start=(half == 0), stop=(half == 1))
                ob2 = wk.tile([NC, 1], f32, tag="wk")
                nc.vector.tensor_scalar(out=ob2, in0=pf, scalar1=fcb_s,
                                        scalar2=None, op0=OP.add)
                nc.sync.dma_start(out=out_d[b_, :], in_=ob2[:, 0])

    nc.compile()
    return nc


def _host_inputs(inputs, n_cores, bloc, lp):
    X = np.asarray(inputs["X"]).astype(np.float32)[:, :lp]
    sa = np.asarray(inputs["surface_availability"], dtype=np.float32)[:, :lp]
    ptm = np.asarray(inputs["ptm"]).astype(np.float32)[:, :lp]
    emb = np.asarray(inputs["emb"], dtype=np.float32)
    pemb = np.asarray(inputs["ptm_emb"], dtype=np.float32)
    rpe = np.asarray(inputs["rpe"], dtype=np.float32)
    inw = np.asarray(inputs["in_proj_w"], dtype=np.float32)
    inb = np.asarray(inputs["in_proj_b"], dtype=np.float32)
    wo = np.asarray(inputs["out_proj_w"], dtype=np.float32)
    bo = np.asarray(inputs["out_proj_b"], dtype=np.float32)
    w1 = np.asarray(inputs["lin1_w"], dtype=np.float32)
    b1 = np.asarray(inputs["lin1_b"], dtype=np.float32)
    w2 = np.asarray(inputs["lin2_w"], dtype=np.float32)
    b2 = np.asarray(inputs["lin2_b"], dtype=np.float32)
    c1w = np.asarray(inputs["conv1_w"], dtype=np.float32)
    c2w = np.asarray(inputs["conv2_w"], dtype=np.float32)
    fcw = np.asarray(inputs["fc_w"], dtype=np.float32)

    pembp = np.zeros((10, D), np.float32)
    pembp[:, ED:] = pemb
    embp = np.zeros((NCAT, D), np.float32)
    embp[:, :ED] = emb

    clo, chi = rpe[0], rpe[2 * MD]
    bandcat = np.zeros((D, H * BAND_TOT), np.float32)
    jj = np.arange(128)[:, None]
    for h in range(H):
        for di, dl in enumerate(BAND_DELTAS):
            w = BAND_W[di]
            ii = np.arange(BAND_C0[di], BAND_C0[di] + w)[None, :]
            e = dl + jj - ii
            val = rpe[np.clip(e, -MD, MD) + MD, h]
            beta = chi[h] if di == 5 else clo[h]
            bandcat[:, h * BAND_TOT + BAND_OFF[di]:
                    h * BAND_TOT + BAND_OFF[di] + w] = val - beta
    cexp = np.zeros((D, 2 * H), np.float32)
    for h in range(H):
        cexp[:, 2 * h] = clo[h]
        cexp[:, 2 * h + 1] = chi[h]

    ob_eff = bo + wo @ inb[2 * D:3 * D]
    l2t = w2.T   # [FF, D]
    shared = {
        "embp": embp, "pembp": pembp,
        "iota25": np.arange(NCAT, dtype=np.float32)[:, None],
        "iota10": np.arange(10, dtype=np.float32)[:, None],
        "inwT": np.ascontiguousarray(inw.T),
        "qb": (inb[0:D] * ISQ)[:, None],
        "kb": inb[D:2 * D][:, None],
        "woT": np.ascontiguousarray(wo.T),
        "ob_eff": ob_eff[:, None].astype(np.float32),
        "l1wT": np.ascontiguousarray(w1.T),
        "l1b": b1.reshape(2, D).T.copy(),
        "l2cat": np.concatenate([l2t[0:D], l2t[D:2 * D]], axis=1).copy(),
        "l2b": b2[:, None],
        "ln1g": np.asarray(inputs["ln1_g"], np.float32)[:, None],
        "ln1b": np.asarray(inputs["ln1_b"], np.float32)[:, None],
        "ln2g": np.asarray(inputs["ln2_g"], np.float32)[:, None],
        "ln2b": np.asarray(inputs["ln2_b"], np.float32)[:, None],
        "bandcat": bandcat, "cexp": cexp,
        "c1wT": np.ascontiguousarray(c1w.transpose(1, 2, 0).reshape(D, K * C1)),
        "c2wT": np.ascontiguousarray(c2w.transpose(1, 2, 0).reshape(C1, K * C2)),
        "bn1g": np.asarray(inputs["bn1_g"], np.float32)[:, None],
        "bn1b": np.asarray(inputs["bn1_b"], np.float32)[:, None],
        "bn2g": np.asarray(inputs["bn2_g"], np.float32).reshape(2, C1).T.copy(),
        "bn2b": np.asarray(inputs["bn2_b"], np.float32).reshape(2, C1).T.copy(),
        "fcwT": np.ascontiguousarray(
            fcw.T.reshape(2, C1, NC).transpose(1, 0, 2).reshape(C1, 2 * NC)),
        "fcb": np.asarray(inputs["fc_b"], np.float32)[:, None],
    }
    in_maps = []
    for c in range(n_cores):
        rows = slice(c * bloc, (c + 1) * bloc)
        m = dict(shared)
        m["Xf"] = np.ascontiguousarray(X[rows].reshape(-1))
        m["saf"] = np.ascontiguousarray(sa[rows].reshape(-1))
        m["ptmf"] = np.ascontiguousarray(ptm[rows].reshape(-1))
        in_maps.append(m)
    return in_maps


_NC_CACHE = {}


def _get_nc(n_cores, bloc, lp, dbg=False):
    key = (n_cores, bloc, lp, dbg)
    if key not in _NC_CACHE:
        _NC_CACHE[key] = _build(n_cores, bloc, lp, dbg=dbg)
    return _NC_CACHE[key]


def kernel(**inputs):
    from concourse.bass_utils import run_bass_kernel_spmd
    nc = _get_nc(NCORES, BLOC, L)
    in_maps = _host_inputs(inputs, NCORES, BLOC, L)
    res = run_bass_kernel_spmd(nc, in_maps, list(range(NCORES)))
    out = np.concatenate([res.results[i]["out"] for i in range(NCORES)], axis=0)
    return out.astype(np.float32)


# revision 20
# speedup vs baseline: 1.8869x; 1.1445x over previous
"""Trainium2 Bass kernel for nn_CNN2LWithRPE (transformer layer + CNN head).

Sharding: data-parallel over batch across 8 NeuronCores (2 batch rows each).
All parameters replicated. The only cross-core communication is two tiny
AllReduces for the training-mode BatchNorm statistics.

Per-core layout (B_loc batches, T = B_loc*L tokens):
  - activations transposed in SBUF as bf16: xT/qT/x1T... are [D=128, T]
    (bf16 operands run the PE at 1 cycle/row vs 4 for fp32; PSUM
    accumulation stays fp32)
  - attention as scores^T tiles [keys=128 part, queries=512 free]:
    QK^T row-packed 4x on PE (K=32) from a shuffled kTp layout + a 4x
    replicated qrep tile; exp on ACT as wide [128,2048] ops with the
    clipped-RPE bias folded into a per-group bias constant (c_lo/c_hi)
    plus narrow host-precomputed banded correction tiles added on DVE;
    PV uses v in natural layout with an appended ones-column so the
    softmax denominator falls out of the same matmul.
  - layernorm in transposed layout: partition stats via ones-matmul,
    rstd = Exp(-0.5*Ln(var+eps)), rank-1 K=1 matmul broadcast back.
  - conv1d as K accumulating shifted matmuls; BN apply fused into
    ACT Relu(scale*x+shift) with per-channel scale/shift APs.
"""

import numpy as np

B, L = 16, 2048
NCAT, ED = 25, 120
D, H, HD = 128, 4, 32
FF = 256
MD = 32
C1, C2, K = 128, 256, 5
NC = 2
EPS = 1e-5
NCORES = 8
BLOC = B // NCORES
ISQ = float(1.0 / np.sqrt(HD))

QT = 512
KTILE = 128

BAND_DELTAS = [-128, 0, 128, 256, 384, 512]
BAND_W = [32, 160, 288, 416, 512, 32]
BAND_C0 = [0, 0, 0, 0, 0, 480]
BAND_OFF = [0, 32, 192, 480, 896, 1408]
BAND_TOT = 1440


def _build(n_cores, bloc, lp, dbg=False):
    import contextlib
    import concourse.bass as bass
    import concourse.tile as tile
    from concourse import bacc, mybir

    f32 = mybir.dt.float32
    bf16 = mybir.dt.bfloat16
    AF = mybir.ActivationFunctionType
    OP = mybir.AluOpType
    AX = mybir.AxisListType

    T = bloc * lp
    NET = T // 512
    NQT = lp // QT
    NKT = lp // KTILE
    NG = NKT // 4
    NLT = lp // 512
    NDEN = bloc * NQT * H
    LP2 = lp // 2
    L2 = LP2 - (K - 1)
    LT2 = L2 // 2
    n1 = float(n_cores * bloc * lp)
    n2 = float(n_cores * bloc * L2)

    nc = bacc.Bacc("TRN2", target_bir_lowering=False, debug=False,
                   num_devices=n_cores)

    def din(name, shape):
        return nc.dram_tensor(name, list(shape), f32, kind="ExternalInput")

    Xf = din("Xf", [T])
    saf = din("saf", [T])
    ptmf = din("ptmf", [T])
    emb_d = din("embp", [NCAT, D])
    pemb_d = din("pembp", [10, D])
    iota25_d = din("iota25", [NCAT, 1])
    iota10_d = din("iota10", [10, 1])
    inwT_d = din("inwT", [D, 3 * D])
    qb_d = din("qb", [D, 1])
    kb_d = din("kb", [D, 1])
    woT_d = din("woT", [D, D])
    ob_d = din("ob_eff", [D, 1])
    l1wT_d = din("l1wT", [D, FF])
    l1b_d = din("l1b", [D, 2])
    l2cat_d = din("l2cat", [D, 2 * D])
    l2b_d = din("l2b", [D, 1])
    ln1g_d = din("ln1g", [D, 1])
    ln1b_d = din("ln1b", [D, 1])
    ln2g_d = din("ln2g", [D, 1])
    ln2b_d = din("ln2b", [D, 1])
    band_d = din("bandcat", [D, H * BAND_TOT])
    cexp_d = din("cexp", [D, 2 * H])
    c1wT_d = din("c1wT", [D, K * C1])
    c2wT_d = din("c2wT", [C1, K * C2])
    bn1g_d = din("bn1g", [C1, 1])
    bn1b_d = din("bn1b", [C1, 1])
    bn2g_d = din("bn2g", [C1, 2])
    bn2b_d = din("bn2b", [C1, 2])
    fcwT_d = din("fcwT", [C1, 2 * NC])
    fcb_d = din("fcb", [NC, 1])

    out_d = nc.dram_tensor("out", [bloc, NC], f32, kind="ExternalOutput")
    dbg_outs = {}
    if dbg:
        for nm, shp, dt_ in [("dbg_xT", [D, T], bf16),
                             ("dbg_qT", [D, T], bf16),
                             ("dbg_kT", [D, T], bf16),
                             ("dbg_attnT", [D, T], bf16),
                             ("dbg_x1T", [D, T], bf16),
                             ("dbg_x2", [D, T], bf16),
                             ("dbg_c1", [C1, bloc * lp], bf16),
                             ("dbg_p1", [C1, bloc * LP2], bf16),
                             ("dbg_feat", [C1, 2 * bloc], f32)]:
            dbg_outs[nm] = nc.dram_tensor(nm, shp, dt_, kind="ExternalOutput")

    def bc(ap1d, parts):
        return bass.AP(tensor=ap1d.tensor, offset=ap1d.offset,
                       ap=[[0, parts]] + [list(p) for p in ap1d.ap])

    with tile.TileContext(nc) as tc:
        ctx = contextlib.ExitStack()
        with ctx:
            pp = ctx.enter_context(tc.tile_pool(name="params", bufs=1))
            big = ctx.enter_context(tc.tile_pool(name="big", bufs=1))
            wk = ctx.enter_context(tc.tile_pool(name="wk", bufs=12))
            wkb = ctx.enter_context(tc.tile_pool(name="wkb", bufs=2))
            longs = ctx.enter_context(tc.tile_pool(name="longs", bufs=1))
            mid = ctx.enter_context(tc.tile_pool(name="mid", bufs=6))
            rowp = ctx.enter_context(tc.tile_pool(name="rows", bufs=14))
            ptp = ctx.enter_context(tc.tile_pool(name="pt", bufs=3))
            qrp = ctx.enter_context(tc.tile_pool(name="qrep", bufs=1))
            dram = ctx.enter_context(tc.tile_pool(name="dram", bufs=1, space="DRAM"))

            def wkt(p=D, f=512):
                return wk.tile([p, f], f32, tag="wk", name="wkt")

            def midt(p, f):
                return mid.tile([p, f], f32, tag="mid", name="midt")

            # ---- params ----
            def ld(dten, shape):
                t = pp.tile(list(shape), f32, tag=dten.name, name=dten.name + "_s")
                nc.sync.dma_start(out=t, in_=dten[tuple(slice(0, s) for s in shape)])
                return t

            def ldb(dten, shape):
                # load f32 param, convert to a bf16 copy for PE consumption
                t = ld(dten, shape)
                tb = pp.tile(list(shape), bf16, tag=dten.name + "_b",
                             name=dten.name + "_b")
                nc.vector.tensor_copy(tb, t)
                return tb

            emb_b = ldb(emb_d, [NCAT, D])
            pemb_b = ldb(pemb_d, [10, D])
            io25 = ld(iota25_d, [NCAT, 1])
            io10 = ld(iota10_d, [10, 1])
            inwT_b = ldb(inwT_d, [D, 3 * D])
            qb_s = ld(qb_d, [D, 1])
            kb_s = ld(kb_d, [D, 1])
            woT_b = ldb(woT_d, [D, D])
            ob_s = ld(ob_d, [D, 1])
            l1wT_b = ldb(l1wT_d, [D, FF])
            l1b_s = ld(l1b_d, [D, 2])
            l2cat_b = ldb(l2cat_d, [D, 2 * D])
            l2b_s = ld(l2b_d, [D, 1])
            ln1g = ld(ln1g_d, [D, 1]); ln1b = ld(ln1b_d, [D, 1])
            ln2g = ld(ln2g_d, [D, 1]); ln2b = ld(ln2b_d, [D, 1])
            cexp = ld(cexp_d, [D, 2 * H])
            c1wT_b = ldb(c1wT_d, [D, K * C1])
            c2wT_b = ldb(c2wT_d, [C1, K * C2])
            bn1g = ld(bn1g_d, [C1, 1]); bn1b = ld(bn1b_d, [C1, 1])
            bn2g = ld(bn2g_d, [C1, 2]); bn2b = ld(bn2b_d, [C1, 2])
            fcwT = ld(fcwT_d, [C1, 2 * NC])
            fcb_s = ld(fcb_d, [NC, 1])

            band_f = big.tile([D, H * BAND_TOT], f32, tag="chF0")
            nc.sync.dma_start(out=band_f, in_=band_d[:, :])
            bandexp = big.tile([D, H * BAND_TOT], bf16, tag="chF")
            nc.scalar.activation(bandexp, band_f, AF.Exp, bias=0.0, scale=1.0)

            ones128b = pp.tile([D, 1], bf16, tag="ones128b")
            nc.vector.memset(ones128b, 1.0)
            ones_rb = pp.tile([1, D], bf16, tag="ones_rb")
            nc.vector.memset(ones_rb, 1.0)
            ones32b = pp.tile([1, HD], bf16, tag="ones32b")
            nc.vector.memset(ones32b, 1.0)
            eps128 = pp.tile([D, 1], f32, tag="eps128")
            nc.vector.memset(eps128, EPS)
            eps1 = pp.tile([1, 1], f32, tag="eps1")
            nc.vector.memset(eps1, EPS)

            # ---- persistent activations (bf16) ----
            xT = big.tile([D, T], bf16, tag="chA")
            qT = big.tile([D, T], bf16, tag="chE")
            kTp = big.tile([D, bloc, H, NG, KTILE], bf16, tag="chB")
            v_sb = big.tile([D, T // KTILE, H, HD + 1], bf16, tag="chC")
            attnT = big.tile([D, T], bf16, tag="chD")
            x1T = big.tile([D, T], bf16, tag="chG")
            nc.vector.memset(v_sb[:, :, :, HD:HD + 1], 1.0)

            # ================= embedding =================
            psA_cm = tc.tile_pool(name="psA", bufs=6, space="PSUM")
            ps_sm = psA_cm.__enter__()
            for e in range(NET):
                sl = slice(e * 512, (e + 1) * 512)
                xb = wkt(NCAT)
                nc.sync.dma_start(out=xb, in_=bc(Xf[sl], NCAT))
                sb_ = wkt(NCAT)
                nc.sync.dma_start(out=sb_, in_=bc(saf[sl], NCAT))
                oh = wk.tile([NCAT, 512], bf16, tag="wk", name="oh")
                nc.vector.tensor_scalar(out=oh, in0=xb, scalar1=io25,
                                        scalar2=None, op0=OP.is_equal)
                nc.vector.tensor_mul(oh, oh, sb_)
                pb = wkt(10)
                nc.sync.dma_start(out=pb, in_=bc(ptmf[sl], 10))
                ohp = wk.tile([10, 512], bf16, tag="wk", name="ohp")
                nc.vector.tensor_scalar(out=ohp, in0=pb, scalar1=io10,
                                        scalar2=None, op0=OP.is_equal)
                pe = ps_sm.tile([D, 512], f32, tag="sm")
                nc.tensor.matmul(pe, pemb_b, ohp, start=True, stop=False)
                nc.tensor.matmul(pe, emb_b, oh, start=False, stop=True)
                nc.vector.tensor_copy(xT[:, sl], pe)

            if dbg:
                nc.sync.dma_start(out=dbg_outs["dbg_xT"][:, :], in_=xT)

            # ================= qkv =================
            for e in range(NET):
                sl = slice(e * 512, (e + 1) * 512)
                pq = ps_sm.tile([D, 512], f32, tag="sm")
                nc.tensor.matmul(pq, inwT_b[:, 0:D], xT[:, sl],
                                 start=True, stop=True)
                nc.vector.tensor_scalar(out=qT[:, sl], in0=pq, scalar1=ISQ,
                                        scalar2=qb_s, op0=OP.mult, op1=OP.add)
                pk = ps_sm.tile([D, 512], f32, tag="sm")
                nc.tensor.matmul(pk, inwT_b[:, D:2 * D], xT[:, sl],
                                 start=True, stop=True)
                ktmp = mid.tile([D, 512], bf16, tag="mid", name="ktmp")
                nc.vector.tensor_scalar(out=ktmp, in0=pk, scalar1=kb_s,
                                        scalar2=None, op0=OP.add)
                b_ = (e * 512) // lp
                for h in range(H):
                    for sub in range(4):
                        ktb = ((e * 512) % lp) // KTILE + sub
                        _q = nc.gpsimd if (h + sub) % 2 == 0 else nc.sync
                        _q.dma_start(
                            out=kTp[32 * (ktb % 4):32 * (ktb % 4) + 32,
                                    b_, h, ktb // 4, :],
                            in_=ktmp[32 * h:32 * h + 32,
                                     sub * KTILE:(sub + 1) * KTILE])
                for sub in range(4):
                    tt = (e * 512) // KTILE + sub
                    pv = ps_sm.tile([KTILE, D], f32, tag="sm")
                    nc.tensor.matmul(pv, xT[:, e * 512 + sub * KTILE:
                                            e * 512 + (sub + 1) * KTILE],
                                     inwT_b[:, 2 * D:3 * D], start=True, stop=True)
                    nc.vector.tensor_copy(
                        v_sb[:, tt, :, 0:HD],
                        pv.rearrange("p (h d) -> p h d", h=H))

            if dbg:
                nc.sync.dma_start(out=dbg_outs["dbg_qT"][:, :], in_=qT)
                kT_dbg = big.tile([D, T], bf16, tag="chDBG")
                for b_ in range(bloc):
                    for h in range(H):
                        for g in range(NG):
                            for r in range(4):
                                kt = 4 * g + r
                                nc.sync.dma_start(
                                    out=kT_dbg[32 * h:32 * h + 32,
                                               b_ * lp + kt * KTILE:
                                               b_ * lp + (kt + 1) * KTILE],
                                    in_=kTp[32 * r:32 * r + 32, b_, h, g, :])
                nc.sync.dma_start(out=dbg_outs["dbg_kT"][:, :], in_=kT_dbg)

            psA_cm.__exit__(None, None, None)

            # ================= attention =================
            # 2 key-tiles per subgroup so sc fits 2 PSUM banks and can be
            # triple-buffered: QK of subgroup p+2 overlaps exp of p,
            # PV of p overlaps exp of p+1. Band bias applied post-exp as a
            # bf16 multiply on pt (off the QK->exp critical path).
            psB1_cm = tc.tile_pool(name="ps_sc", bufs=3, space="PSUM")
            ps_sc = psB1_cm.__enter__()
            psB2_cm = tc.tile_pool(name="ps_pv", bufs=2, space="PSUM")
            ps_pv = psB2_cm.__enter__()
            den32 = longs.tile([NDEN, 512], bf16, tag="den32")
            for b_ in range(bloc):
                for h in range(H):
                    qrep = qrp.tile([D, lp], bf16, tag="qr")
                    for r in range(4):
                        nc.gpsimd.dma_start(
                            out=qrep[32 * r:32 * r + 32, :],
                            in_=qT[32 * h:32 * h + 32, b_ * lp:(b_ + 1) * lp])
                    for qt in range(NQT):
                        ppv = ps_pv.tile([HD + 1, 512], f32, tag="pv")
                        for p_ in range(2 * NG):
                            sc = ps_sc.tile([D, 2 * 512], f32, tag="sc")
                            for j in range(2):
                                kt = 2 * p_ + j
                                nc.tensor.matmul(
                                    sc[:, j * 512:(j + 1) * 512],
                                    kTp[32 * (kt % 4):32 * (kt % 4) + 32,
                                        b_, h, kt // 4, :],
                                    qrep[32 * (kt % 4):32 * (kt % 4) + 32,
                                         qt * QT:(qt + 1) * QT],
                                    start=True, stop=True,
                                    tile_position=(32 * (kt % 4), 0))
                            side = 0 if (p_ >> 1) <= qt else 1
                            pt = ptp.tile([D, 2 * 512], bf16, tag="pt")
                            nc.scalar.activation(pt, sc, AF.Exp,
                                                 bias=cexp[:, 2 * h + side:
                                                           2 * h + side + 1],
                                                 scale=1.0)
                            for j in range(2):
                                kt = 2 * p_ + j
                                dd = kt - 4 * qt
                                if -1 <= dd <= 4:
                                    di = dd + 1
                                    c0, w = BAND_C0[di], BAND_W[di]
                                    nc.vector.tensor_tensor(
                                        out=pt[:, j * 512 + c0:j * 512 + c0 + w],
                                        in0=pt[:, j * 512 + c0:j * 512 + c0 + w],
                                        in1=bandexp[:, h * BAND_TOT + BAND_OFF[di]:
                                                    h * BAND_TOT + BAND_OFF[di] + w],
                                        op=OP.mult)
                            for j in range(2):
                                kt = 2 * p_ + j
                                nc.tensor.matmul(
                                    ppv, v_sb[:, b_ * NKT + kt, h, :],
                                    pt[:, j * 512:(j + 1) * 512],
                                    start=(kt == 0), stop=(kt == NKT - 1))
                        pv_sb = wk.tile([HD + 1, 512], bf16, tag="wk", name="pv_sb")
                        nc.vector.tensor_copy(pv_sb, ppv)
                        nc.gpsimd.dma_start(
                            out=attnT[32 * h:32 * h + 32,
                                      b_ * lp + qt * QT:b_ * lp + (qt + 1) * QT],
                            in_=pv_sb[0:HD, :])
                        nc.gpsimd.dma_start(
                            out=den32[(b_ * NQT + qt) * H + h:
                                      (b_ * NQT + qt) * H + h + 1, :],
                            in_=pv_sb[HD:HD + 1, :])

            psB2_cm.__exit__(None, None, None)
            psB1_cm.__exit__(None, None, None)
            psC_cm = tc.tile_pool(name="psC", bufs=6, space="PSUM")
            ps_sm = psC_cm.__enter__()

            lnden = midt(NDEN, 512)
            nc.scalar.activation(lnden, den32, AF.Ln, bias=0.0, scale=1.0)
            recip = longs.tile([NDEN, 512], bf16, tag="recip")
            nc.scalar.activation(recip, lnden, AF.Exp, bias=0.0, scale=-1.0)

            for b_ in range(bloc):
                for qt in range(NQT):
                    bcp = ps_sm.tile([D, 512], f32, tag="sm")
                    for h in range(H):
                        rr = rowp.tile([1, 512], bf16, tag="row")
                        nc.gpsimd.dma_start(
                            out=rr, in_=recip[(b_ * NQT + qt) * H + h:
                                              (b_ * NQT + qt) * H + h + 1, :])
                        nc.tensor.matmul(bcp[32 * h:32 * h + 32, :], ones32b, rr,
                                         start=True, stop=True,
                                         tile_position=(0, 32 * h))
                    sl = slice(b_ * lp + qt * QT, b_ * lp + (qt + 1) * QT)
                    nc.vector.tensor_mul(attnT[:, sl], attnT[:, sl], bcp)

            if dbg:
                nc.sync.dma_start(out=dbg_outs["dbg_attnT"][:, :], in_=attnT)

            # ======== layernorm helper (per-tile rows, no DMA hops) ========
            def layernorm_T(src, dst_fn, g_s, b_s, b_base):
                for t_ in range(NLT):
                    sl = slice(b_base * lp + t_ * 512, b_base * lp + (t_ + 1) * 512)
                    sq = wk.tile([D, 512], bf16, tag="wk", name="sq")
                    nc.vector.tensor_mul(sq, src[:, sl], src[:, sl])
                    p1_ = ps_sm.tile([1, 512], f32, tag="sm")
                    nc.tensor.matmul(p1_, ones128b, src[:, sl], start=True, stop=True)
                    p2_ = ps_sm.tile([1, 512], f32, tag="sm")
                    nc.tensor.matmul(p2_, ones128b, sq, start=True, stop=True)
                    m_ = rowp.tile([1, 512], f32, tag="row", name="m_")
                    nc.vector.tensor_scalar(out=m_, in0=p1_, scalar1=1.0 / D,
                                            scalar2=None, op0=OP.mult)
                    var = rowp.tile([1, 512], f32, tag="row", name="var")
                    nc.vector.tensor_scalar(out=var, in0=p2_, scalar1=1.0 / D,
                                            scalar2=None, op0=OP.mult)
                    msq = rowp.tile([1, 512], f32, tag="row", name="msq")
                    nc.vector.tensor_mul(msq, m_, m_)
                    nc.vector.tensor_tensor(out=var, in0=var, in1=msq,
                                            op=OP.subtract)
                    lnv = rowp.tile([1, 512], f32, tag="row", name="lnv")
                    nc.scalar.activation(lnv, var, AF.Ln, bias=eps1, scale=1.0)
                    rr = rowp.tile([1, 512], bf16, tag="row", name="rr")
                    nc.scalar.activation(rr, lnv, AF.Exp, bias=0.0, scale=-0.5)
                    rm = rowp.tile([1, 512], bf16, tag="row", name="rm")
                    nc.vector.tensor_mul(rm, m_, rr)
                    br = ps_sm.tile([D, 512], f32, tag="sm")
                    nc.tensor.matmul(br, ones_rb, rr, start=True, stop=True)
                    bm = ps_sm.tile([D, 512], f32, tag="sm")
                    nc.tensor.matmul(bm, ones_rb, rm, start=True, stop=True)
                    tmp = wkt()
                    nc.vector.tensor_mul(tmp, src[:, sl], br)
                    nc.vector.tensor_tensor(out=tmp, in0=tmp, in1=bm,
                                            op=OP.subtract)
                    nc.vector.tensor_scalar(out=dst_fn(t_), in0=tmp, scalar1=g_s,
                                            scalar2=b_s, op0=OP.mult, op1=OP.add)

            # ================= out-proj + residual + LN1 =================
            r1T = big.tile([D, T], bf16, tag="chB")   # after kTp's last read
            for b_ in range(bloc):
                for qt in range(NQT):
                    sl = slice(b_ * lp + qt * QT, b_ * lp + (qt + 1) * QT)
                    po = ps_sm.tile([D, 512], f32, tag="sm")
                    nc.tensor.matmul(po, woT_b, attnT[:, sl], start=True, stop=True)
                    nc.vector.tensor_scalar(out=r1T[:, sl], in0=po, scalar1=ob_s,
                                            scalar2=None, op0=OP.add)
                    nc.vector.tensor_tensor(out=r1T[:, sl], in0=r1T[:, sl],
                                            in1=xT[:, sl], op=OP.add)
            for b_ in range(bloc):
                layernorm_T(
                    r1T,
                    lambda t_, b0=b_: x1T[:, b0 * lp + t_ * 512:
                                          b0 * lp + (t_ + 1) * 512],
                    ln1g, ln1b, b_)

            if dbg:
                nc.sync.dma_start(out=dbg_outs["dbg_x1T"][:, :], in_=x1T)

            # ================= FFN + residual + LN2 =================
            x2pad = big.tile([D, bloc * (lp + 4)], bf16, tag="chA")  # after xT
            nc.vector.memset(x2pad[:, :], 0.0)
            r2T = big.tile([D, T], bf16, tag="chC")                  # after v_sb
            for b_ in range(bloc):
                for qt in range(NQT):
                    sl = slice(b_ * lp + qt * QT, b_ * lp + (qt + 1) * QT)
                    h1a = wk.tile([D, 512], bf16, tag="wk", name="h1a")
                    h1b = wk.tile([D, 512], bf16, tag="wk", name="h1b")
                    for half, dest in ((0, h1a), (1, h1b)):
                        ph = ps_sm.tile([D, 512], f32, tag="sm")
                        nc.tensor.matmul(ph, l1wT_b[:, half * D:(half + 1) * D],
                                         x1T[:, sl], start=True, stop=True)
                        nc.scalar.activation(dest, ph, AF.Relu,
                                             bias=l1b_s[:, half:half + 1],
                                             scale=1.0)
                    py = ps_sm.tile([D, 512], f32, tag="sm")
                    nc.tensor.matmul(py, l2cat_b[:, 0:D], h1a, start=True, stop=False)
                    nc.tensor.matmul(py, l2cat_b[:, D:2 * D], h1b,
                                     start=False, stop=True)
                    nc.vector.tensor_scalar(out=r2T[:, sl], in0=py, scalar1=l2b_s,
                                            scalar2=None, op0=OP.add)
                    nc.vector.tensor_tensor(out=r2T[:, sl], in0=r2T[:, sl],
                                            in1=x1T[:, sl], op=OP.add)
            for b_ in range(bloc):
                layernorm_T(
                    r2T,
                    lambda t_, b0=b_: x2pad[:, b0 * (lp + 4) + 2 + t_ * 512:
                                            b0 * (lp + 4) + 2 + (t_ + 1) * 512],
                    ln2g, ln2b, b_)

            if dbg:
                for b_ in range(bloc):
                    nc.sync.dma_start(
                        out=dbg_outs["dbg_x2"][:, b_ * lp:(b_ + 1) * lp],
                        in_=x2pad[:, b_ * (lp + 4) + 2:b_ * (lp + 4) + 2 + lp])

            # ================= conv1 + bn1 =================
            c1_sb = big.tile([C1, bloc * lp], bf16, tag="chD")   # after attnT
            bnst1 = longs.tile([C1, bloc * NLT, 6], f32, tag="bnst1")
            for b_ in range(bloc):
                for t_ in range(NLT):
                    pc = ps_sm.tile([C1, 512], f32, tag="sm")
                    for k_ in range(K):
                        nc.tensor.matmul(
                            pc, c1wT_b[:, k_ * C1:(k_ + 1) * C1],
                            x2pad[:, b_ * (lp + 4) + t_ * 512 + k_:
                                  b_ * (lp + 4) + t_ * 512 + k_ + 512],
                            start=(k_ == 0), stop=(k_ == K - 1))
                    nc.vector.bn_stats(out=bnst1[:, b_ * NLT + t_, :], in_=pc)
                    nc.vector.tensor_copy(
                        c1_sb[:, b_ * lp + t_ * 512:b_ * lp + (t_ + 1) * 512], pc)
            mv1 = wk.tile([C1, 2], f32, tag="wk")
            nc.vector.bn_aggr(out=mv1, in_=bnst1)
            part1 = wk.tile([C1, 2], f32, tag="wk")
            sqm = wk.tile([C1, 1], f32, tag="wk")
            nc.vector.tensor_mul(sqm, mv1[:, 0:1], mv1[:, 0:1])
            nc.vector.tensor_tensor(out=sqm, in0=sqm, in1=mv1[:, 1:2], op=OP.add)
            nl_ = float(bloc * lp)
            nc.vector.tensor_scalar(out=part1[:, 0:1], in0=mv1[:, 0:1],
                                    scalar1=nl_, scalar2=None, op0=OP.mult)
            nc.vector.tensor_scalar(out=part1[:, 1:2], in0=sqm,
                                    scalar1=nl_, scalar2=None, op0=OP.mult)
            bn1_in = dram.tile([C1, 2], f32, tag="bn1i")
            bn1_out = dram.tile([C1, 2], f32, tag="bn1o")
            nc.sync.dma_start(out=bn1_in, in_=part1)
            nc.gpsimd.collective_compute(
                "AllReduce", OP.add, replica_groups=[list(range(n_cores))],
                ins=[bn1_in[:, :].opt()], outs=[bn1_out[:, :].opt()])
            glob1 = wk.tile([C1, 2], f32, tag="wk")
            nc.sync.dma_start(out=glob1, in_=bn1_out)

            def bn_scale_shift(globc, n_, g_ap, b_ap):
                mean = wk.tile([C1, 1], f32, tag="wk")
                nc.vector.tensor_scalar(out=mean, in0=globc[:, 0:1],
                                        scalar1=1.0 / n_, scalar2=None, op0=OP.mult)
                ex2 = wk.tile([C1, 1], f32, tag="wk")
                nc.vector.tensor_scalar(out=ex2, in0=globc[:, 1:2],
                                        scalar1=1.0 / n_, scalar2=None, op0=OP.mult)
                msq_ = wk.tile([C1, 1], f32, tag="wk")
                nc.vector.tensor_mul(msq_, mean, mean)
                nc.vector.tensor_tensor(out=ex2, in0=ex2, in1=msq_, op=OP.subtract)
                lnv_ = wk.tile([C1, 1], f32, tag="wk")
                nc.scalar.activation(lnv_, ex2, AF.Ln, bias=eps128, scale=1.0)
                rstd_ = wk.tile([C1, 1], f32, tag="wk")
                nc.scalar.activation(rstd_, lnv_, AF.Exp, bias=0.0, scale=-0.5)
                scale = longs.tile([C1, 1], f32, tag="bnsc")
                nc.vector.tensor_mul(scale, rstd_, g_ap)
                shift = longs.tile([C1, 1], f32, tag="bnsh")
                nc.vector.tensor_mul(shift, mean, scale)
                nc.vector.tensor_tensor(out=shift, in0=b_ap, in1=shift,
                                        op=OP.subtract)
                return scale, shift

            sc1, sh1 = bn_scale_shift(glob1, n1, bn1g, bn1b)
            p1_sb = big.tile([C1, bloc * LP2], bf16, tag="chF")   # after band
            if dbg:
                nc.sync.dma_start(out=dbg_outs["dbg_c1"][:, :], in_=c1_sb)
            for b_ in range(bloc):
                for t_ in range(NLT):
                    rel = wk.tile([D, 512], bf16, tag="wk", name="rel")
                    nc.scalar.activation(
                        rel, c1_sb[:, b_ * lp + t_ * 512:b_ * lp + (t_ + 1) * 512],
                        AF.Relu, bias=sh1, scale=sc1)
                    rel2 = rel.rearrange("p (l two) -> p l two", two=2)
                    nc.vector.tensor_tensor(
                        out=p1_sb[:, b_ * LP2 + t_ * 256:b_ * LP2 + (t_ + 1) * 256],
                        in0=rel2[:, :, 0], in1=rel2[:, :, 1], op=OP.max)
            if dbg:
                nc.sync.dma_start(out=dbg_outs["dbg_p1"][:, :], in_=p1_sb)

            # ================= conv2 + bn2 =================
            c2_sb = big.tile([C1, bloc * 2 * L2], bf16, tag="chE")  # after qT
            bnst2 = longs.tile([C1, 2, bloc * 2, 6], f32, tag="bnst2")
            for b_ in range(bloc):
                for half in range(2):
                    for t_ in range(2):
                        pc = ps_sm.tile([C1, LT2], f32, tag="sm")
                        for k_ in range(K):
                            nc.tensor.matmul(
                                pc, c2wT_b[:, k_ * C2 + half * C1:
                                           k_ * C2 + (half + 1) * C1],
                                p1_sb[:, b_ * LP2 + t_ * LT2 + k_:
                                      b_ * LP2 + t_ * LT2 + k_ + LT2],
                                start=(k_ == 0), stop=(k_ == K - 1))
                        nc.vector.bn_stats(out=bnst2[:, half, b_ * 2 + t_, :],
                                           in_=pc)
                        nc.vector.tensor_copy(
                            c2_sb[:, (b_ * 2 + half) * L2 + t_ * LT2:
                                  (b_ * 2 + half) * L2 + (t_ + 1) * LT2], pc)
            part2 = longs.tile([C1, 4], f32, tag="part2")
            for half in range(2):
                mv2 = wk.tile([C1, 2], f32, tag="wk")
                nc.vector.bn_aggr(out=mv2, in_=bnst2[:, half, :, :])
                sqm2 = wk.tile([C1, 1], f32, tag="wk")
                nc.vector.tensor_mul(sqm2, mv2[:, 0:1], mv2[:, 0:1])
                nc.vector.tensor_tensor(out=sqm2, in0=sqm2, in1=mv2[:, 1:2],
                                        op=OP.add)
                nl2 = float(bloc * L2)
                nc.vector.tensor_scalar(out=part2[:, 2 * half:2 * half + 1],
                                        in0=mv2[:, 0:1], scalar1=nl2,
                                        scalar2=None, op0=OP.mult)
                nc.vector.tensor_scalar(out=part2[:, 2 * half + 1:2 * half + 2],
                                        in0=sqm2, scalar1=nl2,
                                        scalar2=None, op0=OP.mult)
            bn2_in = dram.tile([C1, 4], f32, tag="bn2i")
            bn2_out = dram.tile([C1, 4], f32, tag="bn2o")
            nc.sync.dma_start(out=bn2_in, in_=part2)
            nc.gpsimd.collective_compute(
                "AllReduce", OP.add, replica_groups=[list(range(n_cores))],
                ins=[bn2_in[:, :].opt()], outs=[bn2_out[:, :].opt()])
            glob2 = longs.tile([C1, 4], f32, tag="glob2")
            nc.sync.dma_start(out=glob2, in_=bn2_out)

            feat = longs.tile([C1, 2 * bloc], f32, tag="feat")
            for half in range(2):
                sc2, sh2 = bn_scale_shift(glob2[:, 2 * half:2 * half + 2], n2,
                                          bn2g[:, half:half + 1],
                                          bn2b[:, half:half + 1])
                for b_ in range(bloc):
                    rel = wkb.tile([C1, L2], bf16, tag="wkb")
                    nc.scalar.activation(
                        rel, c2_sb[:, (b_ * 2 + half) * L2:
                                   (b_ * 2 + half + 1) * L2],
                        AF.Relu, bias=sh2, scale=sc2)
                    nc.vector.reduce_max(
                        out=feat[:, (b_ * 2 + half):(b_ * 2 + half) + 1],
                        in_=rel, axis=AX.X)
            if dbg:
                nc.sync.dma_start(out=dbg_outs["dbg_feat"][:, :], in_=feat)

            # ================= fc =================
            for b_ in range(bloc):
                pf = ps_sm.tile([NC, 1], f32, tag="sm")
                for half in range(2):
                    nc.tensor.matmul(pf, fcwT[:, half * NC:(half + 1) * NC],
                                     feat[:, b_ * 2 + half:b_ * 2 + half + 1],
                                     start=(half == 0), stop=(half == 1))
                ob2 = wk.tile([NC, 1], f32, tag="wk")
                nc.vector.tensor_scalar(out=ob2, in0=pf, scalar1=fcb_s,
                                        scalar2=None, op0=OP.add)
                nc.sync.dma_start(out=out_d[b_, :], in_=ob2[:, 0])
            psC_cm.__exit__(None, None, None)

    nc.compile()
    return nc


def _host_inputs(inputs, n_cores, bloc, lp):
    X = np.asarray(inputs["X"]).astype(np.float32)[:, :lp]
    sa = np.asarray(inputs["surface_availability"], dtype=np.float32)[:, :lp]
    ptm = np.asarray(inputs["ptm"]).astype(np.float32)[:, :lp]
    emb = np.asarray(inputs["emb"], dtype=np.float32)
    pemb = np.asarray(inputs["ptm_emb"], dtype=np.float32)
    rpe = np.asarray(inputs["rpe"], dtype=np.float32)
    inw = np.asarray(inputs["in_proj_w"], dtype=np.float32)
    inb = np.asarray(inputs["in_proj_b"], dtype=np.float32)
    wo = np.asarray(inputs["out_proj_w"], dtype=np.float32)
    bo = np.asarray(inputs["out_proj_b"], dtype=np.float32)
    w1 = np.asarray(inputs["lin1_w"], dtype=np.float32)
    b1 = np.asarray(inputs["lin1_b"], dtype=np.float32)
    w2 = np.asarray(inputs["lin2_w"], dtype=np.float32)
    b2 = np.asarray(inputs["lin2_b"], dtype=np.float32)
    c1w = np.asarray(inputs["conv1_w"], dtype=np.float32)
    c2w = np.asarray(inputs["conv2_w"], dtype=np.float32)
    fcw = np.asarray(inputs["fc_w"], dtype=np.float32)

    pembp = np.zeros((10, D), np.float32)
    pembp[:, ED:] = pemb
    embp = np.zeros((NCAT, D), np.float32)
    embp[:, :ED] = emb

    clo, chi = rpe[0], rpe[2 * MD]
    bandcat = np.zeros((D, H * BAND_TOT), np.float32)
    jj = np.arange(128)[:, None]
    for h in range(H):
        for di, dl in enumerate(BAND_DELTAS):
            w = BAND_W[di]
            ii = np.arange(BAND_C0[di], BAND_C0[di] + w)[None, :]
            e = dl + jj - ii
            val = rpe[np.clip(e, -MD, MD) + MD, h]
            beta = chi[h] if di == 5 else clo[h]
            bandcat[:, h * BAND_TOT + BAND_OFF[di]:
                    h * BAND_TOT + BAND_OFF[di] + w] = val - beta
    cexp = np.zeros((D, 2 * H), np.float32)
    for h in range(H):
        cexp[:, 2 * h] = clo[h]
        cexp[:, 2 * h + 1] = chi[h]

    ob_eff = bo + wo @ inb[2 * D:3 * D]
    l2t = w2.T   # [FF, D]
    shared = {
        "embp": embp, "pembp": pembp,
        "iota25": np.arange(NCAT, dtype=np.float32)[:, None],
        "iota10": np.arange(10, dtype=np.float32)[:, None],
        "inwT": np.ascontiguousarray(inw.T),
        "qb": (inb[0:D] * ISQ)[:, None],
        "kb": inb[D:2 * D][:, None],
        "woT": np.ascontiguousarray(wo.T),
        "ob_eff": ob_eff[:, None].astype(np.float32),
        "l1wT": np.ascontiguousarray(w1.T),
        "l1b": b1.reshape(2, D).T.copy(),
        "l2cat": np.concatenate([l2t[0:D], l2t[D:2 * D]], axis=1).copy(),
        "l2b": b2[:, None],
        "ln1g": np.asarray(inputs["ln1_g"], np.float32)[:, None],
        "ln1b": np.asarray(inputs["ln1_b"], np.float32)[:, None],
        "ln2g": np.asarray(inputs["ln2_g"], np.float32)[:, None],
        "ln2b": np.asarray(inputs["ln2_b"], np.float32)[:, None],
        "bandcat": bandcat, "cexp": cexp,
        "c1wT": np.ascontiguousarray(c1w.transpose(1, 2, 0).reshape(D, K * C1)),
        "c2wT": np.ascontiguousarray(c2w.transpose(1, 2, 0).reshape(C1, K * C2)),
        "bn1g": np.asarray(inputs["bn1_g"], np.float32)[:, None],
        "bn1b": np.asarray(inputs["bn1_b"], np.float32)[:, None],
        "bn2g": np.asarray(inputs["bn2_g"], np.float32).reshape(2, C1).T.copy(),
        "bn2b": np.asarray(inputs["bn2_b"], np.float32).reshape(2, C1).T.copy(),
        "fcwT": np.ascontiguousarray(
            fcw.T.reshape(2, C1, NC).transpose(1, 0, 2).reshape(C1, 2 * NC)),
        "fcb": np.asarray(inputs["fc_b"], np.float32)[:, None],
    }
    in_maps = []
    for c in range(n_cores):
        rows = slice(c * bloc, (c + 1) * bloc)
        m = dict(shared)
        m["Xf"] = np.ascontiguousarray(X[rows].reshape(-1))
        m["saf"] = np.ascontiguousarray(sa[rows].reshape(-1))
        m["ptmf"] = np.ascontiguousarray(ptm[rows].reshape(-1))
        in_maps.append(m)
    return in_maps


_NC_CACHE = {}


def _get_nc(n_cores, bloc, lp, dbg=False):
    key = (n_cores, bloc, lp, dbg)
    if key not in _NC_CACHE:
        _NC_CACHE[key] = _build(n_cores, bloc, lp, dbg=dbg)
    return _NC_CACHE[key]


def kernel(**inputs):
    from concourse.bass_utils import run_bass_kernel_spmd
    nc = _get_nc(NCORES, BLOC, L)
    in_maps = _host_inputs(inputs, NCORES, BLOC, L)
    res = run_bass_kernel_spmd(nc, in_maps, list(range(NCORES)))
    out = np.concatenate([res.results[i]["out"] for i in range(NCORES)], axis=0)
    return out.astype(np.float32)


# revision 24
# speedup vs baseline: 1.9531x; 1.0351x over previous
"""Trainium2 Bass kernel for nn_CNN2LWithRPE (transformer layer + CNN head).

Sharding: data-parallel over batch across 8 NeuronCores (2 batch rows each).
All parameters replicated. The only cross-core communication is two tiny
AllReduces for the training-mode BatchNorm statistics.

Per-core layout (B_loc batches, T = B_loc*L tokens):
  - activations transposed in SBUF as bf16: xT/qT/x1T... are [D=128, T]
    (bf16 operands run the PE at 1 cycle/row vs 4 for fp32; PSUM
    accumulation stays fp32)
  - attention as scores^T tiles [keys=128 part, queries=512 free]:
    QK^T row-packed 4x on PE (K=32) from a shuffled kTp layout + a 4x
    replicated qrep tile; exp on ACT as wide [128,2048] ops with the
    clipped-RPE bias folded into a per-group bias constant (c_lo/c_hi)
    plus narrow host-precomputed banded correction tiles added on DVE;
    PV uses v in natural layout with an appended ones-column so the
    softmax denominator falls out of the same matmul.
  - layernorm in transposed layout: partition stats via ones-matmul,
    rstd = Exp(-0.5*Ln(var+eps)), rank-1 K=1 matmul broadcast back.
  - conv1d as K accumulating shifted matmuls; BN apply fused into
    ACT Relu(scale*x+shift) with per-channel scale/shift APs.
"""

import numpy as np

B, L = 16, 2048
NCAT, ED = 25, 120
D, H, HD = 128, 4, 32
FF = 256
MD = 32
C1, C2, K = 128, 256, 5
NC = 2
EPS = 1e-5
NCORES = 8
BLOC = B // NCORES
ISQ = float(1.0 / np.sqrt(HD))

QT = 512
KTILE = 128

BAND_DELTAS = [-128, 0, 128, 256, 384, 512]
BAND_W = [32, 160, 288, 416, 512, 32]
BAND_C0 = [0, 0, 0, 0, 0, 480]
BAND_OFF = [0, 32, 192, 480, 896, 1408]
BAND_TOT = 1440


def _build(n_cores, bloc, lp, dbg=False):
    import contextlib
    import concourse.bass as bass
    import concourse.tile as tile
    from concourse import bacc, mybir

    f32 = mybir.dt.float32
    bf16 = mybir.dt.bfloat16
    AF = mybir.ActivationFunctionType
    OP = mybir.AluOpType
    AX = mybir.AxisListType

    T = bloc * lp
    NET = T // 512
    NQT = lp // QT
    NKT = lp // KTILE
    NG = NKT // 4
    NLT = lp // 512
    NDEN = bloc * NQT * H
    LP2 = lp // 2
    L2 = LP2 - (K - 1)
    LT2 = L2 // 2
    n1 = float(n_cores * bloc * lp)
    n2 = float(n_cores * bloc * L2)

    nc = bacc.Bacc("TRN2", target_bir_lowering=False, debug=False,
                   num_devices=n_cores)

    def din(name, shape):
        return nc.dram_tensor(name, list(shape), f32, kind="ExternalInput")

    Xf = din("Xf", [T])
    saf = din("saf", [T])
    ptmf = din("ptmf", [T])
    emb_d = din("embp", [NCAT, D])
    pemb_d = din("pembp", [10, D])
    iota25_d = din("iota25", [NCAT, 1])
    iota10_d = din("iota10", [10, 1])
    inwT_d = din("inwT", [D, 3 * D])
    qb_d = din("qb", [D, 1])
    kb_d = din("kb", [D, 1])
    woT_d = din("woT", [D, D])
    ob_d = din("ob_eff", [D, 1])
    l1wT_d = din("l1wT", [D, FF])
    l1b_d = din("l1b", [D, 2])
    l2cat_d = din("l2cat", [D, 2 * D])
    l2b_d = din("l2b", [D, 1])
    ln1g_d = din("ln1g", [D, 1])
    ln1b_d = din("ln1b", [D, 1])
    ln2g_d = din("ln2g", [D, 1])
    ln2b_d = din("ln2b", [D, 1])
    band_d = din("bandcat", [D, H * BAND_TOT])
    cexp_d = din("cexp", [D, 2 * H])
    c1wT_d = din("c1wT", [D, K * C1])
    c2wT_d = din("c2wT", [C1, K * C2])
    bn1g_d = din("bn1g", [C1, 1])
    bn1b_d = din("bn1b", [C1, 1])
    bn2g_d = din("bn2g", [C1, 2])
    bn2b_d = din("bn2b", [C1, 2])
    fcwT_d = din("fcwT", [C1, 2 * NC])
    fcb_d = din("fcb", [NC, 1])

    out_d = nc.dram_tensor("out", [bloc, NC], f32, kind="ExternalOutput")
    dbg_outs = {}
    if dbg:
        for nm, shp, dt_ in [("dbg_xT", [D, T], bf16),
                             ("dbg_qT", [D, T], bf16),
                             ("dbg_kT", [D, T], bf16),
                             ("dbg_attnT", [D, T], bf16),
                             ("dbg_x1T", [D, T], bf16),
                             ("dbg_x2", [D, T], bf16),
                             ("dbg_c1", [C1, bloc * lp], bf16),
                             ("dbg_p1", [C1, bloc * LP2], bf16),
                             ("dbg_feat", [C1, 2 * bloc], f32)]:
            dbg_outs[nm] = nc.dram_tensor(nm, shp, dt_, kind="ExternalOutput")

    def bc(ap1d, parts):
        return bass.AP(tensor=ap1d.tensor, offset=ap1d.offset,
                       ap=[[0, parts]] + [list(p) for p in ap1d.ap])

    with tile.TileContext(nc) as tc:
        ctx = contextlib.ExitStack()
        with ctx:
            pp = ctx.enter_context(tc.tile_pool(name="params", bufs=1))
            big = ctx.enter_context(tc.tile_pool(name="big", bufs=1))
            wk = ctx.enter_context(tc.tile_pool(name="wk", bufs=12))
            wkb = ctx.enter_context(tc.tile_pool(name="wkb", bufs=2))
            longs = ctx.enter_context(tc.tile_pool(name="longs", bufs=1))
            mid = ctx.enter_context(tc.tile_pool(name="mid", bufs=6))
            rowp = ctx.enter_context(tc.tile_pool(name="rows", bufs=6))
            lnr = ctx.enter_context(tc.tile_pool(name="lnrows", bufs=1))
            ptp = ctx.enter_context(tc.tile_pool(name="pt", bufs=3))
            qrp = ctx.enter_context(tc.tile_pool(name="qrep", bufs=1))
            dram = ctx.enter_context(tc.tile_pool(name="dram", bufs=1, space="DRAM"))

            def wkt(p=D, f=512):
                return wk.tile([p, f], f32, tag="wk", name="wkt")

            def midt(p, f):
                return mid.tile([p, f], f32, tag="mid", name="midt")

            # ---- params ----
            def ld(dten, shape):
                t = pp.tile(list(shape), f32, tag=dten.name, name=dten.name + "_s")
                nc.sync.dma_start(out=t, in_=dten[tuple(slice(0, s) for s in shape)])
                return t

            def ldb(dten, shape):
                # load f32 param, convert to a bf16 copy for PE consumption
                t = ld(dten, shape)
                tb = pp.tile(list(shape), bf16, tag=dten.name + "_b",
                             name=dten.name + "_b")
                nc.vector.tensor_copy(tb, t)
                return tb

            emb_b = ldb(emb_d, [NCAT, D])
            pemb_b = ldb(pemb_d, [10, D])
            io25 = ld(iota25_d, [NCAT, 1])
            io10 = ld(iota10_d, [10, 1])
            inwT_b = ldb(inwT_d, [D, 3 * D])
            qb_s = ld(qb_d, [D, 1])
            kb_s = ld(kb_d, [D, 1])
            woT_b = ldb(woT_d, [D, D])
            ob_s = ld(ob_d, [D, 1])
            l1wT_b = ldb(l1wT_d, [D, FF])
            l1b_s = ld(l1b_d, [D, 2])
            l2cat_b = ldb(l2cat_d, [D, 2 * D])
            l2b_s = ld(l2b_d, [D, 1])
            ln1g = ld(ln1g_d, [D, 1]); ln1b = ld(ln1b_d, [D, 1])
            ln2g = ld(ln2g_d, [D, 1]); ln2b = ld(ln2b_d, [D, 1])
            # transposed bf16 rows of LN gamma / negated beta for rank-1
            # broadcast matmuls: br = g^T x rstd_row, bm += (-b)^T x ones_row
            def trow(col_tile, name, negate=False):
                rf = pp.tile([1, D], f32, tag=name + "f", name=name + "f")
                nc.sync.dma_start(out=rf, in_=col_tile[:, 0])
                rb = pp.tile([1, D], bf16, tag=name, name=name)
                nc.vector.tensor_scalar(out=rb, in0=rf,
                                        scalar1=-1.0 if negate else 1.0,
                                        scalar2=None, op0=OP.mult)
                return rb
            ln1g_r = trow(ln1g, "ln1g_r")
            ln1nb_r = trow(ln1b, "ln1nb_r", negate=True)
            ln2g_r = trow(ln2g, "ln2g_r")
            ln2nb_r = trow(ln2b, "ln2nb_r", negate=True)
            cexp = ld(cexp_d, [D, 2 * H])
            c1wT_b = ldb(c1wT_d, [D, K * C1])
            c2wT_b = ldb(c2wT_d, [C1, K * C2])
            bn1g = ld(bn1g_d, [C1, 1]); bn1b = ld(bn1b_d, [C1, 1])
            bn2g = ld(bn2g_d, [C1, 2]); bn2b = ld(bn2b_d, [C1, 2])
            fcwT = ld(fcwT_d, [C1, 2 * NC])
            fcb_s = ld(fcb_d, [NC, 1])

            band_f = big.tile([D, H * BAND_TOT], f32, tag="chF0")
            nc.sync.dma_start(out=band_f, in_=band_d[:, :])
            bandexp = big.tile([D, H * BAND_TOT], bf16, tag="chF")
            nc.scalar.activation(bandexp, band_f, AF.Exp, bias=0.0, scale=1.0)

            ones128b = pp.tile([D, 1], bf16, tag="ones128b")
            nc.vector.memset(ones128b, 1.0)
            ones_rb = pp.tile([1, D], bf16, tag="ones_rb")
            nc.vector.memset(ones_rb, 1.0)
            ones32b = pp.tile([1, HD], bf16, tag="ones32b")
            nc.vector.memset(ones32b, 1.0)
            ones512b = pp.tile([1, 512], bf16, tag="ones512b")
            nc.vector.memset(ones512b, 1.0)
            eps128 = pp.tile([D, 1], f32, tag="eps128")
            nc.vector.memset(eps128, EPS)
            eps1 = pp.tile([1, 1], f32, tag="eps1")
            nc.vector.memset(eps1, EPS)

            # ---- persistent activations (bf16) ----
            xT = big.tile([D, T], bf16, tag="chA")
            qT = big.tile([D, T], bf16, tag="chE")
            kTp = big.tile([D, bloc, H, NG, KTILE], bf16, tag="chB")
            v_sb = big.tile([D, T // KTILE, H, HD + 1], bf16, tag="chC")
            attnT = big.tile([D, T], bf16, tag="chD")
            x1T = big.tile([D, T], bf16, tag="chG")
            nc.vector.memset(v_sb[:, :, :, HD:HD + 1], 1.0)

            # ================= embedding =================
            psA_cm = tc.tile_pool(name="psA", bufs=6, space="PSUM")
            ps_sm = psA_cm.__enter__()
            for e in range(NET):
                sl = slice(e * 512, (e + 1) * 512)
                xb = wkt(NCAT)
                nc.sync.dma_start(out=xb, in_=bc(Xf[sl], NCAT))
                sb_ = wkt(NCAT)
                nc.sync.dma_start(out=sb_, in_=bc(saf[sl], NCAT))
                oh = wk.tile([NCAT, 512], bf16, tag="wk", name="oh")
                nc.vector.tensor_scalar(out=oh, in0=xb, scalar1=io25,
                                        scalar2=None, op0=OP.is_equal)
                nc.vector.tensor_mul(oh, oh, sb_)
                pb = wkt(10)
                nc.sync.dma_start(out=pb, in_=bc(ptmf[sl], 10))
                ohp = wk.tile([10, 512], bf16, tag="wk", name="ohp")
                nc.vector.tensor_scalar(out=ohp, in0=pb, scalar1=io10,
                                        scalar2=None, op0=OP.is_equal)
                pe = ps_sm.tile([D, 512], f32, tag="sm")
                nc.tensor.matmul(pe, pemb_b, ohp, start=True, stop=False)
                nc.tensor.matmul(pe, emb_b, oh, start=False, stop=True)
                nc.vector.tensor_copy(xT[:, sl], pe)

            if dbg:
                nc.sync.dma_start(out=dbg_outs["dbg_xT"][:, :], in_=xT)

            # ================= qkv =================
            for e in range(NET):
                sl = slice(e * 512, (e + 1) * 512)
                pq = ps_sm.tile([D, 512], f32, tag="sm")
                nc.tensor.matmul(pq, inwT_b[:, 0:D], xT[:, sl],
                                 start=True, stop=True)
                nc.vector.tensor_scalar(out=qT[:, sl], in0=pq, scalar1=ISQ,
                                        scalar2=qb_s, op0=OP.mult, op1=OP.add)
                pk = ps_sm.tile([D, 512], f32, tag="sm")
                nc.tensor.matmul(pk, inwT_b[:, D:2 * D], xT[:, sl],
                                 start=True, stop=True)
                ktmp = mid.tile([D, 512], bf16, tag="mid", name="ktmp")
                nc.vector.tensor_scalar(out=ktmp, in0=pk, scalar1=kb_s,
                                        scalar2=None, op0=OP.add)
                b_ = (e * 512) // lp
                for h in range(H):
                    for sub in range(4):
                        ktb = ((e * 512) % lp) // KTILE + sub
                        _q = nc.gpsimd if (h + sub) % 2 == 0 else nc.sync
                        _q.dma_start(
                            out=kTp[32 * (ktb % 4):32 * (ktb % 4) + 32,
                                    b_, h, ktb // 4, :],
                            in_=ktmp[32 * h:32 * h + 32,
                                     sub * KTILE:(sub + 1) * KTILE])
                for sub in range(4):
                    tt = (e * 512) // KTILE + sub
                    pv = ps_sm.tile([KTILE, D], f32, tag="sm")
                    nc.tensor.matmul(pv, xT[:, e * 512 + sub * KTILE:
                                            e * 512 + (sub + 1) * KTILE],
                                     inwT_b[:, 2 * D:3 * D], start=True, stop=True)
                    nc.vector.tensor_copy(
                        v_sb[:, tt, :, 0:HD],
                        pv.rearrange("p (h d) -> p h d", h=H))

            if dbg:
                nc.sync.dma_start(out=dbg_outs["dbg_qT"][:, :], in_=qT)
                kT_dbg = big.tile([D, T], bf16, tag="chDBG")
                for b_ in range(bloc):
                    for h in range(H):
                        for g in range(NG):
                            for r in range(4):
                                kt = 4 * g + r
                                nc.sync.dma_start(
                                    out=kT_dbg[32 * h:32 * h + 32,
                                               b_ * lp + kt * KTILE:
                                               b_ * lp + (kt + 1) * KTILE],
                                    in_=kTp[32 * r:32 * r + 32, b_, h, g, :])
                nc.sync.dma_start(out=dbg_outs["dbg_kT"][:, :], in_=kT_dbg)

            psA_cm.__exit__(None, None, None)

            # ================= attention =================
            # 2 key-tiles per subgroup so sc fits 2 PSUM banks and can be
            # triple-buffered: QK of subgroup p+2 overlaps exp of p,
            # PV of p overlaps exp of p+1. Band bias applied post-exp as a
            # bf16 multiply on pt (off the QK->exp critical path).
            psB1_cm = tc.tile_pool(name="ps_sc", bufs=3, space="PSUM")
            ps_sc = psB1_cm.__enter__()
            psB2_cm = tc.tile_pool(name="ps_pv", bufs=2, space="PSUM")
            ps_pv = psB2_cm.__enter__()
            den32 = longs.tile([NDEN, 512], bf16, tag="den32")
            for b_ in range(bloc):
                for h in range(H):
                    qrep = qrp.tile([D, lp], bf16, tag="qr")
                    for r in range(4):
                        nc.gpsimd.dma_start(
                            out=qrep[32 * r:32 * r + 32, :],
                            in_=qT[32 * h:32 * h + 32, b_ * lp:(b_ + 1) * lp])
                    for qt in range(NQT):
                        ppv = ps_pv.tile([HD + 1, 512], f32, tag="pv")
                        for p_ in range(2 * NG):
                            sc = ps_sc.tile([D, 2 * 512], f32, tag="sc")
                            for j in range(2):
                                kt = 2 * p_ + j
                                nc.tensor.matmul(
                                    sc[:, j * 512:(j + 1) * 512],
                                    kTp[32 * (kt % 4):32 * (kt % 4) + 32,
                                        b_, h, kt // 4, :],
                                    qrep[32 * (kt % 4):32 * (kt % 4) + 32,
                                         qt * QT:(qt + 1) * QT],
                                    start=True, stop=True,
                                    tile_position=(32 * (kt % 4), 0))
                            side = 0 if (p_ >> 1) <= qt else 1
                            pt = ptp.tile([D, 2 * 512], bf16, tag="pt")
                            nc.scalar.activation(pt, sc, AF.Exp,
                                                 bias=cexp[:, 2 * h + side:
                                                           2 * h + side + 1],
                                                 scale=1.0)
                            for j in range(2):
                                kt = 2 * p_ + j
                                dd = kt - 4 * qt
                                if -1 <= dd <= 4:
                                    di = dd + 1
                                    c0, w = BAND_C0[di], BAND_W[di]
                                    nc.vector.tensor_tensor(
                                        out=pt[:, j * 512 + c0:j * 512 + c0 + w],
                                        in0=pt[:, j * 512 + c0:j * 512 + c0 + w],
                                        in1=bandexp[:, h * BAND_TOT + BAND_OFF[di]:
                                                    h * BAND_TOT + BAND_OFF[di] + w],
                                        op=OP.mult)
                            for j in range(2):
                                kt = 2 * p_ + j
                                nc.tensor.matmul(
                                    ppv, v_sb[:, b_ * NKT + kt, h, :],
                                    pt[:, j * 512:(j + 1) * 512],
                                    start=(kt == 0), stop=(kt == NKT - 1))
                        pv_sb = wk.tile([HD + 1, 512], bf16, tag="wk", name="pv_sb")
                        nc.vector.tensor_copy(pv_sb, ppv)
                        nc.gpsimd.dma_start(
                            out=attnT[32 * h:32 * h + 32,
                                      b_ * lp + qt * QT:b_ * lp + (qt + 1) * QT],
                            in_=pv_sb[0:HD, :])
                        nc.gpsimd.dma_start(
                            out=den32[(b_ * NQT + qt) * H + h:
                                      (b_ * NQT + qt) * H + h + 1, :],
                            in_=pv_sb[HD:HD + 1, :])

            psB2_cm.__exit__(None, None, None)
            psB1_cm.__exit__(None, None, None)
            psC_cm = tc.tile_pool(name="psC", bufs=6, space="PSUM")
            ps_sm = psC_cm.__enter__()

            lnden = midt(NDEN, 512)
            nc.scalar.activation(lnden, den32, AF.Ln, bias=0.0, scale=1.0)
            recip = longs.tile([NDEN, 512], bf16, tag="recip")
            nc.scalar.activation(recip, lnden, AF.Exp, bias=0.0, scale=-1.0)

            for b_ in range(bloc):
                for qt in range(NQT):
                    bcp = ps_sm.tile([D, 512], f32, tag="sm")
                    for h in range(H):
                        rr = rowp.tile([1, 512], bf16, tag="row")
                        nc.gpsimd.dma_start(
                            out=rr, in_=recip[(b_ * NQT + qt) * H + h:
                                              (b_ * NQT + qt) * H + h + 1, :])
                        nc.tensor.matmul(bcp[32 * h:32 * h + 32, :], ones32b, rr,
                                         start=True, stop=True,
                                         tile_position=(0, 32 * h))
                    sl = slice(b_ * lp + qt * QT, b_ * lp + (qt + 1) * QT)
                    nc.vector.tensor_mul(attnT[:, sl], attnT[:, sl], bcp)

            if dbg:
                nc.sync.dma_start(out=dbg_outs["dbg_attnT"][:, :], in_=attnT)

            # ======== layernorm helper: per-tile stats packed into wide
            # [1, NLT*512] rows -> one Ln + one Exp per call (one table
            # switch each); gamma folded into the rank-1 broadcast
            # (lhsT = g row), -beta accumulated via a ones-row matmul ========
            def layernorm_T(src, dst_fn, g_row, nb_row, b_base):
                m_all = lnr.tile([1, NLT * 512], bf16, tag="mall", name="m_all")
                v_all = lnr.tile([1, NLT * 512], f32, tag="vall", name="v_all")
                for t_ in range(NLT):
                    sl = slice(b_base * lp + t_ * 512, b_base * lp + (t_ + 1) * 512)
                    rsl = slice(t_ * 512, (t_ + 1) * 512)
                    sq = wk.tile([D, 512], bf16, tag="wk", name="sq")
                    nc.vector.tensor_mul(sq, src[:, sl], src[:, sl])
                    p1_ = ps_sm.tile([1, 512], f32, tag="sm")
                    nc.tensor.matmul(p1_, ones128b, src[:, sl], start=True, stop=True)
                    p2_ = ps_sm.tile([1, 512], f32, tag="sm")
                    nc.tensor.matmul(p2_, ones128b, sq, start=True, stop=True)
                    nc.vector.tensor_scalar(out=m_all[:, rsl], in0=p1_,
                                            scalar1=1.0 / D,
                                            scalar2=None, op0=OP.mult)
                    nc.vector.tensor_scalar(out=v_all[:, rsl], in0=p2_,
                                            scalar1=1.0 / D,
                                            scalar2=None, op0=OP.mult)
                    msq = rowp.tile([1, 512], f32, tag="row", name="msq")
                    nc.vector.tensor_mul(msq, m_all[:, rsl], m_all[:, rsl])
                    nc.vector.tensor_tensor(out=v_all[:, rsl], in0=v_all[:, rsl],
                                            in1=msq, op=OP.subtract)
                lnv = lnr.tile([1, NLT * 512], f32, tag="lall", name="lnv")
                nc.scalar.activation(lnv, v_all, AF.Ln, bias=eps1, scale=1.0)
                rr_all = lnr.tile([1, NLT * 512], bf16, tag="rall", name="rr_all")
                nc.scalar.activation(rr_all, lnv, AF.Exp, bias=0.0, scale=-0.5)
                rm_all = lnr.tile([1, NLT * 512], bf16, tag="rmall", name="rm_all")
                nc.vector.tensor_mul(rm_all, m_all, rr_all)
                for t_ in range(NLT):
                    sl = slice(b_base * lp + t_ * 512, b_base * lp + (t_ + 1) * 512)
                    rsl = slice(t_ * 512, (t_ + 1) * 512)
                    br = ps_sm.tile([D, 512], f32, tag="sm")
                    nc.tensor.matmul(br, g_row, rr_all[:, rsl], start=True, stop=True)
                    bm = ps_sm.tile([D, 512], f32, tag="sm")
                    nc.tensor.matmul(bm, g_row, rm_all[:, rsl], start=True, stop=False)
                    nc.tensor.matmul(bm, nb_row, ones512b, start=False, stop=True)
                    tmp = wk.tile([D, 512], f32, tag="wk", name="lntmp")
                    nc.vector.tensor_mul(tmp, src[:, sl], br)
                    nc.vector.tensor_tensor(out=dst_fn(t_), in0=tmp, in1=bm,
                                            op=OP.subtract)

            # ================= out-proj + residual + LN1 =================
            r1T = big.tile([D, T], bf16, tag="chB")   # after kTp's last read
            for b_ in range(bloc):
                for qt in range(NQT):
                    sl = slice(b_ * lp + qt * QT, b_ * lp + (qt + 1) * QT)
                    po = ps_sm.tile([D, 512], f32, tag="sm")
                    nc.tensor.matmul(po, woT_b, attnT[:, sl], start=True, stop=True)
                    nc.vector.tensor_scalar(out=r1T[:, sl], in0=po, scalar1=ob_s,
                                            scalar2=None, op0=OP.add)
                    nc.vector.tensor_tensor(out=r1T[:, sl], in0=r1T[:, sl],
                                            in1=xT[:, sl], op=OP.add)
            for b_ in range(bloc):
                layernorm_T(
                    r1T,
                    lambda t_, b0=b_: x1T[:, b0 * lp + t_ * 512:
                                          b0 * lp + (t_ + 1) * 512],
                    ln1g_r, ln1nb_r, b_)

            if dbg:
                nc.sync.dma_start(out=dbg_outs["dbg_x1T"][:, :], in_=x1T)

            # ================= FFN + residual + LN2 =================
            x2pad = big.tile([D, bloc * (lp + 4)], bf16, tag="chA")  # after xT
            nc.vector.memset(x2pad[:, :], 0.0)
            r2T = big.tile([D, T], bf16, tag="chC")                  # after v_sb
            for b_ in range(bloc):
                for qt in range(NQT):
                    sl = slice(b_ * lp + qt * QT, b_ * lp + (qt + 1) * QT)
                    h1a = wk.tile([D, 512], bf16, tag="wk", name="h1a")
                    h1b = wk.tile([D, 512], bf16, tag="wk", name="h1b")
                    for half, dest in ((0, h1a), (1, h1b)):
                        ph = ps_sm.tile([D, 512], f32, tag="sm")
                        nc.tensor.matmul(ph, l1wT_b[:, half * D:(half + 1) * D],
                                         x1T[:, sl], start=True, stop=True)
                        nc.scalar.activation(dest, ph, AF.Relu,
                                             bias=l1b_s[:, half:half + 1],
                                             scale=1.0)
                    py = ps_sm.tile([D, 512], f32, tag="sm")
                    nc.tensor.matmul(py, l2cat_b[:, 0:D], h1a, start=True, stop=False)
                    nc.tensor.matmul(py, l2cat_b[:, D:2 * D], h1b,
                                     start=False, stop=True)
                    nc.vector.tensor_scalar(out=r2T[:, sl], in0=py, scalar1=l2b_s,
                                            scalar2=None, op0=OP.add)
                    nc.vector.tensor_tensor(out=r2T[:, sl], in0=r2T[:, sl],
                                            in1=x1T[:, sl], op=OP.add)
            for b_ in range(bloc):
                layernorm_T(
                    r2T,
                    lambda t_, b0=b_: x2pad[:, b0 * (lp + 4) + 2 + t_ * 512:
                                            b0 * (lp + 4) + 2 + (t_ + 1) * 512],
                    ln2g_r, ln2nb_r, b_)

            if dbg:
                for b_ in range(bloc):
                    nc.sync.dma_start(
                        out=dbg_outs["dbg_x2"][:, b_ * lp:(b_ + 1) * lp],
                        in_=x2pad[:, b_ * (lp + 4) + 2:b_ * (lp + 4) + 2 + lp])

            # ================= conv1 + bn1 =================
            c1_sb = big.tile([C1, bloc * lp], bf16, tag="chD")   # after attnT
            bnst1 = longs.tile([C1, bloc * NLT, 6], f32, tag="bnst1")
            for b_ in range(bloc):
                for t_ in range(NLT):
                    pc = ps_sm.tile([C1, 512], f32, tag="sm")
                    for k_ in range(K):
                        nc.tensor.matmul(
                            pc, c1wT_b[:, k_ * C1:(k_ + 1) * C1],
                            x2pad[:, b_ * (lp + 4) + t_ * 512 + k_:
                                  b_ * (lp + 4) + t_ * 512 + k_ + 512],
                            start=(k_ == 0), stop=(k_ == K - 1))
                    nc.vector.bn_stats(out=bnst1[:, b_ * NLT + t_, :], in_=pc)
                    nc.vector.tensor_copy(
                        c1_sb[:, b_ * lp + t_ * 512:b_ * lp + (t_ + 1) * 512], pc)
            mv1 = wk.tile([C1, 2], f32, tag="wk")
            nc.vector.bn_aggr(out=mv1, in_=bnst1)
            part1 = wk.tile([C1, 2], f32, tag="wk")
            sqm = wk.tile([C1, 1], f32, tag="wk")
            nc.vector.tensor_mul(sqm, mv1[:, 0:1], mv1[:, 0:1])
            nc.vector.tensor_tensor(out=sqm, in0=sqm, in1=mv1[:, 1:2], op=OP.add)
            nl_ = float(bloc * lp)
            nc.vector.tensor_scalar(out=part1[:, 0:1], in0=mv1[:, 0:1],
                                    scalar1=nl_, scalar2=None, op0=OP.mult)
            nc.vector.tensor_scalar(out=part1[:, 1:2], in0=sqm,
                                    scalar1=nl_, scalar2=None, op0=OP.mult)
            bn1_in = dram.tile([C1, 2], f32, tag="bn1i")
            bn1_out = dram.tile([C1, 2], f32, tag="bn1o")
            nc.sync.dma_start(out=bn1_in, in_=part1)
            nc.gpsimd.collective_compute(
                "AllReduce", OP.add, replica_groups=[list(range(n_cores))],
                ins=[bn1_in[:, :].opt()], outs=[bn1_out[:, :].opt()])
            glob1 = wk.tile([C1, 2], f32, tag="wk")
            nc.sync.dma_start(out=glob1, in_=bn1_out)

            def bn_scale_shift(globc, n_, g_ap, b_ap):
                mean = wk.tile([C1, 1], f32, tag="wk")
                nc.vector.tensor_scalar(out=mean, in0=globc[:, 0:1],
                                        scalar1=1.0 / n_, scalar2=None, op0=OP.mult)
                ex2 = wk.tile([C1, 1], f32, tag="wk")
                nc.vector.tensor_scalar(out=ex2, in0=globc[:, 1:2],
                                        scalar1=1.0 / n_, scalar2=None, op0=OP.mult)
                msq_ = wk.tile([C1, 1], f32, tag="wk")
                nc.vector.tensor_mul(msq_, mean, mean)
                nc.vector.tensor_tensor(out=ex2, in0=ex2, in1=msq_, op=OP.subtract)
                lnv_ = wk.tile([C1, 1], f32, tag="wk")
                nc.scalar.activation(lnv_, ex2, AF.Ln, bias=eps128, scale=1.0)
                rstd_ = wk.tile([C1, 1], f32, tag="wk")
                nc.scalar.activation(rstd_, lnv_, AF.Exp, bias=0.0, scale=-0.5)
                scale = longs.tile([C1, 1], f32, tag="bnsc")
                nc.vector.tensor_mul(scale, rstd_, g_ap)
                shift = longs.tile([C1, 1], f32, tag="bnsh")
                nc.vector.tensor_mul(shift, mean, scale)
                nc.vector.tensor_tensor(out=shift, in0=b_ap, in1=shift,
                                        op=OP.subtract)
                return scale, shift

            sc1, sh1 = bn_scale_shift(glob1, n1, bn1g, bn1b)
            p1_sb = big.tile([C1, bloc * LP2], bf16, tag="chF")   # after band
            if dbg:
                nc.sync.dma_start(out=dbg_outs["dbg_c1"][:, :], in_=c1_sb)
            for b_ in range(bloc):
                for t_ in range(NLT):
                    rel = wk.tile([D, 512], bf16, tag="wk", name="rel")
                    nc.scalar.activation(
                        rel, c1_sb[:, b_ * lp + t_ * 512:b_ * lp + (t_ + 1) * 512],
                        AF.Relu, bias=sh1, scale=sc1)
                    rel2 = rel.rearrange("p (l two) -> p l two", two=2)
                    nc.vector.tensor_tensor(
                        out=p1_sb[:, b_ * LP2 + t_ * 256:b_ * LP2 + (t_ + 1) * 256],
                        in0=rel2[:, :, 0], in1=rel2[:, :, 1], op=OP.max)
            if dbg:
                nc.sync.dma_start(out=dbg_outs["dbg_p1"][:, :], in_=p1_sb)

            # ================= conv2 + bn2 =================
            c2_sb = big.tile([C1, bloc * 2 * L2], bf16, tag="chE")  # after qT
            bnst2 = longs.tile([C1, 2, bloc * 2, 6], f32, tag="bnst2")
            for b_ in range(bloc):
                for half in range(2):
                    for t_ in range(2):
                        pc = ps_sm.tile([C1, LT2], f32, tag="sm")
                        for k_ in range(K):
                            nc.tensor.matmul(
                                pc, c2wT_b[:, k_ * C2 + half * C1:
                                           k_ * C2 + (half + 1) * C1],
                                p1_sb[:, b_ * LP2 + t_ * LT2 + k_:
                                      b_ * LP2 + t_ * LT2 + k_ + LT2],
                                start=(k_ == 0), stop=(k_ == K - 1))
                        nc.vector.bn_stats(out=bnst2[:, half, b_ * 2 + t_, :],
                                           in_=pc)
                        nc.vector.tensor_copy(
                            c2_sb[:, (b_ * 2 + half) * L2 + t_ * LT2:
                                  (b_ * 2 + half) * L2 + (t_ + 1) * LT2], pc)
            part2 = longs.tile([C1, 4], f32, tag="part2")
            for half in range(2):
                mv2 = wk.tile([C1, 2], f32, tag="wk")
                nc.vector.bn_aggr(out=mv2, in_=bnst2[:, half, :, :])
                sqm2 = wk.tile([C1, 1], f32, tag="wk")
                nc.vector.tensor_mul(sqm2, mv2[:, 0:1], mv2[:, 0:1])
                nc.vector.tensor_tensor(out=sqm2, in0=sqm2, in1=mv2[:, 1:2],
                                        op=OP.add)
                nl2 = float(bloc * L2)
                nc.vector.tensor_scalar(out=part2[:, 2 * half:2 * half + 1],
                                        in0=mv2[:, 0:1], scalar1=nl2,
                                        scalar2=None, op0=OP.mult)
                nc.vector.tensor_scalar(out=part2[:, 2 * half + 1:2 * half + 2],
                                        in0=sqm2, scalar1=nl2,
                                        scalar2=None, op0=OP.mult)
            bn2_in = dram.tile([C1, 4], f32, tag="bn2i")
            bn2_out = dram.tile([C1, 4], f32, tag="bn2o")
            nc.sync.dma_start(out=bn2_in, in_=part2)
            nc.gpsimd.collective_compute(
                "AllReduce", OP.add, replica_groups=[list(range(n_cores))],
                ins=[bn2_in[:, :].opt()], outs=[bn2_out[:, :].opt()])
            glob2 = longs.tile([C1, 4], f32, tag="glob2")
            nc.sync.dma_start(out=glob2, in_=bn2_out)

            feat = longs.tile([C1, 2 * bloc], f32, tag="feat")
            # vectorized bn2 scale/shift for both halves: [C1, 2]
            g2v = glob2.rearrange("p (h two) -> p h two", two=2)
            mean2 = wk.tile([C1, 2], f32, tag="wk")
            nc.vector.tensor_scalar(out=mean2, in0=g2v[:, :, 0], scalar1=1.0 / n2,
                                    scalar2=None, op0=OP.mult)
            ex22 = wk.tile([C1, 2], f32, tag="wk")
            nc.vector.tensor_scalar(out=ex22, in0=g2v[:, :, 1], scalar1=1.0 / n2,
                                    scalar2=None, op0=OP.mult)
            msq2 = wk.tile([C1, 2], f32, tag="wk")
            nc.vector.tensor_mul(msq2, mean2, mean2)
            nc.vector.tensor_tensor(out=ex22, in0=ex22, in1=msq2, op=OP.subtract)
            lnv2 = wk.tile([C1, 2], f32, tag="wk")
            nc.scalar.activation(lnv2, ex22, AF.Ln, bias=eps128, scale=1.0)
            rstd2 = wk.tile([C1, 2], f32, tag="wk")
            nc.scalar.activation(rstd2, lnv2, AF.Exp, bias=0.0, scale=-0.5)
            scale2 = longs.tile([C1, 2], f32, tag="bnsc2")
            nc.vector.tensor_mul(scale2, rstd2, bn2g)
            shift2 = longs.tile([C1, 2], f32, tag="bnsh2")
            nc.vector.tensor_mul(shift2, mean2, scale2)
            nc.vector.tensor_tensor(out=shift2, in0=bn2b, in1=shift2,
                                    op=OP.subtract)
            for half in range(2):
                sc2 = scale2[:, half:half + 1]
                sh2 = shift2[:, half:half + 1]
                for b_ in range(bloc):
                    rel = wkb.tile([C1, L2], bf16, tag="wkb")
                    nc.scalar.activation(
                        rel, c2_sb[:, (b_ * 2 + half) * L2:
                                   (b_ * 2 + half + 1) * L2],
                        AF.Relu, bias=sh2, scale=sc2)
                    nc.vector.reduce_max(
                        out=feat[:, (b_ * 2 + half):(b_ * 2 + half) + 1],
                        in_=rel, axis=AX.X)
            if dbg:
                nc.sync.dma_start(out=dbg_outs["dbg_feat"][:, :], in_=feat)

            # ================= fc =================
            for b_ in range(bloc):
                pf = ps_sm.tile([NC, 1], f32, tag="sm")
                for half in range(2):
                    nc.tensor.matmul(pf, fcwT[:, half * NC:(half + 1) * NC],
                                     feat[:, b_ * 2 + half:b_ * 2 + half + 1],
                                     start=(half == 0), stop=(half == 1))
                ob2 = wk.tile([NC, 1], f32, tag="wk")
                nc.vector.tensor_scalar(out=ob2, in0=pf, scalar1=fcb_s,
                                        scalar2=None, op0=OP.add)
                nc.sync.dma_start(out=out_d[b_, :], in_=ob2[:, 0])
            psC_cm.__exit__(None, None, None)

    nc.compile()
    return nc


def _host_inputs(inputs, n_cores, bloc, lp):
    X = np.asarray(inputs["X"]).astype(np.float32)[:, :lp]
    sa = np.asarray(inputs["surface_availability"], dtype=np.float32)[:, :lp]
    ptm = np.asarray(inputs["ptm"]).astype(np.float32)[:, :lp]
    emb = np.asarray(inputs["emb"], dtype=np.float32)
    pemb = np.asarray(inputs["ptm_emb"], dtype=np.float32)
    rpe = np.asarray(inputs["rpe"], dtype=np.float32)
    inw = np.asarray(inputs["in_proj_w"], dtype=np.float32)
    inb = np.asarray(inputs["in_proj_b"], dtype=np.float32)
    wo = np.asarray(inputs["out_proj_w"], dtype=np.float32)
    bo = np.asarray(inputs["out_proj_b"], dtype=np.float32)
    w1 = np.asarray(inputs["lin1_w"], dtype=np.float32)
    b1 = np.asarray(inputs["lin1_b"], dtype=np.float32)
    w2 = np.asarray(inputs["lin2_w"], dtype=np.float32)
    b2 = np.asarray(inputs["lin2_b"], dtype=np.float32)
    c1w = np.asarray(inputs["conv1_w"], dtype=np.float32)
    c2w = np.asarray(inputs["conv2_w"], dtype=np.float32)
    fcw = np.asarray(inputs["fc_w"], dtype=np.float32)

    pembp = np.zeros((10, D), np.float32)
    pembp[:, ED:] = pemb
    embp = np.zeros((NCAT, D), np.float32)
    embp[:, :ED] = emb

    clo, chi = rpe[0], rpe[2 * MD]
    bandcat = np.zeros((D, H * BAND_TOT), np.float32)
    jj = np.arange(128)[:, None]
    for h in range(H):
        for di, dl in enumerate(BAND_DELTAS):
            w = BAND_W[di]
            ii = np.arange(BAND_C0[di], BAND_C0[di] + w)[None, :]
            e = dl + jj - ii
            val = rpe[np.clip(e, -MD, MD) + MD, h]
            beta = chi[h] if di == 5 else clo[h]
            bandcat[:, h * BAND_TOT + BAND_OFF[di]:
                    h * BAND_TOT + BAND_OFF[di] + w] = val - beta
    cexp = np.zeros((D, 2 * H), np.float32)
    for h in range(H):
        cexp[:, 2 * h] = clo[h]
        cexp[:, 2 * h + 1] = chi[h]

    ob_eff = bo + wo @ inb[2 * D:3 * D]
    l2t = w2.T   # [FF, D]
    shared = {
        "embp": embp, "pembp": pembp,
        "iota25": np.arange(NCAT, dtype=np.float32)[:, None],
        "iota10": np.arange(10, dtype=np.float32)[:, None],
        "inwT": np.ascontiguousarray(inw.T),
        "qb": (inb[0:D] * ISQ)[:, None],
        "kb": inb[D:2 * D][:, None],
        "woT": np.ascontiguousarray(wo.T),
        "ob_eff": ob_eff[:, None].astype(np.float32),
        "l1wT": np.ascontiguousarray(w1.T),
        "l1b": b1.reshape(2, D).T.copy(),
        "l2cat": np.concatenate([l2t[0:D], l2t[D:2 * D]], axis=1).copy(),
        "l2b": b2[:, None],
        "ln1g": np.asarray(inputs["ln1_g"], np.float32)[:, None],
        "ln1b": np.asarray(inputs["ln1_b"], np.float32)[:, None],
        "ln2g": np.asarray(inputs["ln2_g"], np.float32)[:, None],
        "ln2b": np.asarray(inputs["ln2_b"], np.float32)[:, None],
        "bandcat": bandcat, "cexp": cexp,
        "c1wT": np.ascontiguousarray(c1w.transpose(1, 2, 0).reshape(D, K * C1)),
        "c2wT": np.ascontiguousarray(c2w.transpose(1, 2, 0).reshape(C1, K * C2)),
        "bn1g": np.asarray(inputs["bn1_g"], np.float32)[:, None],
        "bn1b": np.asarray(inputs["bn1_b"], np.float32)[:, None],
        "bn2g": np.asarray(inputs["bn2_g"], np.float32).reshape(2, C1).T.copy(),
        "bn2b": np.asarray(inputs["bn2_b"], np.float32).reshape(2, C1).T.copy(),
        "fcwT": np.ascontiguousarray(
            fcw.T.reshape(2, C1, NC).transpose(1, 0, 2).reshape(C1, 2 * NC)),
        "fcb": np.asarray(inputs["fc_b"], np.float32)[:, None],
    }
    in_maps = []
    for c in range(n_cores):
        rows = slice(c * bloc, (c + 1) * bloc)
        m = dict(shared)
        m["Xf"] = np.ascontiguousarray(X[rows].reshape(-1))
        m["saf"] = np.ascontiguousarray(sa[rows].reshape(-1))
        m["ptmf"] = np.ascontiguousarray(ptm[rows].reshape(-1))
        in_maps.append(m)
    return in_maps


_NC_CACHE = {}


def _get_nc(n_cores, bloc, lp, dbg=False):
    key = (n_cores, bloc, lp, dbg)
    if key not in _NC_CACHE:
        _NC_CACHE[key] = _build(n_cores, bloc, lp, dbg=dbg)
    return _NC_CACHE[key]


def kernel(**inputs):
    from concourse.bass_utils import run_bass_kernel_spmd
    nc = _get_nc(NCORES, BLOC, L)
    in_maps = _host_inputs(inputs, NCORES, BLOC, L)
    res = run_bass_kernel_spmd(nc, in_maps, list(range(NCORES)))
    out = np.concatenate([res.results[i]["out"] for i in range(NCORES)], axis=0)
    return out.astype(np.float32)


# revision 25
# speedup vs baseline: 2.3056x; 1.1805x over previous
"""Trainium2 Bass kernel for nn_CNN2LWithRPE (transformer layer + CNN head).

Sharding: data-parallel over batch across 8 NeuronCores (2 batch rows each).
All parameters replicated. The only cross-core communication is two tiny
AllReduces for the training-mode BatchNorm statistics.

Per-core layout (B_loc batches, T = B_loc*L tokens):
  - activations transposed in SBUF as bf16: xT/qT/x1T... are [D=128, T]
    (bf16 operands run the PE at 1 cycle/row vs 4 for fp32; PSUM
    accumulation stays fp32)
  - attention as scores^T tiles [keys=128 part, queries=512 free]:
    QK^T row-packed 4x on PE (K=32) from a shuffled kTp layout + a 4x
    replicated qrep tile; exp on ACT as wide [128,2048] ops with the
    clipped-RPE bias folded into a per-group bias constant (c_lo/c_hi)
    plus narrow host-precomputed banded correction tiles added on DVE;
    PV uses v in natural layout with an appended ones-column so the
    softmax denominator falls out of the same matmul.
  - layernorm in transposed layout: partition stats via ones-matmul,
    rstd = Exp(-0.5*Ln(var+eps)), rank-1 K=1 matmul broadcast back.
  - conv1d as K accumulating shifted matmuls; BN apply fused into
    ACT Relu(scale*x+shift) with per-channel scale/shift APs.
"""

import numpy as np

B, L = 16, 2048
NCAT, ED = 25, 120
D, H, HD = 128, 4, 32
FF = 256
MD = 32
C1, C2, K = 128, 256, 5
NC = 2
EPS = 1e-5
NCORES = 8
BLOC = B // NCORES
ISQ = float(1.0 / np.sqrt(HD))

QT = 512
KTILE = 128

BAND_DELTAS = [-128, 0, 128, 256, 384, 512]
BAND_W = [32, 160, 288, 416, 512, 32]
BAND_C0 = [0, 0, 0, 0, 0, 480]
BAND_OFF = [0, 32, 192, 480, 896, 1408]
BAND_TOT = 1440


def _build(n_cores, bloc, lp, dbg=False):
    import contextlib
    import concourse.bass as bass
    import concourse.tile as tile
    from concourse import bacc, mybir

    f32 = mybir.dt.float32
    bf16 = mybir.dt.bfloat16
    AF = mybir.ActivationFunctionType
    OP = mybir.AluOpType
    AX = mybir.AxisListType

    T = bloc * lp
    NET = T // 512
    NQT = lp // QT
    NKT = lp // KTILE
    NG = NKT // 4
    NLT = lp // 512
    NDEN = bloc * NQT * H
    LP2 = lp // 2
    L2 = LP2 - (K - 1)
    LT2 = L2 // 2
    n1 = float(n_cores * bloc * lp)
    n2 = float(n_cores * bloc * L2)

    nc = bacc.Bacc("TRN2", target_bir_lowering=False, debug=False,
                   num_devices=n_cores)

    def din(name, shape):
        return nc.dram_tensor(name, list(shape), f32, kind="ExternalInput")

    Xf = din("Xf", [T])
    saf = din("saf", [T])
    ptmf = din("ptmf", [T])
    emb_d = din("embp", [NCAT, D])
    pemb_d = din("pembp", [10, D])
    iota25_d = din("iota25", [NCAT, 1])
    iota10_d = din("iota10", [10, 1])
    inwT_d = din("inwT", [D, 3 * D])
    qb_d = din("qb", [D, 1])
    kb_d = din("kb", [D, 1])
    woT_d = din("woT", [D, D])
    ob_d = din("ob_eff", [D, 1])
    l1wT_d = din("l1wT", [D, FF])
    l1b_d = din("l1b", [D, 2])
    l2cat_d = din("l2cat", [D, 2 * D])
    l2b_d = din("l2b", [D, 1])
    ln1g_d = din("ln1g", [D, 1])
    ln1b_d = din("ln1b", [D, 1])
    ln2g_d = din("ln2g", [D, 1])
    ln2b_d = din("ln2b", [D, 1])
    band_d = din("bandcat", [D, H * BAND_TOT])
    cexp_d = din("cexp", [D, 2 * H])
    c1wT_d = din("c1wT", [D, K * C1])
    c2wT_d = din("c2wT", [C1, K * C2])
    bn1g_d = din("bn1g", [C1, 1])
    bn1b_d = din("bn1b", [C1, 1])
    bn2g_d = din("bn2g", [C1, 2])
    bn2b_d = din("bn2b", [C1, 2])
    fcwT_d = din("fcwT", [C1, 2 * NC])
    fcb_d = din("fcb", [NC, 1])

    out_d = nc.dram_tensor("out", [bloc, NC], f32, kind="ExternalOutput")
    dbg_outs = {}
    if dbg:
        for nm, shp, dt_ in [("dbg_xT", [D, T], bf16),
                             ("dbg_qT", [D, T], bf16),
                             ("dbg_kT", [D, T], bf16),
                             ("dbg_attnT", [D, T], bf16),
                             ("dbg_x1T", [D, T], bf16),
                             ("dbg_x2", [D, T], bf16),
                             ("dbg_c1", [C1, bloc * lp], bf16),
                             ("dbg_p1", [C1, bloc * LP2], bf16),
                             ("dbg_feat", [C1, 2 * bloc], f32)]:
            dbg_outs[nm] = nc.dram_tensor(nm, shp, dt_, kind="ExternalOutput")

    def bc(ap1d, parts):
        return bass.AP(tensor=ap1d.tensor, offset=ap1d.offset,
                       ap=[[0, parts]] + [list(p) for p in ap1d.ap])

    with tile.TileContext(nc) as tc:
        ctx = contextlib.ExitStack()
        with ctx:
            pp = ctx.enter_context(tc.tile_pool(name="params", bufs=1))
            big = ctx.enter_context(tc.tile_pool(name="big", bufs=1))
            wk = ctx.enter_context(tc.tile_pool(name="wk", bufs=12))
            wkb = ctx.enter_context(tc.tile_pool(name="wkb", bufs=2))
            longs = ctx.enter_context(tc.tile_pool(name="longs", bufs=1))
            mid = ctx.enter_context(tc.tile_pool(name="mid", bufs=6))
            rowp = ctx.enter_context(tc.tile_pool(name="rows", bufs=6))
            lnr = ctx.enter_context(tc.tile_pool(name="lnrows", bufs=1))
            ptp = ctx.enter_context(tc.tile_pool(name="pt", bufs=3))
            qrp = ctx.enter_context(tc.tile_pool(name="qrep", bufs=2))
            dram = ctx.enter_context(tc.tile_pool(name="dram", bufs=1, space="DRAM"))

            def wkt(p=D, f=512):
                return wk.tile([p, f], f32, tag="wk", name="wkt")

            def midt(p, f):
                return mid.tile([p, f], f32, tag="mid", name="midt")

            # ---- params ----
            def ld(dten, shape):
                t = pp.tile(list(shape), f32, tag=dten.name, name=dten.name + "_s")
                nc.sync.dma_start(out=t, in_=dten[tuple(slice(0, s) for s in shape)])
                return t

            def ldb(dten, shape):
                # load f32 param, convert to a bf16 copy for PE consumption
                t = ld(dten, shape)
                tb = pp.tile(list(shape), bf16, tag=dten.name + "_b",
                             name=dten.name + "_b")
                nc.vector.tensor_copy(tb, t)
                return tb

            emb_b = ldb(emb_d, [NCAT, D])
            pemb_b = ldb(pemb_d, [10, D])
            io25 = ld(iota25_d, [NCAT, 1])
            io10 = ld(iota10_d, [10, 1])
            inwT_b = ldb(inwT_d, [D, 3 * D])
            qb_s = ld(qb_d, [D, 1])
            kb_s = ld(kb_d, [D, 1])
            woT_b = ldb(woT_d, [D, D])
            ob_s = ld(ob_d, [D, 1])
            l1wT_b = ldb(l1wT_d, [D, FF])
            l1b_s = ld(l1b_d, [D, 2])
            l2cat_b = ldb(l2cat_d, [D, 2 * D])
            l2b_s = ld(l2b_d, [D, 1])
            ln1g = ld(ln1g_d, [D, 1]); ln1b = ld(ln1b_d, [D, 1])
            ln2g = ld(ln2g_d, [D, 1]); ln2b = ld(ln2b_d, [D, 1])
            # transposed bf16 rows of LN gamma / negated beta for rank-1
            # broadcast matmuls: br = g^T x rstd_row, bm += (-b)^T x ones_row
            def trow(col_tile, name, negate=False):
                rf = pp.tile([1, D], f32, tag=name + "f", name=name + "f")
                nc.sync.dma_start(out=rf, in_=col_tile[:, 0])
                rb = pp.tile([1, D], bf16, tag=name, name=name)
                nc.vector.tensor_scalar(out=rb, in0=rf,
                                        scalar1=-1.0 if negate else 1.0,
                                        scalar2=None, op0=OP.mult)
                return rb
            ob_r = trow(ob_s, "ob_r")
            l2b_r = trow(l2b_s, "l2b_r")
            ln1g_r = trow(ln1g, "ln1g_r")
            ln1nb_r = trow(ln1b, "ln1nb_r", negate=True)
            ln2g_r = trow(ln2g, "ln2g_r")
            ln2nb_r = trow(ln2b, "ln2nb_r", negate=True)
            cexp = ld(cexp_d, [D, 2 * H])
            c1wT_b = ldb(c1wT_d, [D, K * C1])
            c2wT_b = ldb(c2wT_d, [C1, K * C2])
            bn1g = ld(bn1g_d, [C1, 1]); bn1b = ld(bn1b_d, [C1, 1])
            bn2g = ld(bn2g_d, [C1, 2]); bn2b = ld(bn2b_d, [C1, 2])
            fcwT = ld(fcwT_d, [C1, 2 * NC])
            fcb_s = ld(fcb_d, [NC, 1])

            band_f = big.tile([D, H * BAND_TOT], f32, tag="chF0")
            nc.sync.dma_start(out=band_f, in_=band_d[:, :])
            bandexp = big.tile([D, H * BAND_TOT], bf16, tag="chF")
            nc.scalar.activation(bandexp, band_f, AF.Exp, bias=0.0, scale=1.0)

            ones128b = pp.tile([D, 1], bf16, tag="ones128b")
            nc.vector.memset(ones128b, 1.0)
            ones_rb = pp.tile([1, D], bf16, tag="ones_rb")
            nc.vector.memset(ones_rb, 1.0)
            ones32b = pp.tile([1, HD], bf16, tag="ones32b")
            nc.vector.memset(ones32b, 1.0)
            ones512b = pp.tile([1, 512], bf16, tag="ones512b")
            nc.vector.memset(ones512b, 1.0)
            eps128 = pp.tile([D, 1], f32, tag="eps128")
            nc.vector.memset(eps128, EPS)
            eps1 = pp.tile([1, 1], f32, tag="eps1")
            nc.vector.memset(eps1, EPS)

            # ---- persistent activations (bf16) ----
            xT = big.tile([D, T], bf16, tag="chA")
            qT = big.tile([D, T], bf16, tag="chE")
            kTp = big.tile([D, bloc, H, NG, KTILE], bf16, tag="chB")
            v_sb = big.tile([D, T // KTILE, H, HD + 1], bf16, tag="chC")
            attnT = big.tile([D, T], bf16, tag="chD")
            x1T = big.tile([D, T], bf16, tag="chG")
            nc.vector.memset(v_sb[:, :, :, HD:HD + 1], 1.0)

            # ================= embedding =================
            psA_cm = tc.tile_pool(name="psA", bufs=6, space="PSUM")
            ps_sm = psA_cm.__enter__()
            for e in range(NET):
                sl = slice(e * 512, (e + 1) * 512)
                xb = wkt(NCAT)
                nc.scalar.dma_start(out=xb, in_=bc(Xf[sl], NCAT))
                sb_ = wkt(NCAT)
                nc.scalar.dma_start(out=sb_, in_=bc(saf[sl], NCAT))
                oh = wk.tile([NCAT, 512], bf16, tag="wk", name="oh")
                nc.vector.tensor_scalar(out=oh, in0=xb, scalar1=io25,
                                        scalar2=None, op0=OP.is_equal)
                nc.vector.tensor_mul(oh, oh, sb_)
                pb = wkt(10)
                nc.scalar.dma_start(out=pb, in_=bc(ptmf[sl], 10))
                ohp = wk.tile([10, 512], bf16, tag="wk", name="ohp")
                nc.vector.tensor_scalar(out=ohp, in0=pb, scalar1=io10,
                                        scalar2=None, op0=OP.is_equal)
                pe = ps_sm.tile([D, 512], f32, tag="sm")
                nc.tensor.matmul(pe, pemb_b, ohp, start=True, stop=False)
                nc.tensor.matmul(pe, emb_b, oh, start=False, stop=True)
                nc.vector.tensor_copy(xT[:, sl], pe)

            if dbg:
                nc.sync.dma_start(out=dbg_outs["dbg_xT"][:, :], in_=xT)

            # ================= qkv =================
            for e in range(NET):
                sl = slice(e * 512, (e + 1) * 512)
                pq = ps_sm.tile([D, 512], f32, tag="sm")
                nc.tensor.matmul(pq, inwT_b[:, 0:D], xT[:, sl],
                                 start=True, stop=True)
                nc.vector.tensor_scalar(out=qT[:, sl], in0=pq, scalar1=ISQ,
                                        scalar2=qb_s, op0=OP.mult, op1=OP.add)
                pk = ps_sm.tile([D, 512], f32, tag="sm")
                nc.tensor.matmul(pk, inwT_b[:, D:2 * D], xT[:, sl],
                                 start=True, stop=True)
                ktmp = mid.tile([D, 512], bf16, tag="mid", name="ktmp")
                nc.vector.tensor_scalar(out=ktmp, in0=pk, scalar1=kb_s,
                                        scalar2=None, op0=OP.add)
                b_ = (e * 512) // lp
                for h in range(H):
                    for sub in range(4):
                        ktb = ((e * 512) % lp) // KTILE + sub
                        _q = nc.gpsimd if (h + sub) % 2 == 0 else nc.sync
                        _q.dma_start(
                            out=kTp[32 * (ktb % 4):32 * (ktb % 4) + 32,
                                    b_, h, ktb // 4, :],
                            in_=ktmp[32 * h:32 * h + 32,
                                     sub * KTILE:(sub + 1) * KTILE])
                for sub in range(4):
                    tt = (e * 512) // KTILE + sub
                    pv = ps_sm.tile([KTILE, D], f32, tag="sm")
                    nc.tensor.matmul(pv, xT[:, e * 512 + sub * KTILE:
                                            e * 512 + (sub + 1) * KTILE],
                                     inwT_b[:, 2 * D:3 * D], start=True, stop=True)
                    nc.vector.tensor_copy(
                        v_sb[:, tt, :, 0:HD],
                        pv.rearrange("p (h d) -> p h d", h=H))

            if dbg:
                nc.sync.dma_start(out=dbg_outs["dbg_qT"][:, :], in_=qT)
                kT_dbg = big.tile([D, T], bf16, tag="chDBG")
                for b_ in range(bloc):
                    for h in range(H):
                        for g in range(NG):
                            for r in range(4):
                                kt = 4 * g + r
                                nc.sync.dma_start(
                                    out=kT_dbg[32 * h:32 * h + 32,
                                               b_ * lp + kt * KTILE:
                                               b_ * lp + (kt + 1) * KTILE],
                                    in_=kTp[32 * r:32 * r + 32, b_, h, g, :])
                nc.sync.dma_start(out=dbg_outs["dbg_kT"][:, :], in_=kT_dbg)

            psA_cm.__exit__(None, None, None)

            # ================= attention =================
            # 2 key-tiles per subgroup so sc fits 2 PSUM banks and can be
            # triple-buffered: QK of subgroup p+2 overlaps exp of p,
            # PV of p overlaps exp of p+1. Band bias applied post-exp as a
            # bf16 multiply on pt (off the QK->exp critical path).
            psB1_cm = tc.tile_pool(name="ps_sc", bufs=3, space="PSUM")
            ps_sc = psB1_cm.__enter__()
            psB2_cm = tc.tile_pool(name="ps_pv", bufs=2, space="PSUM")
            ps_pv = psB2_cm.__enter__()
            den32 = longs.tile([NDEN, 512], bf16, tag="den32")
            for b_ in range(bloc):
                for h in range(H):
                    qrep = qrp.tile([D, lp], bf16, tag="qr")
                    for r in range(4):
                        nc.gpsimd.dma_start(
                            out=qrep[32 * r:32 * r + 32, :],
                            in_=qT[32 * h:32 * h + 32, b_ * lp:(b_ + 1) * lp])
                    for qt in range(NQT):
                        ppv = ps_pv.tile([HD + 1, 512], f32, tag="pv")
                        for p_ in range(2 * NG):
                            sc = ps_sc.tile([D, 2 * 512], f32, tag="sc")
                            for j in range(2):
                                kt = 2 * p_ + j
                                nc.tensor.matmul(
                                    sc[:, j * 512:(j + 1) * 512],
                                    kTp[32 * (kt % 4):32 * (kt % 4) + 32,
                                        b_, h, kt // 4, :],
                                    qrep[32 * (kt % 4):32 * (kt % 4) + 32,
                                         qt * QT:(qt + 1) * QT],
                                    start=True, stop=True,
                                    tile_position=(32 * (kt % 4), 0))
                            side = 0 if (p_ >> 1) <= qt else 1
                            pt = ptp.tile([D, 2 * 512], bf16, tag="pt")
                            nc.scalar.activation(pt, sc, AF.Exp,
                                                 bias=cexp[:, 2 * h + side:
                                                           2 * h + side + 1],
                                                 scale=1.0)
                            for j in range(2):
                                kt = 2 * p_ + j
                                dd = kt - 4 * qt
                                if -1 <= dd <= 4:
                                    di = dd + 1
                                    c0, w = BAND_C0[di], BAND_W[di]
                                    nc.vector.tensor_tensor(
                                        out=pt[:, j * 512 + c0:j * 512 + c0 + w],
                                        in0=pt[:, j * 512 + c0:j * 512 + c0 + w],
                                        in1=bandexp[:, h * BAND_TOT + BAND_OFF[di]:
                                                    h * BAND_TOT + BAND_OFF[di] + w],
                                        op=OP.mult)
                            for j in range(2):
                                kt = 2 * p_ + j
                                nc.tensor.matmul(
                                    ppv, v_sb[:, b_ * NKT + kt, h, :],
                                    pt[:, j * 512:(j + 1) * 512],
                                    start=(kt == 0), stop=(kt == NKT - 1))
                        pv_sb = wk.tile([HD + 1, 512], bf16, tag="wk", name="pv_sb")
                        nc.vector.tensor_copy(pv_sb, ppv)
                        nc.gpsimd.dma_start(
                            out=attnT[32 * h:32 * h + 32,
                                      b_ * lp + qt * QT:b_ * lp + (qt + 1) * QT],
                            in_=pv_sb[0:HD, :])
                        nc.gpsimd.dma_start(
                            out=den32[(b_ * NQT + qt) * H + h:
                                      (b_ * NQT + qt) * H + h + 1, :],
                            in_=pv_sb[HD:HD + 1, :])

            psB2_cm.__exit__(None, None, None)
            psB1_cm.__exit__(None, None, None)
            psC_cm = tc.tile_pool(name="psC", bufs=6, space="PSUM")
            ps_sm = psC_cm.__enter__()

            sync_in = dram.tile([1, 1], f32, tag="synci")
            sync_out = dram.tile([1, 1], f32, tag="synco")
            syncs = wk.tile([1, 1], f32, tag="wk", name="syncs")
            nc.vector.memset(syncs, 0.0)
            nc.gpsimd.dma_start(out=sync_in, in_=syncs)
            nc.gpsimd.collective_compute(
                "AllReduce", OP.add, replica_groups=[list(range(n_cores))],
                ins=[sync_in[:, :].opt()], outs=[sync_out[:, :].opt()])

            lnden = midt(NDEN, 512)
            nc.scalar.activation(lnden, den32, AF.Ln, bias=0.0, scale=1.0)
            recip = longs.tile([NDEN, 512], bf16, tag="recip")
            nc.scalar.activation(recip, lnden, AF.Exp, bias=0.0, scale=-1.0)

            for b_ in range(bloc):
                for qt in range(NQT):
                    bcp = ps_sm.tile([D, 512], f32, tag="sm")
                    for h in range(H):
                        rr = rowp.tile([1, 512], bf16, tag="row")
                        nc.gpsimd.dma_start(
                            out=rr, in_=recip[(b_ * NQT + qt) * H + h:
                                              (b_ * NQT + qt) * H + h + 1, :])
                        nc.tensor.matmul(bcp[32 * h:32 * h + 32, :], ones32b, rr,
                                         start=True, stop=True,
                                         tile_position=(0, 32 * h))
                    sl = slice(b_ * lp + qt * QT, b_ * lp + (qt + 1) * QT)
                    nc.vector.tensor_mul(attnT[:, sl], attnT[:, sl], bcp)

            if dbg:
                nc.sync.dma_start(out=dbg_outs["dbg_attnT"][:, :], in_=attnT)

            # ======== layernorm helper: per-tile stats packed into wide
            # [1, NLT*512] rows -> one Ln + one Exp per call (one table
            # switch each); gamma folded into the rank-1 broadcast
            # (lhsT = g row), -beta accumulated via a ones-row matmul ========
            def layernorm_T(src, dst_fn, g_row, nb_row, b_base):
                m_all = lnr.tile([1, NLT * 512], bf16, tag="mall", name="m_all")
                v_all = lnr.tile([1, NLT * 512], f32, tag="vall", name="v_all")
                for t_ in range(NLT):
                    sl = slice(b_base * lp + t_ * 512, b_base * lp + (t_ + 1) * 512)
                    rsl = slice(t_ * 512, (t_ + 1) * 512)
                    sq = wk.tile([D, 512], bf16, tag="wk", name="sq")
                    nc.vector.tensor_mul(sq, src[:, sl], src[:, sl])
                    p1_ = ps_sm.tile([1, 512], f32, tag="sm")
                    nc.tensor.matmul(p1_, ones128b, src[:, sl], start=True, stop=True)
                    p2_ = ps_sm.tile([1, 512], f32, tag="sm")
                    nc.tensor.matmul(p2_, ones128b, sq, start=True, stop=True)
                    nc.vector.tensor_scalar(out=m_all[:, rsl], in0=p1_,
                                            scalar1=1.0 / D,
                                            scalar2=None, op0=OP.mult)
                    nc.vector.tensor_scalar(out=v_all[:, rsl], in0=p2_,
                                            scalar1=1.0 / D,
                                            scalar2=None, op0=OP.mult)
                    msq = rowp.tile([1, 512], f32, tag="row", name="msq")
                    nc.vector.tensor_mul(msq, m_all[:, rsl], m_all[:, rsl])
                    nc.vector.tensor_tensor(out=v_all[:, rsl], in0=v_all[:, rsl],
                                            in1=msq, op=OP.subtract)
                lnv = lnr.tile([1, NLT * 512], f32, tag="lall", name="lnv")
                nc.scalar.activation(lnv, v_all, AF.Ln, bias=eps1, scale=1.0)
                rr_all = lnr.tile([1, NLT * 512], bf16, tag="rall", name="rr_all")
                nc.scalar.activation(rr_all, lnv, AF.Exp, bias=0.0, scale=-0.5)
                rm_all = lnr.tile([1, NLT * 512], bf16, tag="rmall", name="rm_all")
                nc.vector.tensor_mul(rm_all, m_all, rr_all)
                for t_ in range(NLT):
                    sl = slice(b_base * lp + t_ * 512, b_base * lp + (t_ + 1) * 512)
                    rsl = slice(t_ * 512, (t_ + 1) * 512)
                    br = ps_sm.tile([D, 512], f32, tag="sm")
                    nc.tensor.matmul(br, g_row, rr_all[:, rsl], start=True, stop=True)
                    bm = ps_sm.tile([D, 512], f32, tag="sm")
                    nc.tensor.matmul(bm, g_row, rm_all[:, rsl], start=True, stop=False)
                    nc.tensor.matmul(bm, nb_row, ones512b, start=False, stop=True)
                    tmp = wk.tile([D, 512], f32, tag="wk", name="lntmp")
                    nc.vector.tensor_mul(tmp, src[:, sl], br)
                    nc.vector.tensor_tensor(out=dst_fn(t_), in0=tmp, in1=bm,
                                            op=OP.subtract)

            # ================= out-proj + residual + LN1 =================
            r1T = big.tile([D, T], bf16, tag="chB")   # after kTp's last read
            for b_ in range(bloc):
                for qt in range(NQT):
                    sl = slice(b_ * lp + qt * QT, b_ * lp + (qt + 1) * QT)
                    po = ps_sm.tile([D, 512], f32, tag="sm")
                    nc.tensor.matmul(po, woT_b, attnT[:, sl], start=True, stop=False)
                    nc.tensor.matmul(po, ob_r, ones512b, start=False, stop=True)
                    nc.vector.tensor_tensor(out=r1T[:, sl], in0=po,
                                            in1=xT[:, sl], op=OP.add)
            for b_ in range(bloc):
                layernorm_T(
                    r1T,
                    lambda t_, b0=b_: x1T[:, b0 * lp + t_ * 512:
                                          b0 * lp + (t_ + 1) * 512],
                    ln1g_r, ln1nb_r, b_)

            if dbg:
                nc.sync.dma_start(out=dbg_outs["dbg_x1T"][:, :], in_=x1T)

            # ================= FFN + residual + LN2 =================
            x2pad = big.tile([D, bloc * (lp + 4)], bf16, tag="chA")  # after xT
            nc.vector.memset(x2pad[:, :], 0.0)
            r2T = big.tile([D, T], bf16, tag="chC")                  # after v_sb
            for b_ in range(bloc):
                for qt in range(NQT):
                    sl = slice(b_ * lp + qt * QT, b_ * lp + (qt + 1) * QT)
                    h1a = wk.tile([D, 512], bf16, tag="wk", name="h1a")
                    h1b = wk.tile([D, 512], bf16, tag="wk", name="h1b")
                    for half, dest in ((0, h1a), (1, h1b)):
                        ph = ps_sm.tile([D, 512], f32, tag="sm")
                        nc.tensor.matmul(ph, l1wT_b[:, half * D:(half + 1) * D],
                                         x1T[:, sl], start=True, stop=True)
                        nc.scalar.activation(dest, ph, AF.Relu,
                                             bias=l1b_s[:, half:half + 1],
                                             scale=1.0)
                    py = ps_sm.tile([D, 512], f32, tag="sm")
                    nc.tensor.matmul(py, l2cat_b[:, 0:D], h1a, start=True, stop=False)
                    nc.tensor.matmul(py, l2cat_b[:, D:2 * D], h1b,
                                     start=False, stop=False)
                    nc.tensor.matmul(py, l2b_r, ones512b, start=False, stop=True)
                    nc.vector.tensor_tensor(out=r2T[:, sl], in0=py,
                                            in1=x1T[:, sl], op=OP.add)
            for b_ in range(bloc):
                layernorm_T(
                    r2T,
                    lambda t_, b0=b_: x2pad[:, b0 * (lp + 4) + 2 + t_ * 512:
                                            b0 * (lp + 4) + 2 + (t_ + 1) * 512],
                    ln2g_r, ln2nb_r, b_)

            if dbg:
                for b_ in range(bloc):
                    nc.sync.dma_start(
                        out=dbg_outs["dbg_x2"][:, b_ * lp:(b_ + 1) * lp],
                        in_=x2pad[:, b_ * (lp + 4) + 2:b_ * (lp + 4) + 2 + lp])

            # ================= conv1 + bn1 =================
            c1_sb = big.tile([C1, bloc * lp], bf16, tag="chD")   # after attnT
            bnst1 = longs.tile([C1, bloc * NLT, 6], f32, tag="bnst1")
            for b_ in range(bloc):
                for t_ in range(NLT):
                    pc = ps_sm.tile([C1, 512], f32, tag="sm")
                    for k_ in range(K):
                        nc.tensor.matmul(
                            pc, c1wT_b[:, k_ * C1:(k_ + 1) * C1],
                            x2pad[:, b_ * (lp + 4) + t_ * 512 + k_:
                                  b_ * (lp + 4) + t_ * 512 + k_ + 512],
                            start=(k_ == 0), stop=(k_ == K - 1))
                    nc.vector.bn_stats(out=bnst1[:, b_ * NLT + t_, :], in_=pc)
                    nc.vector.tensor_copy(
                        c1_sb[:, b_ * lp + t_ * 512:b_ * lp + (t_ + 1) * 512], pc)
            mv1 = wk.tile([C1, 2], f32, tag="wk")
            nc.vector.bn_aggr(out=mv1, in_=bnst1)
            part1 = wk.tile([C1, 2], f32, tag="wk")
            sqm = wk.tile([C1, 1], f32, tag="wk")
            nc.vector.tensor_mul(sqm, mv1[:, 0:1], mv1[:, 0:1])
            nc.vector.tensor_tensor(out=sqm, in0=sqm, in1=mv1[:, 1:2], op=OP.add)
            nl_ = float(bloc * lp)
            nc.vector.tensor_scalar(out=part1[:, 0:1], in0=mv1[:, 0:1],
                                    scalar1=nl_, scalar2=None, op0=OP.mult)
            nc.vector.tensor_scalar(out=part1[:, 1:2], in0=sqm,
                                    scalar1=nl_, scalar2=None, op0=OP.mult)
            bn1_in = dram.tile([C1, 2], f32, tag="bn1i")
            bn1_out = dram.tile([C1, 2], f32, tag="bn1o")
            nc.sync.dma_start(out=bn1_in, in_=part1)
            nc.gpsimd.collective_compute(
                "AllReduce", OP.add, replica_groups=[list(range(n_cores))],
                ins=[bn1_in[:, :].opt()], outs=[bn1_out[:, :].opt()])
            glob1 = wk.tile([C1, 2], f32, tag="wk")
            nc.sync.dma_start(out=glob1, in_=bn1_out)

            def bn_scale_shift(globc, n_, g_ap, b_ap):
                mean = wk.tile([C1, 1], f32, tag="wk")
                nc.vector.tensor_scalar(out=mean, in0=globc[:, 0:1],
                                        scalar1=1.0 / n_, scalar2=None, op0=OP.mult)
                ex2 = wk.tile([C1, 1], f32, tag="wk")
                nc.vector.tensor_scalar(out=ex2, in0=globc[:, 1:2],
                                        scalar1=1.0 / n_, scalar2=None, op0=OP.mult)
                msq_ = wk.tile([C1, 1], f32, tag="wk")
                nc.vector.tensor_mul(msq_, mean, mean)
                nc.vector.tensor_tensor(out=ex2, in0=ex2, in1=msq_, op=OP.subtract)
                lnv_ = wk.tile([C1, 1], f32, tag="wk")
                nc.scalar.activation(lnv_, ex2, AF.Ln, bias=eps128, scale=1.0)
                rstd_ = wk.tile([C1, 1], f32, tag="wk")
                nc.scalar.activation(rstd_, lnv_, AF.Exp, bias=0.0, scale=-0.5)
                scale = longs.tile([C1, 1], f32, tag="bnsc")
                nc.vector.tensor_mul(scale, rstd_, g_ap)
                shift = longs.tile([C1, 1], f32, tag="bnsh")
                nc.vector.tensor_mul(shift, mean, scale)
                nc.vector.tensor_tensor(out=shift, in0=b_ap, in1=shift,
                                        op=OP.subtract)
                return scale, shift

            sc1, sh1 = bn_scale_shift(glob1, n1, bn1g, bn1b)
            p1_sb = big.tile([C1, bloc * LP2], bf16, tag="chF")   # after band
            if dbg:
                nc.sync.dma_start(out=dbg_outs["dbg_c1"][:, :], in_=c1_sb)
            for b_ in range(bloc):
                for t_ in range(NLT):
                    rel = wk.tile([D, 512], bf16, tag="wk", name="rel")
                    nc.scalar.activation(
                        rel, c1_sb[:, b_ * lp + t_ * 512:b_ * lp + (t_ + 1) * 512],
                        AF.Relu, bias=sh1, scale=sc1)
                    rel2 = rel.rearrange("p (l two) -> p l two", two=2)
                    nc.vector.tensor_tensor(
                        out=p1_sb[:, b_ * LP2 + t_ * 256:b_ * LP2 + (t_ + 1) * 256],
                        in0=rel2[:, :, 0], in1=rel2[:, :, 1], op=OP.max)
            if dbg:
                nc.sync.dma_start(out=dbg_outs["dbg_p1"][:, :], in_=p1_sb)

            # ================= conv2 + bn2 =================
            c2_sb = big.tile([C1, bloc * 2 * L2], bf16, tag="chE")  # after qT
            bnst2 = longs.tile([C1, 2, bloc * 2, 6], f32, tag="bnst2")
            for b_ in range(bloc):
                for half in range(2):
                    for t_ in range(2):
                        pc = ps_sm.tile([C1, LT2], f32, tag="sm")
                        for k_ in range(K):
                            nc.tensor.matmul(
                                pc, c2wT_b[:, k_ * C2 + half * C1:
                                           k_ * C2 + (half + 1) * C1],
                                p1_sb[:, b_ * LP2 + t_ * LT2 + k_:
                                      b_ * LP2 + t_ * LT2 + k_ + LT2],
                                start=(k_ == 0), stop=(k_ == K - 1))
                        nc.vector.bn_stats(out=bnst2[:, half, b_ * 2 + t_, :],
                                           in_=pc)
                        nc.vector.tensor_copy(
                            c2_sb[:, (b_ * 2 + half) * L2 + t_ * LT2:
                                  (b_ * 2 + half) * L2 + (t_ + 1) * LT2], pc)
            part2 = longs.tile([C1, 4], f32, tag="part2")
            for half in range(2):
                mv2 = wk.tile([C1, 2], f32, tag="wk")
                nc.vector.bn_aggr(out=mv2, in_=bnst2[:, half, :, :])
                sqm2 = wk.tile([C1, 1], f32, tag="wk")
                nc.vector.tensor_mul(sqm2, mv2[:, 0:1], mv2[:, 0:1])
                nc.vector.tensor_tensor(out=sqm2, in0=sqm2, in1=mv2[:, 1:2],
                                        op=OP.add)
                nl2 = float(bloc * L2)
                nc.vector.tensor_scalar(out=part2[:, 2 * half:2 * half + 1],
                                        in0=mv2[:, 0:1], scalar1=nl2,
                                        scalar2=None, op0=OP.mult)
                nc.vector.tensor_scalar(out=part2[:, 2 * half + 1:2 * half + 2],
                                        in0=sqm2, scalar1=nl2,
                                        scalar2=None, op0=OP.mult)
            bn2_in = dram.tile([C1, 4], f32, tag="bn2i")
            bn2_out = dram.tile([C1, 4], f32, tag="bn2o")
            nc.sync.dma_start(out=bn2_in, in_=part2)
            nc.gpsimd.collective_compute(
                "AllReduce", OP.add, replica_groups=[list(range(n_cores))],
                ins=[bn2_in[:, :].opt()], outs=[bn2_out[:, :].opt()])
            glob2 = longs.tile([C1, 4], f32, tag="glob2")
            nc.sync.dma_start(out=glob2, in_=bn2_out)

            feat = longs.tile([C1, 2 * bloc], f32, tag="feat")
            # vectorized bn2 scale/shift for both halves: [C1, 2]
            g2v = glob2.rearrange("p (h two) -> p h two", two=2)
            mean2 = wk.tile([C1, 2], f32, tag="wk")
            nc.vector.tensor_scalar(out=mean2, in0=g2v[:, :, 0], scalar1=1.0 / n2,
                                    scalar2=None, op0=OP.mult)
            ex22 = wk.tile([C1, 2], f32, tag="wk")
            nc.vector.tensor_scalar(out=ex22, in0=g2v[:, :, 1], scalar1=1.0 / n2,
                                    scalar2=None, op0=OP.mult)
            msq2 = wk.tile([C1, 2], f32, tag="wk")
            nc.vector.tensor_mul(msq2, mean2, mean2)
            nc.vector.tensor_tensor(out=ex22, in0=ex22, in1=msq2, op=OP.subtract)
            lnv2 = wk.tile([C1, 2], f32, tag="wk")
            nc.scalar.activation(lnv2, ex22, AF.Ln, bias=eps128, scale=1.0)
            rstd2 = wk.tile([C1, 2], f32, tag="wk")
            nc.scalar.activation(rstd2, lnv2, AF.Exp, bias=0.0, scale=-0.5)
            scale2 = longs.tile([C1, 2], f32, tag="bnsc2")
            nc.vector.tensor_mul(scale2, rstd2, bn2g)
            shift2 = longs.tile([C1, 2], f32, tag="bnsh2")
            nc.vector.tensor_mul(shift2, mean2, scale2)
            nc.vector.tensor_tensor(out=shift2, in0=bn2b, in1=shift2,
                                    op=OP.subtract)
            for half in range(2):
                sc2 = scale2[:, half:half + 1]
                sh2 = shift2[:, half:half + 1]
                for b_ in range(bloc):
                    rel = wkb.tile([C1, L2], bf16, tag="wkb")
                    nc.scalar.activation(
                        rel, c2_sb[:, (b_ * 2 + half) * L2:
                                   (b_ * 2 + half + 1) * L2],
                        AF.Relu, bias=sh2, scale=sc2)
                    nc.vector.reduce_max(
                        out=feat[:, (b_ * 2 + half):(b_ * 2 + half) + 1],
                        in_=rel, axis=AX.X)
            if dbg:
                nc.sync.dma_start(out=dbg_outs["dbg_feat"][:, :], in_=feat)

            # ================= fc =================
            for b_ in range(bloc):
                pf = ps_sm.tile([NC, 1], f32, tag="sm")
                for half in range(2):
                    nc.tensor.matmul(pf, fcwT[:, half * NC:(half + 1) * NC],
                                     feat[:, b_ * 2 + half:b_ * 2 + half + 1],
                                     start=(half == 0), stop=(half == 1))
                ob2 = wk.tile([NC, 1], f32, tag="wk")
                nc.vector.tensor_scalar(out=ob2, in0=pf, scalar1=fcb_s,
                                        scalar2=None, op0=OP.add)
                nc.sync.dma_start(out=out_d[b_, :], in_=ob2[:, 0])
            psC_cm.__exit__(None, None, None)

    nc.compile()
    return nc


def _host_inputs(inputs, n_cores, bloc, lp):
    X = np.asarray(inputs["X"]).astype(np.float32)[:, :lp]
    sa = np.asarray(inputs["surface_availability"], dtype=np.float32)[:, :lp]
    ptm = np.asarray(inputs["ptm"]).astype(np.float32)[:, :lp]
    emb = np.asarray(inputs["emb"], dtype=np.float32)
    pemb = np.asarray(inputs["ptm_emb"], dtype=np.float32)
    rpe = np.asarray(inputs["rpe"], dtype=np.float32)
    inw = np.asarray(inputs["in_proj_w"], dtype=np.float32)
    inb = np.asarray(inputs["in_proj_b"], dtype=np.float32)
    wo = np.asarray(inputs["out_proj_w"], dtype=np.float32)
    bo = np.asarray(inputs["out_proj_b"], dtype=np.float32)
    w1 = np.asarray(inputs["lin1_w"], dtype=np.float32)
    b1 = np.asarray(inputs["lin1_b"], dtype=np.float32)
    w2 = np.asarray(inputs["lin2_w"], dtype=np.float32)
    b2 = np.asarray(inputs["lin2_b"], dtype=np.float32)
    c1w = np.asarray(inputs["conv1_w"], dtype=np.float32)
    c2w = np.asarray(inputs["conv2_w"], dtype=np.float32)
    fcw = np.asarray(inputs["fc_w"], dtype=np.float32)

    pembp = np.zeros((10, D), np.float32)
    pembp[:, ED:] = pemb
    embp = np.zeros((NCAT, D), np.float32)
    embp[:, :ED] = emb

    clo, chi = rpe[0], rpe[2 * MD]
    bandcat = np.zeros((D, H * BAND_TOT), np.float32)
    jj = np.arange(128)[:, None]
    for h in range(H):
        for di, dl in enumerate(BAND_DELTAS):
            w = BAND_W[di]
            ii = np.arange(BAND_C0[di], BAND_C0[di] + w)[None, :]
            e = dl + jj - ii
            val = rpe[np.clip(e, -MD, MD) + MD, h]
            beta = chi[h] if di == 5 else clo[h]
            bandcat[:, h * BAND_TOT + BAND_OFF[di]:
                    h * BAND_TOT + BAND_OFF[di] + w] = val - beta
    cexp = np.zeros((D, 2 * H), np.float32)
    for h in range(H):
        cexp[:, 2 * h] = clo[h]
        cexp[:, 2 * h + 1] = chi[h]

    ob_eff = bo + wo @ inb[2 * D:3 * D]
    l2t = w2.T   # [FF, D]
    shared = {
        "embp": embp, "pembp": pembp,
        "iota25": np.arange(NCAT, dtype=np.float32)[:, None],
        "iota10": np.arange(10, dtype=np.float32)[:, None],
        "inwT": np.ascontiguousarray(inw.T),
        "qb": (inb[0:D] * ISQ)[:, None],
        "kb": inb[D:2 * D][:, None],
        "woT": np.ascontiguousarray(wo.T),
        "ob_eff": ob_eff[:, None].astype(np.float32),
        "l1wT": np.ascontiguousarray(w1.T),
        "l1b": b1.reshape(2, D).T.copy(),
        "l2cat": np.concatenate([l2t[0:D], l2t[D:2 * D]], axis=1).copy(),
        "l2b": b2[:, None],
        "ln1g": np.asarray(inputs["ln1_g"], np.float32)[:, None],
        "ln1b": np.asarray(inputs["ln1_b"], np.float32)[:, None],
        "ln2g": np.asarray(inputs["ln2_g"], np.float32)[:, None],
        "ln2b": np.asarray(inputs["ln2_b"], np.float32)[:, None],
        "bandcat": bandcat, "cexp": cexp,
        "c1wT": np.ascontiguousarray(c1w.transpose(1, 2, 0).reshape(D, K * C1)),
        "c2wT": np.ascontiguousarray(c2w.transpose(1, 2, 0).reshape(C1, K * C2)),
        "bn1g": np.asarray(inputs["bn1_g"], np.float32)[:, None],
        "bn1b": np.asarray(inputs["bn1_b"], np.float32)[:, None],
        "bn2g": np.asarray(inputs["bn2_g"], np.float32).reshape(2, C1).T.copy(),
        "bn2b": np.asarray(inputs["bn2_b"], np.float32).reshape(2, C1).T.copy(),
        "fcwT": np.ascontiguousarray(
            fcw.T.reshape(2, C1, NC).transpose(1, 0, 2).reshape(C1, 2 * NC)),
        "fcb": np.asarray(inputs["fc_b"], np.float32)[:, None],
    }
    in_maps = []
    for c in range(n_cores):
        rows = slice(c * bloc, (c + 1) * bloc)
        m = dict(shared)
        m["Xf"] = np.ascontiguousarray(X[rows].reshape(-1))
        m["saf"] = np.ascontiguousarray(sa[rows].reshape(-1))
        m["ptmf"] = np.ascontiguousarray(ptm[rows].reshape(-1))
        in_maps.append(m)
    return in_maps


_NC_CACHE = {}


def _get_nc(n_cores, bloc, lp, dbg=False):
    key = (n_cores, bloc, lp, dbg)
    if key not in _NC_CACHE:
        _NC_CACHE[key] = _build(n_cores, bloc, lp, dbg=dbg)
    return _NC_CACHE[key]


def kernel(**inputs):
    from concourse.bass_utils import run_bass_kernel_spmd
    nc = _get_nc(NCORES, BLOC, L)
    in_maps = _host_inputs(inputs, NCORES, BLOC, L)
    res = run_bass_kernel_spmd(nc, in_maps, list(range(NCORES)))
    out = np.concatenate([res.results[i]["out"] for i in range(NCORES)], axis=0)
    return out.astype(np.float32)


# revision 27
# speedup vs baseline: 2.3549x; 1.0214x over previous
"""Trainium2 Bass kernel for nn_CNN2LWithRPE (transformer layer + CNN head).

Sharding: data-parallel over batch across 8 NeuronCores (2 batch rows each).
All parameters replicated. The only cross-core communication is two tiny
AllReduces for the training-mode BatchNorm statistics.

Per-core layout (B_loc batches, T = B_loc*L tokens):
  - activations transposed in SBUF as bf16: xT/qT/x1T... are [D=128, T]
    (bf16 operands run the PE at 1 cycle/row vs 4 for fp32; PSUM
    accumulation stays fp32)
  - attention as scores^T tiles [keys=128 part, queries=512 free]:
    QK^T row-packed 4x on PE (K=32) from a shuffled kTp layout + a 4x
    replicated qrep tile; exp on ACT as wide [128,2048] ops with the
    clipped-RPE bias folded into a per-group bias constant (c_lo/c_hi)
    plus narrow host-precomputed banded correction tiles added on DVE;
    PV uses v in natural layout with an appended ones-column so the
    softmax denominator falls out of the same matmul.
  - layernorm in transposed layout: partition stats via ones-matmul,
    rstd = Exp(-0.5*Ln(var+eps)), rank-1 K=1 matmul broadcast back.
  - conv1d as K accumulating shifted matmuls; BN apply fused into
    ACT Relu(scale*x+shift) with per-channel scale/shift APs.
"""

import numpy as np

B, L = 16, 2048
NCAT, ED = 25, 120
D, H, HD = 128, 4, 32
FF = 256
MD = 32
C1, C2, K = 128, 256, 5
NC = 2
EPS = 1e-5
NCORES = 8
BLOC = B // NCORES
ISQ = float(1.0 / np.sqrt(HD))

QT = 512
KTILE = 128

BAND_DELTAS = [-128, 0, 128, 256, 384, 512]
BAND_W = [32, 160, 288, 416, 512, 32]
BAND_C0 = [0, 0, 0, 0, 0, 480]
BAND_OFF = [0, 32, 192, 480, 896, 1408]
BAND_TOT = 1440


def _build(n_cores, bloc, lp, dbg=False):
    import contextlib
    import concourse.bass as bass
    import concourse.tile as tile
    from concourse import bacc, mybir

    f32 = mybir.dt.float32
    bf16 = mybir.dt.bfloat16
    AF = mybir.ActivationFunctionType
    OP = mybir.AluOpType
    AX = mybir.AxisListType

    T = bloc * lp
    NET = T // 512
    NQT = lp // QT
    NKT = lp // KTILE
    NG = NKT // 4
    NLT = lp // 512
    NDEN = bloc * NQT * H
    LP2 = lp // 2
    L2 = LP2 - (K - 1)
    LT2 = L2 // 2
    n1 = float(n_cores * bloc * lp)
    n2 = float(n_cores * bloc * L2)

    nc = bacc.Bacc("TRN2", target_bir_lowering=False, debug=False,
                   num_devices=n_cores)

    def din(name, shape):
        return nc.dram_tensor(name, list(shape), f32, kind="ExternalInput")

    Xf = din("Xf", [T])
    saf = din("saf", [T])
    ptmf = din("ptmf", [T])
    emb_d = din("embp", [NCAT, D])
    pemb_d = din("pembp", [10, D])
    iota25_d = din("iota25", [NCAT, 1])
    iota10_d = din("iota10", [10, 1])
    inwT_d = din("inwT", [D, 3 * D])
    qb_d = din("qb", [D, 1])
    kb_d = din("kb", [D, 1])
    woT_d = din("woT", [D, D])
    ob_d = din("ob_eff", [D, 1])
    l1wT_d = din("l1wT", [D, FF])
    l1b_d = din("l1b", [D, 2])
    l2cat_d = din("l2cat", [D, 2 * D])
    l2b_d = din("l2b", [D, 1])
    ln1g_d = din("ln1g", [D, 1])
    ln1b_d = din("ln1b", [D, 1])
    ln2g_d = din("ln2g", [D, 1])
    ln2b_d = din("ln2b", [D, 1])
    band_d = din("bandcat", [D, H * BAND_TOT])
    cexp_d = din("cexp", [D, 2 * H])
    c1wT_d = din("c1wT", [D, K * C1])
    c2wT_d = din("c2wT", [C1, K * C2])
    bn1g_d = din("bn1g", [C1, 1])
    bn1b_d = din("bn1b", [C1, 1])
    bn2g_d = din("bn2g", [C1, 2])
    bn2b_d = din("bn2b", [C1, 2])
    fcwT_d = din("fcwT", [C1, 2 * NC])
    fcb_d = din("fcb", [NC, 1])

    out_d = nc.dram_tensor("out", [bloc, NC], f32, kind="ExternalOutput")
    dbg_outs = {}
    if dbg:
        for nm, shp, dt_ in [("dbg_xT", [D, T], bf16),
                             ("dbg_qT", [D, T], bf16),
                             ("dbg_kT", [D, T], bf16),
                             ("dbg_attnT", [D, T], bf16),
                             ("dbg_x1T", [D, T], bf16),
                             ("dbg_x2", [D, T], bf16),
                             ("dbg_c1", [C1, bloc * lp], bf16),
                             ("dbg_p1", [C1, bloc * LP2], bf16),
                             ("dbg_feat", [C1, 2 * bloc], f32)]:
            dbg_outs[nm] = nc.dram_tensor(nm, shp, dt_, kind="ExternalOutput")

    def bc(ap1d, parts):
        return bass.AP(tensor=ap1d.tensor, offset=ap1d.offset,
                       ap=[[0, parts]] + [list(p) for p in ap1d.ap])

    with tile.TileContext(nc) as tc:
        ctx = contextlib.ExitStack()
        with ctx:
            pp = ctx.enter_context(tc.tile_pool(name="params", bufs=1))
            big = ctx.enter_context(tc.tile_pool(name="big", bufs=1))
            wk = ctx.enter_context(tc.tile_pool(name="wk", bufs=12))
            wkb = ctx.enter_context(tc.tile_pool(name="wkb", bufs=2))
            longs = ctx.enter_context(tc.tile_pool(name="longs", bufs=1))
            mid = ctx.enter_context(tc.tile_pool(name="mid", bufs=6))
            rowp = ctx.enter_context(tc.tile_pool(name="rows", bufs=6))
            lnr = ctx.enter_context(tc.tile_pool(name="lnrows", bufs=1))
            ptp = ctx.enter_context(tc.tile_pool(name="pt", bufs=4))
            qrp = ctx.enter_context(tc.tile_pool(name="qrep", bufs=2))
            dram = ctx.enter_context(tc.tile_pool(name="dram", bufs=1, space="DRAM"))

            def wkt(p=D, f=512):
                return wk.tile([p, f], f32, tag="wk", name="wkt")

            def midt(p, f):
                return mid.tile([p, f], f32, tag="mid", name="midt")

            # ---- params ----
            _ldq = [0]

            def ld(dten, shape):
                t = pp.tile(list(shape), f32, tag=dten.name, name=dten.name + "_s")
                q = nc.sync if _ldq[0] % 2 == 0 else nc.gpsimd
                _ldq[0] += 1
                q.dma_start(out=t, in_=dten[tuple(slice(0, s) for s in shape)])
                return t

            def ldb(dten, shape):
                # load f32 param, convert to a bf16 copy for PE consumption
                t = ld(dten, shape)
                tb = pp.tile(list(shape), bf16, tag=dten.name + "_b",
                             name=dten.name + "_b")
                nc.vector.tensor_copy(tb, t)
                return tb

            emb_b = ldb(emb_d, [NCAT, D])
            pemb_b = ldb(pemb_d, [10, D])
            io25 = ld(iota25_d, [NCAT, 1])
            io10 = ld(iota10_d, [10, 1])
            inwT_b = ldb(inwT_d, [D, 3 * D])
            qb_s = ld(qb_d, [D, 1])
            kb_s = ld(kb_d, [D, 1])
            woT_b = ldb(woT_d, [D, D])
            ob_s = ld(ob_d, [D, 1])
            l1wT_b = ldb(l1wT_d, [D, FF])
            l1b_s = ld(l1b_d, [D, 2])
            l2cat_b = ldb(l2cat_d, [D, 2 * D])
            l2b_s = ld(l2b_d, [D, 1])
            ln1g = ld(ln1g_d, [D, 1]); ln1b = ld(ln1b_d, [D, 1])
            ln2g = ld(ln2g_d, [D, 1]); ln2b = ld(ln2b_d, [D, 1])
            # transposed bf16 rows of LN gamma / negated beta for rank-1
            # broadcast matmuls: br = g^T x rstd_row, bm += (-b)^T x ones_row
            def trow(col_tile, name, negate=False):
                rf = pp.tile([1, D], f32, tag=name + "f", name=name + "f")
                nc.sync.dma_start(out=rf, in_=col_tile[:, 0])
                rb = pp.tile([1, D], bf16, tag=name, name=name)
                nc.vector.tensor_scalar(out=rb, in0=rf,
                                        scalar1=-1.0 if negate else 1.0,
                                        scalar2=None, op0=OP.mult)
                return rb
            ob_r = trow(ob_s, "ob_r")
            l2b_r = trow(l2b_s, "l2b_r")
            ln1g_r = trow(ln1g, "ln1g_r")
            ln1nb_r = trow(ln1b, "ln1nb_r", negate=True)
            ln2g_r = trow(ln2g, "ln2g_r")
            ln2nb_r = trow(ln2b, "ln2nb_r", negate=True)
            cexp = ld(cexp_d, [D, 2 * H])
            cexp2 = pp.tile([D, 2 * H], f32, tag="cexp2")
            nc.vector.tensor_scalar(out=cexp2, in0=cexp, scalar1=0.5,
                                    scalar2=1.0, op0=OP.mult, op1=OP.add)
            c1wT_b = ldb(c1wT_d, [D, K * C1])
            c2wT_b = ldb(c2wT_d, [C1, K * C2])
            bn1g = ld(bn1g_d, [C1, 1]); bn1b = ld(bn1b_d, [C1, 1])
            bn2g = ld(bn2g_d, [C1, 2]); bn2b = ld(bn2b_d, [C1, 2])
            fcwT = ld(fcwT_d, [C1, 2 * NC])
            fcb_s = ld(fcb_d, [NC, 1])

            band_f = big.tile([D, H * BAND_TOT], f32, tag="chF0")
            nc.sync.dma_start(out=band_f, in_=band_d[:, :])
            bandexp = big.tile([D, H * BAND_TOT], bf16, tag="chF")
            nc.scalar.activation(bandexp, band_f, AF.Exp, bias=0.0, scale=1.0)

            ones128b = pp.tile([D, 1], bf16, tag="ones128b")
            nc.vector.memset(ones128b, 1.0)
            ones_rb = pp.tile([1, D], bf16, tag="ones_rb")
            nc.vector.memset(ones_rb, 1.0)
            ones32b = pp.tile([1, HD], bf16, tag="ones32b")
            nc.vector.memset(ones32b, 1.0)
            ones512b = pp.tile([1, 512], bf16, tag="ones512b")
            nc.vector.memset(ones512b, 1.0)
            eps128 = pp.tile([D, 1], f32, tag="eps128")
            nc.vector.memset(eps128, EPS)
            eps1 = pp.tile([1, 1], f32, tag="eps1")
            nc.vector.memset(eps1, EPS)

            # ---- persistent activations (bf16) ----
            xT = big.tile([D, T], bf16, tag="chA")
            qT = big.tile([D, T], bf16, tag="chE")
            kTp = big.tile([D, bloc, H, NG, KTILE], bf16, tag="chB")
            v_sb = big.tile([D, T // KTILE, H, HD + 1], bf16, tag="chC")
            attnT = big.tile([D, T], bf16, tag="chD")
            x1T = big.tile([D, T], bf16, tag="chG")
            nc.vector.memset(v_sb[:, :, :, HD:HD + 1], 1.0)

            # ================= embedding =================
            psA_cm = tc.tile_pool(name="psA", bufs=6, space="PSUM")
            ps_sm = psA_cm.__enter__()
            for e in range(NET):
                sl = slice(e * 512, (e + 1) * 512)
                xb = wkt(NCAT)
                nc.scalar.dma_start(out=xb, in_=bc(Xf[sl], NCAT))
                sb_ = wkt(NCAT)
                nc.scalar.dma_start(out=sb_, in_=bc(saf[sl], NCAT))
                oh = wk.tile([NCAT, 512], bf16, tag="wk", name="oh")
                nc.vector.tensor_scalar(out=oh, in0=xb, scalar1=io25,
                                        scalar2=None, op0=OP.is_equal)
                nc.vector.tensor_mul(oh, oh, sb_)
                pb = wkt(10)
                nc.scalar.dma_start(out=pb, in_=bc(ptmf[sl], 10))
                ohp = wk.tile([10, 512], bf16, tag="wk", name="ohp")
                nc.vector.tensor_scalar(out=ohp, in0=pb, scalar1=io10,
                                        scalar2=None, op0=OP.is_equal)
                pe = ps_sm.tile([D, 512], f32, tag="sm")
                nc.tensor.matmul(pe, pemb_b, ohp, start=True, stop=False)
                nc.tensor.matmul(pe, emb_b, oh, start=False, stop=True)
                nc.vector.tensor_copy(xT[:, sl], pe)

            if dbg:
                nc.sync.dma_start(out=dbg_outs["dbg_xT"][:, :], in_=xT)

            # ================= qkv =================
            for e in range(NET):
                sl = slice(e * 512, (e + 1) * 512)
                pq = ps_sm.tile([D, 512], f32, tag="sm")
                nc.tensor.matmul(pq, inwT_b[:, 0:D], xT[:, sl],
                                 start=True, stop=True)
                nc.vector.tensor_scalar(out=qT[:, sl], in0=pq, scalar1=ISQ,
                                        scalar2=qb_s, op0=OP.mult, op1=OP.add)
                pk = ps_sm.tile([D, 512], f32, tag="sm")
                nc.tensor.matmul(pk, inwT_b[:, D:2 * D], xT[:, sl],
                                 start=True, stop=True)
                ktmp = mid.tile([D, 512], bf16, tag="mid", name="ktmp")
                nc.vector.tensor_scalar(out=ktmp, in0=pk, scalar1=kb_s,
                                        scalar2=None, op0=OP.add)
                b_ = (e * 512) // lp
                for h in range(H):
                    for sub in range(4):
                        ktb = ((e * 512) % lp) // KTILE + sub
                        _q = nc.gpsimd if (h + sub) % 2 == 0 else nc.sync
                        _q.dma_start(
                            out=kTp[32 * (ktb % 4):32 * (ktb % 4) + 32,
                                    b_, h, ktb // 4, :],
                            in_=ktmp[32 * h:32 * h + 32,
                                     sub * KTILE:(sub + 1) * KTILE])
                for sub in range(4):
                    tt = (e * 512) // KTILE + sub
                    pv = ps_sm.tile([KTILE, D], f32, tag="sm")
                    nc.tensor.matmul(pv, xT[:, e * 512 + sub * KTILE:
                                            e * 512 + (sub + 1) * KTILE],
                                     inwT_b[:, 2 * D:3 * D], start=True, stop=True)
                    nc.vector.tensor_copy(
                        v_sb[:, tt, :, 0:HD],
                        pv.rearrange("p (h d) -> p h d", h=H))

            if dbg:
                nc.sync.dma_start(out=dbg_outs["dbg_qT"][:, :], in_=qT)
                kT_dbg = big.tile([D, T], bf16, tag="chDBG")
                for b_ in range(bloc):
                    for h in range(H):
                        for g in range(NG):
                            for r in range(4):
                                kt = 4 * g + r
                                nc.sync.dma_start(
                                    out=kT_dbg[32 * h:32 * h + 32,
                                               b_ * lp + kt * KTILE:
                                               b_ * lp + (kt + 1) * KTILE],
                                    in_=kTp[32 * r:32 * r + 32, b_, h, g, :])
                nc.sync.dma_start(out=dbg_outs["dbg_kT"][:, :], in_=kT_dbg)

            psA_cm.__exit__(None, None, None)

            # ================= attention =================
            # 2 key-tiles per subgroup so sc fits 2 PSUM banks and can be
            # triple-buffered: QK of subgroup p+2 overlaps exp of p,
            # PV of p overlaps exp of p+1. Band bias applied post-exp as a
            # bf16 multiply on pt (off the QK->exp critical path).
            psB1_cm = tc.tile_pool(name="ps_sc", bufs=3, space="PSUM")
            ps_sc = psB1_cm.__enter__()
            psB2_cm = tc.tile_pool(name="ps_pv", bufs=2, space="PSUM")
            ps_pv = psB2_cm.__enter__()
            den32 = longs.tile([NDEN, 512], bf16, tag="den32")
            for b_ in range(bloc):
                for h in range(H):
                    qrep = qrp.tile([D, lp], bf16, tag="qr")
                    for r in range(4):
                        nc.gpsimd.dma_start(
                            out=qrep[32 * r:32 * r + 32, :],
                            in_=qT[32 * h:32 * h + 32, b_ * lp:(b_ + 1) * lp])
                    for qt in range(NQT):
                        ppv = ps_pv.tile([HD + 1, 512], f32, tag="pv")
                        for p_ in range(2 * NG):
                            sc = ps_sc.tile([D, 2 * 512], f32, tag="sc")
                            for j in range(2):
                                kt = 2 * p_ + j
                                nc.tensor.matmul(
                                    sc[:, j * 512:(j + 1) * 512],
                                    kTp[32 * (kt % 4):32 * (kt % 4) + 32,
                                        b_, h, kt // 4, :],
                                    qrep[32 * (kt % 4):32 * (kt % 4) + 32,
                                         qt * QT:(qt + 1) * QT],
                                    start=True, stop=True,
                                    tile_position=(32 * (kt % 4), 0))
                            side = 0 if (p_ >> 1) <= qt else 1
                            banded = any(-1 <= (2 * p_ + j) - 4 * qt <= 4
                                         for j in range(2))
                            # offload the last two unbanded subgroups of each
                            # qt to DVE via exp(u) ~= (1 + u/2)^2 (|u| tiny)
                            unb = [pp2 for pp2 in range(2 * NG)
                                   if not any(-1 <= (2 * pp2 + j) - 4 * qt <= 4
                                              for j in range(2))]
                            on_dve = p_ in unb[-2:]
                            pt = ptp.tile([D, 2 * 512], bf16, tag="pt")
                            if on_dve:
                                th = ptp.tile([D, 2 * 512], bf16, tag="pt",
                                              name="th")
                                nc.vector.tensor_scalar(
                                    out=th, in0=sc, scalar1=0.5,
                                    scalar2=cexp2[:, 2 * h + side:
                                                  2 * h + side + 1],
                                    op0=OP.mult, op1=OP.add)
                                nc.vector.tensor_mul(pt, th, th)
                            else:
                                nc.scalar.activation(pt, sc, AF.Exp,
                                                     bias=cexp[:, 2 * h + side:
                                                               2 * h + side + 1],
                                                     scale=1.0)
                            for j in range(2):
                                kt = 2 * p_ + j
                                dd = kt - 4 * qt
                                if -1 <= dd <= 4:
                                    di = dd + 1
                                    c0, w = BAND_C0[di], BAND_W[di]
                                    nc.vector.tensor_tensor(
                                        out=pt[:, j * 512 + c0:j * 512 + c0 + w],
                                        in0=pt[:, j * 512 + c0:j * 512 + c0 + w],
                                        in1=bandexp[:, h * BAND_TOT + BAND_OFF[di]:
                                                    h * BAND_TOT + BAND_OFF[di] + w],
                                        op=OP.mult)
                            for j in range(2):
                                kt = 2 * p_ + j
                                nc.tensor.matmul(
                                    ppv, v_sb[:, b_ * NKT + kt, h, :],
                                    pt[:, j * 512:(j + 1) * 512],
                                    start=(kt == 0), stop=(kt == NKT - 1))
                        pv_sb = wk.tile([HD + 1, 512], bf16, tag="wk", name="pv_sb")
                        nc.vector.tensor_copy(pv_sb, ppv)
                        nc.gpsimd.dma_start(
                            out=attnT[32 * h:32 * h + 32,
                                      b_ * lp + qt * QT:b_ * lp + (qt + 1) * QT],
                            in_=pv_sb[0:HD, :])
                        nc.gpsimd.dma_start(
                            out=den32[(b_ * NQT + qt) * H + h:
                                      (b_ * NQT + qt) * H + h + 1, :],
                            in_=pv_sb[HD:HD + 1, :])

            psB2_cm.__exit__(None, None, None)
            psB1_cm.__exit__(None, None, None)
            psC_cm = tc.tile_pool(name="psC", bufs=6, space="PSUM")
            ps_sm = psC_cm.__enter__()

            sync_in = dram.tile([1, 1], f32, tag="synci")
            sync_out = dram.tile([1, 1], f32, tag="synco")
            syncs = wk.tile([1, 1], f32, tag="wk", name="syncs")
            nc.vector.memset(syncs, 0.0)
            nc.gpsimd.dma_start(out=sync_in, in_=syncs)
            nc.gpsimd.collective_compute(
                "AllReduce", OP.add, replica_groups=[list(range(n_cores))],
                ins=[sync_in[:, :].opt()], outs=[sync_out[:, :].opt()])

            lnden = midt(NDEN, 512)
            nc.scalar.activation(lnden, den32, AF.Ln, bias=0.0, scale=1.0)
            recip = longs.tile([NDEN, 512], bf16, tag="recip")
            nc.scalar.activation(recip, lnden, AF.Exp, bias=0.0, scale=-1.0)

            for b_ in range(bloc):
                for qt in range(NQT):
                    bcp = ps_sm.tile([D, 512], f32, tag="sm")
                    for h in range(H):
                        rr = rowp.tile([1, 512], bf16, tag="row")
                        nc.gpsimd.dma_start(
                            out=rr, in_=recip[(b_ * NQT + qt) * H + h:
                                              (b_ * NQT + qt) * H + h + 1, :])
                        nc.tensor.matmul(bcp[32 * h:32 * h + 32, :], ones32b, rr,
                                         start=True, stop=True,
                                         tile_position=(0, 32 * h))
                    sl = slice(b_ * lp + qt * QT, b_ * lp + (qt + 1) * QT)
                    nc.vector.tensor_mul(attnT[:, sl], attnT[:, sl], bcp)

            if dbg:
                nc.sync.dma_start(out=dbg_outs["dbg_attnT"][:, :], in_=attnT)

            # ======== layernorm helper: per-tile stats packed into wide
            # [1, NLT*512] rows -> one Ln + one Exp per call (one table
            # switch each); gamma folded into the rank-1 broadcast
            # (lhsT = g row), -beta accumulated via a ones-row matmul ========
            def layernorm_T(src, dst_fn, g_row, nb_row, b_base):
                m_all = lnr.tile([1, NLT * 512], bf16, tag="mall", name="m_all")
                v_all = lnr.tile([1, NLT * 512], f32, tag="vall", name="v_all")
                for t_ in range(NLT):
                    sl = slice(b_base * lp + t_ * 512, b_base * lp + (t_ + 1) * 512)
                    rsl = slice(t_ * 512, (t_ + 1) * 512)
                    sq = wk.tile([D, 512], bf16, tag="wk", name="sq")
                    nc.vector.tensor_mul(sq, src[:, sl], src[:, sl])
                    p1_ = ps_sm.tile([1, 512], f32, tag="sm")
                    nc.tensor.matmul(p1_, ones128b, src[:, sl], start=True, stop=True)
                    p2_ = ps_sm.tile([1, 512], f32, tag="sm")
                    nc.tensor.matmul(p2_, ones128b, sq, start=True, stop=True)
                    nc.vector.tensor_scalar(out=m_all[:, rsl], in0=p1_,
                                            scalar1=1.0 / D,
                                            scalar2=None, op0=OP.mult)
                    nc.vector.tensor_scalar(out=v_all[:, rsl], in0=p2_,
                                            scalar1=1.0 / D,
                                            scalar2=None, op0=OP.mult)
                    msq = rowp.tile([1, 512], f32, tag="row", name="msq")
                    nc.vector.tensor_mul(msq, m_all[:, rsl], m_all[:, rsl])
                    nc.vector.tensor_tensor(out=v_all[:, rsl], in0=v_all[:, rsl],
                                            in1=msq, op=OP.subtract)
                lnv = lnr.tile([1, NLT * 512], f32, tag="lall", name="lnv")
                nc.scalar.activation(lnv, v_all, AF.Ln, bias=eps1, scale=1.0)
                rr_all = lnr.tile([1, NLT * 512], bf16, tag="rall", name="rr_all")
                nc.scalar.activation(rr_all, lnv, AF.Exp, bias=0.0, scale=-0.5)
                rm_all = lnr.tile([1, NLT * 512], bf16, tag="rmall", name="rm_all")
                nc.vector.tensor_mul(rm_all, m_all, rr_all)
                for t_ in range(NLT):
                    sl = slice(b_base * lp + t_ * 512, b_base * lp + (t_ + 1) * 512)
                    rsl = slice(t_ * 512, (t_ + 1) * 512)
                    br = ps_sm.tile([D, 512], f32, tag="sm")
                    nc.tensor.matmul(br, g_row, rr_all[:, rsl], start=True, stop=True)
                    bm = ps_sm.tile([D, 512], f32, tag="sm")
                    nc.tensor.matmul(bm, g_row, rm_all[:, rsl], start=True, stop=False)
                    nc.tensor.matmul(bm, nb_row, ones512b, start=False, stop=True)
                    tmp = wk.tile([D, 512], f32, tag="wk", name="lntmp")
                    nc.vector.tensor_mul(tmp, src[:, sl], br)
                    nc.vector.tensor_tensor(out=dst_fn(t_), in0=tmp, in1=bm,
                                            op=OP.subtract)

            # ================= out-proj + residual + LN1 =================
            r1T = big.tile([D, T], bf16, tag="chB")   # after kTp's last read
            for b_ in range(bloc):
                for qt in range(NQT):
                    sl = slice(b_ * lp + qt * QT, b_ * lp + (qt + 1) * QT)
                    po = ps_sm.tile([D, 512], f32, tag="sm")
                    nc.tensor.matmul(po, woT_b, attnT[:, sl], start=True, stop=False)
                    nc.tensor.matmul(po, ob_r, ones512b, start=False, stop=True)
                    nc.vector.tensor_tensor(out=r1T[:, sl], in0=po,
                                            in1=xT[:, sl], op=OP.add)
            for b_ in range(bloc):
                layernorm_T(
                    r1T,
                    lambda t_, b0=b_: x1T[:, b0 * lp + t_ * 512:
                                          b0 * lp + (t_ + 1) * 512],
                    ln1g_r, ln1nb_r, b_)

            if dbg:
                nc.sync.dma_start(out=dbg_outs["dbg_x1T"][:, :], in_=x1T)

            # ================= FFN + residual + LN2 =================
            x2pad = big.tile([D, bloc * (lp + 4)], bf16, tag="chA")  # after xT
            nc.vector.memset(x2pad[:, :], 0.0)
            r2T = big.tile([D, T], bf16, tag="chC")                  # after v_sb
            for b_ in range(bloc):
                for qt in range(NQT):
                    sl = slice(b_ * lp + qt * QT, b_ * lp + (qt + 1) * QT)
                    h1a = wk.tile([D, 512], bf16, tag="wk", name="h1a")
                    h1b = wk.tile([D, 512], bf16, tag="wk", name="h1b")
                    for half, dest in ((0, h1a), (1, h1b)):
                        ph = ps_sm.tile([D, 512], f32, tag="sm")
                        nc.tensor.matmul(ph, l1wT_b[:, half * D:(half + 1) * D],
                                         x1T[:, sl], start=True, stop=True)
                        nc.scalar.activation(dest, ph, AF.Relu,
                                             bias=l1b_s[:, half:half + 1],
                                             scale=1.0)
                    py = ps_sm.tile([D, 512], f32, tag="sm")
                    nc.tensor.matmul(py, l2cat_b[:, 0:D], h1a, start=True, stop=False)
                    nc.tensor.matmul(py, l2cat_b[:, D:2 * D], h1b,
                                     start=False, stop=False)
                    nc.tensor.matmul(py, l2b_r, ones512b, start=False, stop=True)
                    nc.vector.tensor_tensor(out=r2T[:, sl], in0=py,
                                            in1=x1T[:, sl], op=OP.add)
            for b_ in range(bloc):
                layernorm_T(
                    r2T,
                    lambda t_, b0=b_: x2pad[:, b0 * (lp + 4) + 2 + t_ * 512:
                                            b0 * (lp + 4) + 2 + (t_ + 1) * 512],
                    ln2g_r, ln2nb_r, b_)

            if dbg:
                for b_ in range(bloc):
                    nc.sync.dma_start(
                        out=dbg_outs["dbg_x2"][:, b_ * lp:(b_ + 1) * lp],
                        in_=x2pad[:, b_ * (lp + 4) + 2:b_ * (lp + 4) + 2 + lp])

            sync2_in = dram.tile([1, 1], f32, tag="sync2i")
            sync2_out = dram.tile([1, 1], f32, tag="sync2o")
            nc.gpsimd.dma_start(out=sync2_in, in_=syncs)
            nc.gpsimd.collective_compute(
                "AllReduce", OP.add, replica_groups=[list(range(n_cores))],
                ins=[sync2_in[:, :].opt()], outs=[sync2_out[:, :].opt()])

            # ================= conv1 + bn1 =================
            c1_sb = big.tile([C1, bloc * lp], bf16, tag="chD")   # after attnT
            bnst1 = longs.tile([C1, bloc * NLT, 6], f32, tag="bnst1")
            for b_ in range(bloc):
                for t_ in range(NLT):
                    pc = ps_sm.tile([C1, 512], f32, tag="sm")
                    for k_ in range(K):
                        nc.tensor.matmul(
                            pc, c1wT_b[:, k_ * C1:(k_ + 1) * C1],
                            x2pad[:, b_ * (lp + 4) + t_ * 512 + k_:
                                  b_ * (lp + 4) + t_ * 512 + k_ + 512],
                            start=(k_ == 0), stop=(k_ == K - 1))
                    nc.vector.bn_stats(out=bnst1[:, b_ * NLT + t_, :], in_=pc)
                    nc.vector.tensor_copy(
                        c1_sb[:, b_ * lp + t_ * 512:b_ * lp + (t_ + 1) * 512], pc)
            mv1 = wk.tile([C1, 2], f32, tag="wk")
            nc.vector.bn_aggr(out=mv1, in_=bnst1)
            part1 = wk.tile([C1, 2], f32, tag="wk")
            sqm = wk.tile([C1, 1], f32, tag="wk")
            nc.vector.tensor_mul(sqm, mv1[:, 0:1], mv1[:, 0:1])
            nc.vector.tensor_tensor(out=sqm, in0=sqm, in1=mv1[:, 1:2], op=OP.add)
            nl_ = float(bloc * lp)
            nc.vector.tensor_scalar(out=part1[:, 0:1], in0=mv1[:, 0:1],
                                    scalar1=nl_, scalar2=None, op0=OP.mult)
            nc.vector.tensor_scalar(out=part1[:, 1:2], in0=sqm,
                                    scalar1=nl_, scalar2=None, op0=OP.mult)
            bn1_in = dram.tile([C1, 2], f32, tag="bn1i")
            bn1_out = dram.tile([C1, 2], f32, tag="bn1o")
            nc.sync.dma_start(out=bn1_in, in_=part1)
            nc.gpsimd.collective_compute(
                "AllReduce", OP.add, replica_groups=[list(range(n_cores))],
                ins=[bn1_in[:, :].opt()], outs=[bn1_out[:, :].opt()])
            glob1 = wk.tile([C1, 2], f32, tag="wk")
            nc.sync.dma_start(out=glob1, in_=bn1_out)

            def bn_scale_shift(globc, n_, g_ap, b_ap):
                mean = wk.tile([C1, 1], f32, tag="wk")
                nc.vector.tensor_scalar(out=mean, in0=globc[:, 0:1],
                                        scalar1=1.0 / n_, scalar2=None, op0=OP.mult)
                ex2 = wk.tile([C1, 1], f32, tag="wk")
                nc.vector.tensor_scalar(out=ex2, in0=globc[:, 1:2],
                                        scalar1=1.0 / n_, scalar2=None, op0=OP.mult)
                msq_ = wk.tile([C1, 1], f32, tag="wk")
                nc.vector.tensor_mul(msq_, mean, mean)
                nc.vector.tensor_tensor(out=ex2, in0=ex2, in1=msq_, op=OP.subtract)
                lnv_ = wk.tile([C1, 1], f32, tag="wk")
                nc.scalar.activation(lnv_, ex2, AF.Ln, bias=eps128, scale=1.0)
                rstd_ = wk.tile([C1, 1], f32, tag="wk")
                nc.scalar.activation(rstd_, lnv_, AF.Exp, bias=0.0, scale=-0.5)
                scale = longs.tile([C1, 1], f32, tag="bnsc")
                nc.vector.tensor_mul(scale, rstd_, g_ap)
                shift = longs.tile([C1, 1], f32, tag="bnsh")
                nc.vector.tensor_mul(shift, mean, scale)
                nc.vector.tensor_tensor(out=shift, in0=b_ap, in1=shift,
                                        op=OP.subtract)
                return scale, shift

            sc1, sh1 = bn_scale_shift(glob1, n1, bn1g, bn1b)
            p1_sb = big.tile([C1, bloc * LP2], bf16, tag="chF")   # after band
            if dbg:
                nc.sync.dma_start(out=dbg_outs["dbg_c1"][:, :], in_=c1_sb)
            for b_ in range(bloc):
                for t_ in range(NLT):
                    rel = wk.tile([D, 512], bf16, tag="wk", name="rel")
                    nc.scalar.activation(
                        rel, c1_sb[:, b_ * lp + t_ * 512:b_ * lp + (t_ + 1) * 512],
                        AF.Relu, bias=sh1, scale=sc1)
                    rel2 = rel.rearrange("p (l two) -> p l two", two=2)
                    nc.vector.tensor_tensor(
                        out=p1_sb[:, b_ * LP2 + t_ * 256:b_ * LP2 + (t_ + 1) * 256],
                        in0=rel2[:, :, 0], in1=rel2[:, :, 1], op=OP.max)
            if dbg:
                nc.sync.dma_start(out=dbg_outs["dbg_p1"][:, :], in_=p1_sb)

            # ================= conv2 + bn2 =================
            c2_sb = big.tile([C1, bloc * 2 * L2], bf16, tag="chE")  # after qT
            bnst2 = longs.tile([C1, 2, bloc * 2, 6], f32, tag="bnst2")
            for b_ in range(bloc):
                for half in range(2):
                    for t_ in range(2):
                        pc = ps_sm.tile([C1, LT2], f32, tag="sm")
                        for k_ in range(K):
                            nc.tensor.matmul(
                                pc, c2wT_b[:, k_ * C2 + half * C1:
                                           k_ * C2 + (half + 1) * C1],
                                p1_sb[:, b_ * LP2 + t_ * LT2 + k_:
                                      b_ * LP2 + t_ * LT2 + k_ + LT2],
                                start=(k_ == 0), stop=(k_ == K - 1))
                        nc.vector.bn_stats(out=bnst2[:, half, b_ * 2 + t_, :],
                                           in_=pc)
                        nc.vector.tensor_copy(
                            c2_sb[:, (b_ * 2 + half) * L2 + t_ * LT2:
                                  (b_ * 2 + half) * L2 + (t_ + 1) * LT2], pc)
            part2 = longs.tile([C1, 4], f32, tag="part2")
            for half in range(2):
                mv2 = wk.tile([C1, 2], f32, tag="wk")
                nc.vector.bn_aggr(out=mv2, in_=bnst2[:, half, :, :])
                sqm2 = wk.tile([C1, 1], f32, tag="wk")
                nc.vector.tensor_mul(sqm2, mv2[:, 0:1], mv2[:, 0:1])
                nc.vector.tensor_tensor(out=sqm2, in0=sqm2, in1=mv2[:, 1:2],
                                        op=OP.add)
                nl2 = float(bloc * L2)
                nc.vector.tensor_scalar(out=part2[:, 2 * half:2 * half + 1],
                                        in0=mv2[:, 0:1], scalar1=nl2,
                                        scalar2=None, op0=OP.mult)
                nc.vector.tensor_scalar(out=part2[:, 2 * half + 1:2 * half + 2],
                                        in0=sqm2, scalar1=nl2,
                                        scalar2=None, op0=OP.mult)
            bn2_in = dram.tile([C1, 4], f32, tag="bn2i")
            bn2_out = dram.tile([C1, 4], f32, tag="bn2o")
            nc.sync.dma_start(out=bn2_in, in_=part2)
            nc.gpsimd.collective_compute(
                "AllReduce", OP.add, replica_groups=[list(range(n_cores))],
                ins=[bn2_in[:, :].opt()], outs=[bn2_out[:, :].opt()])
            glob2 = longs.tile([C1, 4], f32, tag="glob2")
            nc.sync.dma_start(out=glob2, in_=bn2_out)

            feat = longs.tile([C1, 2 * bloc], f32, tag="feat")
            # vectorized bn2 scale/shift for both halves: [C1, 2]
            g2v = glob2.rearrange("p (h two) -> p h two", two=2)
            mean2 = wk.tile([C1, 2], f32, tag="wk")
            nc.vector.tensor_scalar(out=mean2, in0=g2v[:, :, 0], scalar1=1.0 / n2,
                                    scalar2=None, op0=OP.mult)
            ex22 = wk.tile([C1, 2], f32, tag="wk")
            nc.vector.tensor_scalar(out=ex22, in0=g2v[:, :, 1], scalar1=1.0 / n2,
                                    scalar2=None, op0=OP.mult)
            msq2 = wk.tile([C1, 2], f32, tag="wk")
            nc.vector.tensor_mul(msq2, mean2, mean2)
            nc.vector.tensor_tensor(out=ex22, in0=ex22, in1=msq2, op=OP.subtract)
            lnv2 = wk.tile([C1, 2], f32, tag="wk")
            nc.scalar.activation(lnv2, ex22, AF.Ln, bias=eps128, scale=1.0)
            rstd2 = wk.tile([C1, 2], f32, tag="wk")
            nc.scalar.activation(rstd2, lnv2, AF.Exp, bias=0.0, scale=-0.5)
            scale2 = longs.tile([C1, 2], f32, tag="bnsc2")
            nc.vector.tensor_mul(scale2, rstd2, bn2g)
            shift2 = longs.tile([C1, 2], f32, tag="bnsh2")
            nc.vector.tensor_mul(shift2, mean2, scale2)
            nc.vector.tensor_tensor(out=shift2, in0=bn2b, in1=shift2,
                                    op=OP.subtract)
            for half in range(2):
                sc2 = scale2[:, half:half + 1]
                sh2 = shift2[:, half:half + 1]
                for b_ in range(bloc):
                    rel = wkb.tile([C1, L2], bf16, tag="wkb")
                    nc.scalar.activation(
                        rel, c2_sb[:, (b_ * 2 + half) * L2:
                                   (b_ * 2 + half + 1) * L2],
                        AF.Relu, bias=sh2, scale=sc2)
                    nc.vector.reduce_max(
                        out=feat[:, (b_ * 2 + half):(b_ * 2 + half) + 1],
                        in_=rel, axis=AX.X)
            if dbg:
                nc.sync.dma_start(out=dbg_outs["dbg_feat"][:, :], in_=feat)

            # ================= fc =================
            for b_ in range(bloc):
                pf = ps_sm.tile([NC, 1], f32, tag="sm")
                for half in range(2):
                    nc.tensor.matmul(pf, fcwT[:, half * NC:(half + 1) * NC],
                                     feat[:, b_ * 2 + half:b_ * 2 + half + 1],
                                     start=(half == 0), stop=(half == 1))
                ob2 = wk.tile([NC, 1], f32, tag="wk")
                nc.vector.tensor_scalar(out=ob2, in0=pf, scalar1=fcb_s,
                                        scalar2=None, op0=OP.add)
                nc.sync.dma_start(out=out_d[b_, :], in_=ob2[:, 0])
            psC_cm.__exit__(None, None, None)

    nc.compile()
    return nc


def _host_inputs(inputs, n_cores, bloc, lp):
    X = np.asarray(inputs["X"]).astype(np.float32)[:, :lp]
    sa = np.asarray(inputs["surface_availability"], dtype=np.float32)[:, :lp]
    ptm = np.asarray(inputs["ptm"]).astype(np.float32)[:, :lp]
    emb = np.asarray(inputs["emb"], dtype=np.float32)
    pemb = np.asarray(inputs["ptm_emb"], dtype=np.float32)
    rpe = np.asarray(inputs["rpe"], dtype=np.float32)
    inw = np.asarray(inputs["in_proj_w"], dtype=np.float32)
    inb = np.asarray(inputs["in_proj_b"], dtype=np.float32)
    wo = np.asarray(inputs["out_proj_w"], dtype=np.float32)
    bo = np.asarray(inputs["out_proj_b"], dtype=np.float32)
    w1 = np.asarray(inputs["lin1_w"], dtype=np.float32)
    b1 = np.asarray(inputs["lin1_b"], dtype=np.float32)
    w2 = np.asarray(inputs["lin2_w"], dtype=np.float32)
    b2 = np.asarray(inputs["lin2_b"], dtype=np.float32)
    c1w = np.asarray(inputs["conv1_w"], dtype=np.float32)
    c2w = np.asarray(inputs["conv2_w"], dtype=np.float32)
    fcw = np.asarray(inputs["fc_w"], dtype=np.float32)

    pembp = np.zeros((10, D), np.float32)
    pembp[:, ED:] = pemb
    embp = np.zeros((NCAT, D), np.float32)
    embp[:, :ED] = emb

    clo, chi = rpe[0], rpe[2 * MD]
    bandcat = np.zeros((D, H * BAND_TOT), np.float32)
    jj = np.arange(128)[:, None]
    for h in range(H):
        for di, dl in enumerate(BAND_DELTAS):
            w = BAND_W[di]
            ii = np.arange(BAND_C0[di], BAND_C0[di] + w)[None, :]
            e = dl + jj - ii
            val = rpe[np.clip(e, -MD, MD) + MD, h]
            beta = chi[h] if di == 5 else clo[h]
            bandcat[:, h * BAND_TOT + BAND_OFF[di]:
                    h * BAND_TOT + BAND_OFF[di] + w] = val - beta
    cexp = np.zeros((D, 2 * H), np.float32)
    for h in range(H):
        cexp[:, 2 * h] = clo[h]
        cexp[:, 2 * h + 1] = chi[h]

    ob_eff = bo + wo @ inb[2 * D:3 * D]
    l2t = w2.T   # [FF, D]
    shared = {
        "embp": embp, "pembp": pembp,
        "iota25": np.arange(NCAT, dtype=np.float32)[:, None],
        "iota10": np.arange(10, dtype=np.float32)[:, None],
        "inwT": np.ascontiguousarray(inw.T),
        "qb": (inb[0:D] * ISQ)[:, None],
        "kb": inb[D:2 * D][:, None],
        "woT": np.ascontiguousarray(wo.T),
        "ob_eff": ob_eff[:, None].astype(np.float32),
        "l1wT": np.ascontiguousarray(w1.T),
        "l1b": b1.reshape(2, D).T.copy(),
        "l2cat": np.concatenate([l2t[0:D], l2t[D:2 * D]], axis=1).copy(),
        "l2b": b2[:, None],
        "ln1g": np.asarray(inputs["ln1_g"], np.float32)[:, None],
        "ln1b": np.asarray(inputs["ln1_b"], np.float32)[:, None],
        "ln2g": np.asarray(inputs["ln2_g"], np.float32)[:, None],
        "ln2b": np.asarray(inputs["ln2_b"], np.float32)[:, None],
        "bandcat": bandcat, "cexp": cexp,
        "c1wT": np.ascontiguousarray(c1w.transpose(1, 2, 0).reshape(D, K * C1)),
        "c2wT": np.ascontiguousarray(c2w.transpose(1, 2, 0).reshape(C1, K * C2)),
        "bn1g": np.asarray(inputs["bn1_g"], np.float32)[:, None],
        "bn1b": np.asarray(inputs["bn1_b"], np.float32)[:, None],
        "bn2g": np.asarray(inputs["bn2_g"], np.float32).reshape(2, C1).T.copy(),
        "bn2b": np.asarray(inputs["bn2_b"], np.float32).reshape(2, C1).T.copy(),
        "fcwT": np.ascontiguousarray(
            fcw.T.reshape(2, C1, NC).transpose(1, 0, 2).reshape(C1, 2 * NC)),
        "fcb": np.asarray(inputs["fc_b"], np.float32)[:, None],
    }
    in_maps = []
    for c in range(n_cores):
        rows = slice(c * bloc, (c + 1) * bloc)
        m = dict(shared)
        m["Xf"] = np.ascontiguousarray(X[rows].reshape(-1))
        m["saf"] = np.ascontiguousarray(sa[rows].reshape(-1))
        m["ptmf"] = np.ascontiguousarray(ptm[rows].reshape(-1))
        in_maps.append(m)
    return in_maps


_NC_CACHE = {}


def _get_nc(n_cores, bloc, lp, dbg=False):
    key = (n_cores, bloc, lp, dbg)
    if key not in _NC_CACHE:
        _NC_CACHE[key] = _build(n_cores, bloc, lp, dbg=dbg)
    return _NC_CACHE[key]


def kernel(**inputs):
    from concourse.bass_utils import run_bass_kernel_spmd
    nc = _get_nc(NCORES, BLOC, L)
    in_maps = _host_inputs(inputs, NCORES, BLOC, L)
    res = run_bass_kernel_spmd(nc, in_maps, list(range(NCORES)))
    out = np.concatenate([res.results[i]["out"] for i in range(NCORES)], axis=0)
    return out.astype(np.float32)
